# revision 6
# baseline (speedup 1.0000x reference)
"""Dual-GAT (nn_GAT_48017734369678) on 8 TRN2 NeuronCores via Bass/Tile.

Self-contained: host-side sharding/preprocessing in numpy, device program in
Bass (Tile), executed through run_bass_kernel_spmd on cores 0-7.

Sharding: data-parallel over destination nodes (6250/core). Upload volume is
the bottleneck (axon tunnel ~50MB/s), so each core uploads only its own
transposed x1 shard / Ab rows / x2 rows; full copies are assembled on-device
with AllGather over NeuronLink. Gather-index tables are uploaded compact
([16, n/16] wrap) and replicated to the 128-partition layout on device.

Edge aggregation: per-node gather tables in DRAM + dma_gather by src, one-hot
mask matmuls (fp32r) accumulating (numer | softmax-denominator) in PSUM.
Group graph replicated on every core. Identities used:
  exp(LeakyReLU(al+ar)) == max(exp(al)exp(ar), exp(.2al)exp(.2ar))
  segment softmax is shift-invariant (edge scores are O(10): no max needed)
  (A+I)[gidx] row gather folds the group-attention self term exactly.
"""
import sys

sys.path.insert(0, "/opt/trn_rl_repo")

import numpy as np

N, G = 50000, 1024
F_IN, HID, HEADS, NCLS = 128, 32, 4, 32
LN_EPS = 1e-5
NCORES = 8
NPER = N // NCORES            # 6250
NT = (NPER + 127) // 128      # 49 own tiles (last tile 106 rows)
SPLIT = 32768                 # int16 gather split
P = 128
SENT = 255.0                  # pad-edge dstlocal sentinel (mask never matches)
TAB1_COLS = 192               # [h(128) | u(4) | u2(4) | junk]  (768B rows)
TAB2_COLS = 64                # [h2(32) | u(1) | u2(1) | junk]  (256B rows)
VT_COLS = 64                  # [v(H) | v2(H) | junk]           (256B rows)
MAXCH = 12                    # gather chunk (blocks) for 192-col tables


# --------------------------------------------------------------------------
# host-side preprocessing
# --------------------------------------------------------------------------

def _wrap16(ix):
    """Compact dma_gather idx layout: [16, n/16]; idx i at [i%16, i//16].
    Replication to the 8 groups of 16 partitions happens on device."""
    ix = np.asarray(ix, np.int64)
    n = len(ix)
    assert n % 16 == 0, n
    return np.ascontiguousarray(ix.reshape(n // 16, 16).T.astype(np.int16))


def _segments(src, dst, ntile, dst_base, split):
    dstl = dst - dst_base
    tile = dstl // 128
    segs = []
    for t in range(ntile):
        m = tile == t
        s, d = src[m], dstl[m] - t * 128
        if split:
            lo = s < SPLIT
            segs.append((s[lo], d[lo], s[~lo], d[~lo]))
        else:
            segs.append((s, d, s[:0], d[:0]))
    return segs


def _flatten(segs, nblk_lo, nblk_hi, ntile):
    nblk = nblk_lo + nblk_hi
    idx_lo, idx_hi, dmod, dglob = [], [], [], []
    for t in range(ntile):
        slo, dlo, shi, dhi = segs[t]
        a = np.zeros(nblk_lo * 128, np.int64); a[:len(slo)] = slo
        b = np.zeros(nblk_hi * 128, np.int64); b[:len(shi)] = shi - SPLIT
        dm = np.full(nblk * 128, SENT, np.float64)
        dm[:len(dlo)] = dlo
        dm[nblk_lo * 128:nblk_lo * 128 + len(dhi)] = dhi
        dg = np.zeros(nblk * 128, np.int64)
        dg[:len(dlo)] = dlo + t * 128
        dg[nblk_lo * 128:nblk_lo * 128 + len(dhi)] = dhi + t * 128
        idx_lo.append(a); idx_hi.append(b); dmod.append(dm); dglob.append(dg)
    idx_lo = np.concatenate(idx_lo) if nblk_lo else np.zeros(0, np.int64)
    idx_hi = np.concatenate(idx_hi) if nblk_hi else np.zeros(0, np.int64)
    dmod = np.concatenate(dmod)
    dglob = np.concatenate(dglob)
    # block layout [128, ntile*nblk]: column t*nblk+b holds block b's dstlocal
    import ml_dtypes
    dmod2 = np.ascontiguousarray(
        dmod.reshape(ntile * nblk, 128).T.astype(ml_dtypes.bfloat16))
    return idx_lo, idx_hi, dmod2, dglob


def _wext(W, a_src, a_dst, b, ncols):
    W = np.asarray(W, np.float32)
    a_src = np.asarray(a_src, np.float32)
    a_dst = np.asarray(a_dst, np.float32)
    b = np.asarray(b, np.float32)
    H, C = a_src.shape
    D = W.shape[1]
    asrc_m = np.zeros((D, H), np.float32)
    adst_m = np.zeros((D, H), np.float32)
    for h in range(H):
        asrc_m[h * C:(h + 1) * C, h] = a_src[h]
        adst_m[h * C:(h + 1) * C, h] = a_dst[h]
    Wx = np.concatenate([W, W @ asrc_m, W @ adst_m], axis=1)
    Wx = np.concatenate(
        [Wx, np.zeros((W.shape[0], ncols - Wx.shape[1]), np.float32)], axis=1)
    brow = np.concatenate([b, b @ asrc_m, b @ adst_m,
                           np.zeros(ncols - D - 2 * H, np.float32)])
    return np.ascontiguousarray(Wx), brow.astype(np.float32)


def host_prep(inputs):
    import ml_dtypes
    f32 = np.float32
    x1 = np.asarray(inputs["x1"], f32)
    ei1 = np.asarray(inputs["edge_index1"], np.int64)
    x2 = np.asarray(inputs["x2"], f32)
    ei2 = np.asarray(inputs["edge_index2"], np.int64)
    gidx = np.asarray(inputs["group_index"], np.int64)

    A = np.zeros((G, G), f32)
    u, v = ei2[0], ei2[1]
    np.add.at(A, (u, v), 1.0)
    np.add.at(A, (v, u), (u != v).astype(f32))
    Ap = A + np.eye(G, dtype=f32)
    assert Ap.max() < 256

    src_g, dst_g = ei1[0], ei1[1]
    core_of = dst_g // NPER
    all_segs = []
    for c in range(NCORES):
        m = core_of == c
        loops = np.arange(c * NPER, (c + 1) * NPER, dtype=np.int64)
        s = np.concatenate([src_g[m], loops])
        d = np.concatenate([dst_g[m], loops])
        all_segs.append(_segments(s, d, NT, c * NPER, True))
    nblk_lo = max(max((len(t[0]) + 127) // 128 for t in sg) for sg in all_segs)
    nblk_hi = max(max((len(t[2]) + 127) // 128 for t in sg) for sg in all_segs)

    loops2 = np.arange(G, dtype=np.int64)
    s2 = np.concatenate([ei2[0], loops2])
    d2 = np.concatenate([ei2[1], loops2])
    sm_segs = _segments(s2, d2, G // 128, 0, False)
    nblk_sm = max((len(t[0]) + 127) // 128 for t in sm_segs)

    meta = dict(nblk_lo=nblk_lo, nblk_hi=nblk_hi, nblk=nblk_lo + nblk_hi,
                nblk_sm=nblk_sm)

    w1a, b1a = _wext(inputs["W1a"], inputs["a1a_src"], inputs["a1a_dst"],
                     inputs["b1a"], 256)
    w1b, b1b = _wext(inputs["W1b"], inputs["a1b_src"], inputs["a1b_dst"],
                     inputs["b1b"], 256)
    w2a, b2a = _wext(inputs["W2a"], inputs["a2a_src"], inputs["a2a_dst"],
                     inputs["b2a"], 64)
    w2b, b2b = _wext(inputs["W2b"], inputs["a2b_src"], inputs["a2b_dst"],
                     inputs["b2b"], 64)

    i_sm, _, dm_sm, dg_sm = _flatten(sm_segs, nblk_sm, 0, G // 128)

    # [b1a(0:256)|b1b(256:512)|b2a(512:576)|b2b(576:640)|
    #  ln1g(640:768)|ln1b(768:896)|ln2g(896:928)|ln2b(928:960)]
    rowcat = np.concatenate([
        b1a, b1b, b2a, b2b,
        np.asarray(inputs["ln1_g"], f32), np.asarray(inputs["ln1_b"], f32),
        np.asarray(inputs["ln2_g"], f32), np.asarray(inputs["ln2_b"], f32)])
    rowcat16 = np.ascontiguousarray(
        np.broadcast_to(rowcat[None, :], (16, rowcat.shape[0])))

    shared = dict(
        wext1a=w1a, wext1b=w1b, wext2a=w2a, wext2b=w2b,
        rowcat=rowcat16,
        idx_smc=_wrap16(i_sm), dstl_sm=dm_sm, dglob_smc=_wrap16(dg_sm),
    )

    per_core = []
    for c in range(NCORES):
        ilo, ihi, dmod, dglob = _flatten(all_segs[c], nblk_lo, nblk_hi, NT)
        gown = np.concatenate([gidx[c * NPER:(c + 1) * NPER],
                               np.zeros(NT * 128 - NPER, np.int64)])
        per_core.append(dict(
            idx_loc=_wrap16(ilo), idx_hic=_wrap16(ihi),
            dstl=dmod, dglobc=_wrap16(dglob), gidxc=_wrap16(gown),
            x1Tsh=np.ascontiguousarray(x1[c * NPER:(c + 1) * NPER].T),
            x2Tsh=np.ascontiguousarray(x2[c * 128:(c + 1) * 128].T),
            Absh=np.asarray(Ap[c * 128:(c + 1) * 128], ml_dtypes.bfloat16),
        ))
    return shared, per_core, meta


# --------------------------------------------------------------------------
# device program
# --------------------------------------------------------------------------

def build_nc(meta):
    import contextlib
    from concourse import bacc, mybir
    from concourse.tile import TileContext

    f32 = mybir.dt.float32
    f32r = mybir.dt.float32r
    bf16 = mybir.dt.bfloat16
    i16 = mybir.dt.int16
    i32 = mybir.dt.int32
    Alu = mybir.AluOpType
    Act = mybir.ActivationFunctionType
    Ax = mybir.AxisListType

    NBLK = meta["nblk"]
    NBLK_LO = meta["nblk_lo"]
    NBLK_HI = meta["nblk_hi"]
    NBLK_SM = meta["nblk_sm"]

    nc = bacc.Bacc(None, target_bir_lowering=False, debug=True)

    dp = lambda n, s, d: nc.declare_dram_parameter(n, list(s), d, isOutput=False)
    x1Tsh_d = dp("x1Tsh", [P, NPER], f32r)
    x2Tsh_d = dp("x2Tsh", [P, P], f32r)
    Absh_d = dp("Absh", [P, G], bf16)
    wext1a_d = dp("wext1a", [P, 256], f32r)
    wext1b_d = dp("wext1b", [P, 256], f32r)
    wext2a_d = dp("wext2a", [P, 64], f32r)
    wext2b_d = dp("wext2b", [P, 64], f32r)
    rowcat_d = dp("rowcat", [16, 960], f32)
    idx_smc_d = dp("idx_smc", [16, 8 * NBLK_SM * 8], i16)
    dstl_sm_d = dp("dstl_sm", [P, 8 * NBLK_SM], bf16)
    dglob_smc_d = dp("dglob_smc", [16, 8 * NBLK_SM * 8], i16)
    idx_loc_d = dp("idx_loc", [16, NT * NBLK_LO * 8], i16)
    idx_hic_d = dp("idx_hic", [16, NT * NBLK_HI * 8], i16)
    dstl_d = dp("dstl", [P, NT * NBLK], bf16)
    dglobc_d = dp("dglobc", [16, NT * NBLK * 8], i16)
    gidxc_d = dp("gidxc", [16, NT * 8], i16)

    out_d = nc.declare_dram_parameter("out", [NPER, NCLS], bf16, isOutput=True)

    # AllGather-assembled full tensors (collectives cannot read IO tensors
    # directly, so shards are staged into internal DRAM first)
    x1st_d = nc.dram_tensor("x1st", [P, NPER], f32r)
    x2st_d = nc.dram_tensor("x2st", [P, P], f32r)
    Abst_d = nc.dram_tensor("Abst", [P, G], bf16)
    x1TG_d = nc.dram_tensor("x1TG", [8 * P, NPER], f32r, addr_space="Shared")
    x2TG_d = nc.dram_tensor("x2TG", [8 * P, P], f32r, addr_space="Shared")
    AbG_d = nc.dram_tensor("AbG", [G, G], bf16, addr_space="Shared")
    # full-layout (8x replicated) gather index tables, built on device
    idx_lo_d = nc.dram_tensor("idx_lo", [P, NT * NBLK_LO * 8], i16)
    idx_hi_d = nc.dram_tensor("idx_hi", [P, NT * NBLK_HI * 8], i16)
    dglob_d = nc.dram_tensor("dglob", [P, NT * NBLK * 8], i16)

    tab1_d = nc.dram_tensor("tab1", [N, TAB1_COLS], f32)
    vtab1_d = nc.dram_tensor("vtab1", [NPER, VT_COLS], f32)
    smtab1_d = nc.dram_tensor("smtab1", [G, TAB1_COLS], f32)
    smvtab1_d = nc.dram_tensor("smvtab1", [G, VT_COLS], f32)
    tab2own_d = nc.dram_tensor("tab2own", [NPER, TAB2_COLS], f32)
    tab2_d = nc.dram_tensor("tab2", [N, TAB2_COLS], f32, addr_space="Shared")
    vtab2_d = nc.dram_tensor("vtab2", [NPER, VT_COLS], f32)
    smtab2_d = nc.dram_tensor("smtab2", [G, TAB2_COLS], f32)
    smvtab2_d = nc.dram_tensor("smvtab2", [G, VT_COLS], f32)

    with TileContext(nc) as tc, contextlib.ExitStack() as ctx:
        pool = ctx.enter_context(tc.tile_pool(name="main", bufs=2))
        cpool = ctx.enter_context(tc.tile_pool(name="consts", bufs=1))
        spool = ctx.enter_context(tc.tile_pool(name="stash", bufs=1))
        gpool = ctx.enter_context(tc.tile_pool(name="gather", bufs=2))
        qpool = ctx.enter_context(tc.tile_pool(name="q", bufs=2))
        ipool = ctx.enter_context(tc.tile_pool(name="idxs", bufs=2))
        ppool = ctx.enter_context(tc.tile_pool(name="psA", bufs=2, space="PSUM"))
        npool = ctx.enter_context(tc.tile_pool(name="psN", bufs=2, space="PSUM"))
        tpool = ctx.enter_context(tc.tile_pool(name="psT", bufs=2, space="PSUM"))
        spsum = ctx.enter_context(tc.tile_pool(name="psS", bufs=1, space="PSUM"))

        # ---- AllGathers: assemble full x1T / x2T / A on device ----
        nc.sync.dma_start(out=x1st_d[:], in_=x1Tsh_d[:])
        nc.sync.dma_start(out=Abst_d[:], in_=Absh_d[:])
        nc.sync.dma_start(out=x2st_d[:], in_=x2Tsh_d[:])
        nc.gpsimd.collective_compute(
            "AllGather", Alu.bypass, replica_groups=[list(range(NCORES))],
            ins=[x1st_d[:]], outs=[x1TG_d[:]])
        nc.gpsimd.collective_compute(
            "AllGather", Alu.bypass, replica_groups=[list(range(NCORES))],
            ins=[Abst_d[:]], outs=[AbG_d[:]])
        nc.gpsimd.collective_compute(
            "AllGather", Alu.bypass, replica_groups=[list(range(NCORES))],
            ins=[x2st_d[:]], outs=[x2TG_d[:]])

        # ---- replicate compact idx tables to full 128-partition layout ----
        for g in range(8):
            nc.sync.dma_start(out=idx_lo_d[16 * g:16 * (g + 1), :],
                              in_=idx_loc_d[:])
            nc.sync.dma_start(out=idx_hi_d[16 * g:16 * (g + 1), :],
                              in_=idx_hic_d[:])
            nc.sync.dma_start(out=dglob_d[16 * g:16 * (g + 1), :],
                              in_=dglobc_d[:])

        def load_const(dram, shape, dtype, tag):
            t = cpool.tile(shape, dtype, tag=tag)
            nc.sync.dma_start(out=t[:], in_=dram[:])
            return t

        def load_rep16(dram, cols, dtype, tag):
            """[16, cols] DRAM -> [128, cols] SBUF, replicated 8x."""
            t = cpool.tile([P, cols], dtype, tag=tag)
            for g in range(8):
                nc.sync.dma_start(out=t[16 * g:16 * (g + 1), :], in_=dram[:])
            return t

        # iota row / per-partition index / identity, generated on device
        iotaI = cpool.tile([P, P], i32, tag="iotaI")
        nc.gpsimd.iota(iotaI[:], pattern=[[1, P]], base=0, channel_multiplier=0)
        iota_s = cpool.tile([P, P], f32, tag="iota")
        nc.vector.tensor_copy(out=iota_s[:], in_=iotaI[:])
        iotaPI = cpool.tile([P, 1], i32, tag="iotaPI")
        nc.gpsimd.iota(iotaPI[:], pattern=[[0, 1]], base=0, channel_multiplier=1)
        iotaP_s = cpool.tile([P, 1], f32, tag="iotaP")
        nc.vector.tensor_copy(out=iotaP_s[:], in_=iotaPI[:])
        ident_s = cpool.tile([P, P], f32, tag="ident")
        nc.vector.tensor_scalar(out=ident_s[:], in0=iota_s[:],
                                scalar1=iotaP_s[:, 0:1], scalar2=None,
                                op0=Alu.is_equal)

        wext1a_s = load_const(wext1a_d, [P, 256], f32r, "wext1a")
        wext1b_s = load_const(wext1b_d, [P, 256], f32r, "wext1b")
        wext2a_s = load_const(wext2a_d, [P, 64], f32r, "wext2a")
        wext2b_s = load_const(wext2b_d, [P, 64], f32r, "wext2b")
        rc_s = load_rep16(rowcat_d, 960, f32, "rowcat")
        brep1a_s = rc_s[:, 0:256]
        brep1b_s = rc_s[:, 256:512]
        brep2a_s = rc_s[:, 512:576]
        brep2b_s = rc_s[:, 576:640]
        g1rep_s = rc_s[:, 640:768]
        b1rep_s = rc_s[:, 768:896]
        g2rep_s = rc_s[:, 896:928]
        b2rep_s = rc_s[:, 928:960]

        idxsm_s = load_rep16(idx_smc_d, 8 * NBLK_SM * 8, i16, "idxsm")
        dglobsm_s = load_rep16(dglob_smc_d, 8 * NBLK_SM * 8, i16, "dglobsm")
        gidx_s = load_rep16(gidxc_d, NT * 8, i16, "gidx")

        def load_bf_as_f32(dram, cols, tag):
            tb = pool.tile([P, cols], bf16, tag=f"{tag}_bf")
            nc.sync.dma_start(out=tb[:], in_=dram[:])
            t = cpool.tile([P, cols], f32, tag=tag)
            nc.vector.tensor_copy(out=t[:], in_=tb[:])
            return t

        dstlsm_s = load_bf_as_f32(dstl_sm_d, 8 * NBLK_SM, "dstlsm")
        dstl_s = load_bf_as_f32(dstl_d, NT * NBLK, "dstl")

        # pre-zero the q-slots so junk pad columns of the fp32r rhs are finite
        for _ in range(2):
            zq = qpool.tile([P, MAXCH, 256], f32r, tag="q256")
            nc.vector.memset(zq[:].rearrange("p a b -> p (a b)").bitcast(f32), 0.0)
        for _ in range(2):
            zq = qpool.tile([P, max(NBLK, NBLK_SM), 40], f32r, tag="q33")
            nc.vector.memset(zq[:].rearrange("p a b -> p (a b)").bitcast(f32), 0.0)

        # ---------------- table builder ----------------
        def build_table(lhs_src, wext_s, brep_s, tab_dram, vtab_dram,
                        ntile, nrows, F, H, tab_cols, tag):
            """lhs_src(t) -> (dram_ap [128, rows], global row base)."""
            ncols = wext_s.shape[1]
            for t in range(ntile):
                ap, r0, rows = lhs_src(t)
                lhs = pool.tile([P, 128], f32r, tag="tb_lhs")
                nc.sync.dma_start(out=lhs[:, :rows], in_=ap)
                ps = ppool.tile([P, 256], f32, tag="agg", space="PSUM")
                nc.tensor.matmul(out=ps[:rows, :ncols], lhsT=lhs[:, :rows],
                                 rhs=wext_s[:], start=True, stop=True)
                st = pool.tile([P, ncols], f32, tag=f"tb_st{ncols}")
                nc.vector.scalar_tensor_tensor(
                    out=st[:rows, :], in0=ps[:rows, :ncols], scalar=1.0,
                    in1=brep_s[:rows, :], op0=Alu.bypass, op1=Alu.add)
                if vtab_dram is not None:
                    vst = pool.tile([P, 8], f32, tag="tb_vst")
                    nc.scalar.activation(vst[:rows, 0:H],
                                         st[:rows, F + H:F + 2 * H], Act.Exp)
                    nc.scalar.activation(vst[:rows, H:2 * H],
                                         st[:rows, F + H:F + 2 * H],
                                         Act.Exp, scale=0.2)
                    nc.sync.dma_start(out=vtab_dram[r0:r0 + rows, 0:2 * H],
                                      in_=vst[:rows, 0:2 * H])
                if tab_dram is not None:
                    # u2 then u (u overwrites the al cols u2 reads)
                    nc.scalar.activation(st[:rows, F + H:F + 2 * H],
                                         st[:rows, F:F + H], Act.Exp, scale=0.2)
                    nc.scalar.activation(st[:rows, F:F + H],
                                         st[:rows, F:F + H], Act.Exp)
                    nc.sync.dma_start(out=tab_dram[r0:r0 + rows, :],
                                      in_=st[:rows, 0:tab_cols])

        # ---------------- edge aggregation ----------------
        def edge_gat_tile(t, tab_dram, vtab_dram, idxlo_src, idxhi_src,
                          dstl_ap, dglob_src, nblk, nblk_lo, F, H, rhs_n,
                          idx_in_sbuf):
            """Returns agg psum [(numer F) | (s H)] for dst-tile t."""
            tabcols = TAB1_COLS if F == 128 else TAB2_COLS
            maxch = MAXCH if tabcols == TAB1_COLS else nblk
            gtag = "g192" if tabcols == TAB1_COLS else "g64"
            qtag = "q256" if F == 128 else "q33"
            chunks = []
            b0 = 0
            while b0 < nblk:
                chunks.append((b0, min(b0 + maxch, nblk)))
                b0 = min(b0 + maxch, nblk)
            ps = ppool.tile([P, 256], f32, tag="agg", space="PSUM")
            first = True
            for (c0, c1) in chunks:
                ch = c1 - c0
                gt = gpool.tile([P, maxch, tabcols], f32, tag=gtag)
                GCAP = 2  # max 256 indices per dma_gather (HW limit found)
                nlo = max(min(nblk_lo, c1) - c0, 0)
                if nlo > 0:
                    losl = slice((t * nblk_lo + c0) * 8,
                                 (t * nblk_lo + c0 + nlo) * 8)
                    if idx_in_sbuf:
                        ilo = idxlo_src[:, losl]
                    else:
                        ilo_t = ipool.tile([P, nlo * 8], i16, tag="ilo")
                        nc.sync.dma_start(out=ilo_t[:], in_=idxlo_src[:, losl])
                        ilo = ilo_t[:]
                    for g0 in range(0, nlo, GCAP):
                        g1 = min(g0 + GCAP, nlo)
                        nc.gpsimd.dma_gather(
                            out_ap=gt[:, g0:g1, :], in_ap=tab_dram[:],
                            idxs_ap=ilo[:, g0 * 8:g1 * 8],
                            num_idxs=(g1 - g0) * 128,
                            num_idxs_reg=(g1 - g0) * 128, elem_size=tabcols)
                if ch - nlo > 0:
                    h0 = max(c0 - nblk_lo, 0)
                    nhi = ch - nlo
                    hisl = slice((t * (nblk - nblk_lo) + h0) * 8,
                                 (t * (nblk - nblk_lo) + h0 + nhi) * 8)
                    if idx_in_sbuf:
                        ihi = idxhi_src[:, hisl]
                    else:
                        ihi_t = ipool.tile([P, nhi * 8], i16, tag="ihi")
                        nc.sync.dma_start(out=ihi_t[:], in_=idxhi_src[:, hisl])
                        ihi = ihi_t[:]
                    for g0 in range(0, nhi, GCAP):
                        g1 = min(g0 + GCAP, nhi)
                        nc.gpsimd.dma_gather(
                            out_ap=gt[:, nlo + g0:nlo + g1, :],
                            in_ap=tab_dram[SPLIT:, :],
                            idxs_ap=ihi[:, g0 * 8:g1 * 8],
                            num_idxs=(g1 - g0) * 128,
                            num_idxs_reg=(g1 - g0) * 128, elem_size=tabcols)
                vt = gpool.tile([P, maxch, VT_COLS], f32, tag="v64")
                dgsl = slice((t * nblk + c0) * 8, (t * nblk + c1) * 8)
                if idx_in_sbuf:
                    dg = dglob_src[:, dgsl]
                else:
                    dg_t = ipool.tile([P, ch * 8], i16, tag="dg")
                    nc.sync.dma_start(out=dg_t[:], in_=dglob_src[:, dgsl])
                    dg = dg_t[:]
                for g0 in range(0, ch, GCAP):
                    g1 = min(g0 + GCAP, ch)
                    nc.gpsimd.dma_gather(
                        out_ap=vt[:, g0:g1, :], in_ap=vtab_dram[:],
                        idxs_ap=dg[:, g0 * 8:g1 * 8],
                        num_idxs=(g1 - g0) * 128,
                        num_idxs_reg=(g1 - g0) * 128, elem_size=VT_COLS)
                mask = qpool.tile([P, maxch, 128], f32r, tag="mask")
                nc.vector.tensor_tensor(
                    out=mask[:, 0:ch, :],
                    in0=iota_s[:][:, None, :].to_broadcast([P, ch, 128]),
                    in1=dstl_ap[:, c0:c1][:, :, None].to_broadcast([P, ch, 128]),
                    op=Alu.is_equal)
                q = qpool.tile([P, maxch, rhs_n], f32r, tag=qtag)
                m1 = pool.tile([P, maxch, H], f32, tag="pm1")
                m2 = pool.tile([P, maxch, H], f32, tag="pm2")
                nc.vector.tensor_tensor(out=m1[:, 0:ch, :],
                                        in0=gt[:, 0:ch, F:F + H],
                                        in1=vt[:, 0:ch, 0:H], op=Alu.mult)
                nc.vector.tensor_tensor(out=m2[:, 0:ch, :],
                                        in0=gt[:, 0:ch, F + H:F + 2 * H],
                                        in1=vt[:, 0:ch, H:2 * H], op=Alu.mult)
                nc.vector.tensor_tensor(out=q[:, 0:ch, F:F + H],
                                        in0=m1[:, 0:ch, :], in1=m2[:, 0:ch, :],
                                        op=Alu.max)
                C = F // H
                for h in range(H):
                    nc.vector.tensor_tensor(
                        out=q[:, 0:ch, h * C:(h + 1) * C],
                        in0=gt[:, 0:ch, h * C:(h + 1) * C],
                        in1=q[:, 0:ch, F + h:F + h + 1].to_broadcast([P, ch, C]),
                        op=Alu.mult)
                for b in range(c0, c1):
                    nc.tensor.matmul(
                        out=ps[:, 0:rhs_n], lhsT=mask[:, b - c0, :],
                        rhs=q[:, b - c0, :], start=first, stop=(b == nblk - 1))
                    first = False
            return ps

        def xout_from_ps(ps, F, H, brep_s, tag):
            rec = pool.tile([P, H], f32, tag=f"{tag}_rec")
            nc.vector.reciprocal(out=rec[:], in_=ps[:, F:F + H])
            xo = pool.tile([P, F], f32, tag=f"{tag}_xo")
            C = F // H
            for h in range(H):
                nc.vector.tensor_scalar(
                    out=xo[:, h * C:(h + 1) * C], in0=ps[:, h * C:(h + 1) * C],
                    scalar1=rec[:, h:h + 1], scalar2=None, op0=Alu.mult)
            nc.vector.tensor_tensor(out=xo[:], in0=xo[:], in1=brep_s[:, 0:F],
                                    op=Alu.add)
            return xo

        # ---------------- group attention ----------------
        def group_attn(t, xo, X2pT_ap, X2ext_list, Fs, rhs_n, tag):
            """Returns 0.5*grp tile [P, Fs] f32."""
            pt = tpool.tile([P, 128], f32, tag="ptr", space="PSUM")
            nc.tensor.transpose(out=pt[:Fs, :], in_=xo[:, 0:Fs],
                                identity=ident_s[:])
            xT = pool.tile([P, 128], f32r, tag="ga_xT")
            nc.scalar.copy(out=xT[:Fs, :], in_=pt[:Fs, :])
            pss = spsum.tile([P, 1024], f32, tag="s", space="PSUM")
            nc.tensor.matmul(out=pss[:, 0:512], lhsT=xT[:Fs, :],
                             rhs=X2pT_ap[:, 0:512], start=True, stop=True)
            nc.tensor.matmul(out=pss[:, 512:1024], lhsT=xT[:Fs, :],
                             rhs=X2pT_ap[:, 512:1024], start=True, stop=True)
            mx0 = pool.tile([P, 1], f32, tag="ga_mx0")
            mx1 = pool.tile([P, 1], f32, tag="ga_mx1")
            nc.vector.reduce_max(mx0[:], pss[:, 0:512], axis=Ax.X)
            nc.vector.reduce_max(mx1[:], pss[:, 512:1024], axis=Ax.X)
            negmx = pool.tile([P, 1], f32, tag="ga_negmx")
            nc.vector.tensor_tensor(out=negmx[:], in0=mx0[:], in1=mx1[:],
                                    op=Alu.max)
            nc.vector.tensor_scalar(out=negmx[:], in0=negmx[:], scalar1=-1.0,
                                    scalar2=None, op0=Alu.mult)
            wx = pool.tile([P, G], f32, tag="ga_wx")
            nc.scalar.activation(wx[:, 0:512], pss[:, 0:512], Act.Exp,
                                 bias=negmx[:])
            nc.scalar.activation(wx[:, 512:1024], pss[:, 512:1024], Act.Exp,
                                 bias=negmx[:])
            at = pool.tile([P, 8, 128], bf16, tag="ga_at")
            nc.gpsimd.dma_gather(
                out_ap=at[:], in_ap=AbG_d[:], idxs_ap=gidx_s[:, t * 8:(t + 1) * 8],
                num_idxs=128, num_idxs_reg=128, elem_size=G, transpose=True)
            psn = npool.tile([P, 256], f32, tag="num", space="PSUM")
            for j in range(8):
                wt = tpool.tile([P, 128], f32, tag="ptr", space="PSUM")
                nc.tensor.transpose(out=wt[:], in_=wx[:, j * 128:(j + 1) * 128],
                                    identity=ident_s[:])
                bmt = pool.tile([P, 128], f32r, tag="ga_bmt")
                nc.vector.scalar_tensor_tensor(
                    out=bmt[:], in0=wt[:], scalar=1.0, in1=at[:, j, :],
                    op0=Alu.bypass, op1=Alu.mult)
                nc.tensor.matmul(out=psn[:, 0:rhs_n], lhsT=bmt[:],
                                 rhs=X2ext_list[j][:], start=(j == 0),
                                 stop=(j == 7))
            rec = pool.tile([P, 1], f32, tag="ga_grec")
            nc.vector.reciprocal(out=rec[:], in_=psn[:, Fs:Fs + 1])
            grp = pool.tile([P, Fs], f32, tag="ga_grp")
            nc.vector.tensor_scalar(out=grp[:], in0=psn[:, 0:Fs],
                                    scalar1=rec[:], scalar2=0.5, op0=Alu.mult,
                                    op1=Alu.mult)
            return grp

        # ================= phase 1: tables =================
        # global tab1: 8 sections x 49 tiles, straight from AllGathered x1TG
        def sec_lhs(sec):
            def f(t):
                r0 = t * 128
                rows = min(128, NPER - r0)
                return (x1TG_d[sec * P:(sec + 1) * P, r0:r0 + rows],
                        sec * NPER + r0, rows)
            return f
        for sec in range(NCORES):
            build_table(sec_lhs(sec), wext1a_s, brep1a_s, tab1_d, None,
                        NT, NPER, 128, 4, TAB1_COLS, f"t1s{sec}")

        # own v-table straight from the uploaded shard parameter
        def own_lhs(t):
            r0 = t * 128
            rows = min(128, NPER - r0)
            return x1Tsh_d[:, r0:r0 + rows], r0, rows
        build_table(own_lhs, wext1a_s, brep1a_s, None, vtab1_d,
                    NT, NPER, 128, 4, TAB1_COLS, "t1o")

        # small-graph tables from AllGathered x2TG (sections == tiles)
        def sm_lhs(t):
            return x2TG_d[t * P:(t + 1) * P, :], t * P, 128
        build_table(sm_lhs, wext1b_s, brep1b_s, smtab1_d, smvtab1_d,
                    8, G, 128, 4, TAB1_COLS, "ts1")

        # ================= small-graph GAT layer 1 =================
        xg1 = []
        X2pT = cpool.tile([P, G], f32r, tag="X2pT")
        X2ext = []
        for t in range(8):
            ps = edge_gat_tile(
                t, smtab1_d, smvtab1_d, idxsm_s, None,
                dstlsm_s[:, t * NBLK_SM:(t + 1) * NBLK_SM], dglobsm_s,
                NBLK_SM, NBLK_SM, 128, 4, 256, True)
            xo = xout_from_ps(ps, 128, 4, brep1b_s, "sm1")
            keep = spool.tile([P, 128], f32, tag=f"xg1_{t}")
            nc.vector.tensor_copy(out=keep[:], in_=xo[:])
            xg1.append(keep)
            pt = tpool.tile([P, 128], f32, tag="ptr", space="PSUM")
            nc.tensor.transpose(out=pt[:], in_=keep[:], identity=ident_s[:])
            nc.scalar.copy(out=X2pT[:, t * 128:(t + 1) * 128], in_=pt[:])
            xe = spool.tile([P, 256], f32r, tag=f"X2ext_{t}")
            nc.scalar.copy(out=xe[:, 0:128], in_=keep[:])
            nc.vector.memset(xe[:, 128:129].bitcast(f32), 1.0)
            nc.vector.memset(xe[:, 129:256].bitcast(f32), 0.0)
            X2ext.append(xe)

        # ================= big-graph layer 1 =================
        var49 = cpool.tile([P, NT], f32, tag="var49")
        s1_tiles = []
        for t in range(NT):
            ps = edge_gat_tile(
                t, tab1_d, vtab1_d, idx_lo_d, idx_hi_d,
                dstl_s[:, t * NBLK:(t + 1) * NBLK], dglob_d,
                NBLK, NBLK_LO, 128, 4, 256, False)
            xo = xout_from_ps(ps, 128, 4, brep1a_s, "b1")
            grp = group_attn(t, xo, X2pT[:], X2ext, 128, 256, "g1")
            s1 = spool.tile([P, 128], f32, tag=f"s1_{t}")
            nc.vector.scalar_tensor_tensor(out=s1[:], in0=xo[:], scalar=0.5,
                                           in1=grp[:], op0=Alu.mult, op1=Alu.add)
            mu = pool.tile([P, 1], f32, tag="b1_mu")
            nc.vector.tensor_reduce(out=mu[:], in_=s1[:], axis=Ax.X, op=Alu.add)
            nc.vector.tensor_scalar(out=mu[:], in0=mu[:], scalar1=-1.0 / 128,
                                    scalar2=None, op0=Alu.mult)
            nc.vector.tensor_scalar(out=s1[:], in0=s1[:], scalar1=mu[:],
                                    scalar2=None, op0=Alu.add)
            sq = pool.tile([P, 128], f32, tag="b1_sq")
            nc.vector.tensor_tensor(out=sq[:], in0=s1[:], in1=s1[:], op=Alu.mult)
            nc.vector.tensor_reduce(out=var49[:, t:t + 1], in_=sq[:], axis=Ax.X,
                                    op=Alu.add)
            s1_tiles.append(s1)

        sd49 = cpool.tile([P, NT], f32, tag="sd49")
        nc.vector.tensor_scalar(out=sd49[:], in0=var49[:], scalar1=1.0 / 128,
                                scalar2=LN_EPS, op0=Alu.mult, op1=Alu.add)
        sq49 = cpool.tile([P, NT], f32, tag="sq49")
        nc.scalar.activation(sq49[:], sd49[:], Act.Sqrt)
        rstd49 = cpool.tile([P, NT], f32, tag="rstd49")
        nc.vector.reciprocal(out=rstd49[:], in_=sq49[:])

        for t in range(NT):
            s1 = s1_tiles[t]
            y = pool.tile([P, 128], f32, tag="b1_y")
            nc.vector.scalar_tensor_tensor(
                out=y[:], in0=s1[:], scalar=rstd49[:, t:t + 1], in1=g1rep_s[:],
                op0=Alu.mult, op1=Alu.mult)
            nc.vector.tensor_tensor(out=y[:], in0=y[:], in1=b1rep_s[:],
                                    op=Alu.add)
            emin = pool.tile([P, 128], f32, tag="b1_emin")
            nc.vector.tensor_scalar(out=emin[:], in0=y[:], scalar1=0.0,
                                    scalar2=None, op0=Alu.min)
            nc.scalar.activation(emin[:], emin[:], Act.Exp)
            h1 = pool.tile([P, 128], f32, tag="b1_h1")
            nc.vector.tensor_scalar(out=h1[:], in0=y[:], scalar1=0.0,
                                    scalar2=-1.0, op0=Alu.max, op1=Alu.add)
            nc.vector.tensor_tensor(out=h1[:], in0=h1[:], in1=emin[:], op=Alu.add)
            pt = tpool.tile([P, 128], f32, tag="ptr", space="PSUM")
            nc.tensor.transpose(out=pt[:], in_=h1[:], identity=ident_s[:])
            h1T = pool.tile([P, 128], f32r, tag="b1_h1T")
            nc.scalar.copy(out=h1T[:], in_=pt[:])
            ps2 = npool.tile([P, 256], f32, tag="num", space="PSUM")
            nc.tensor.matmul(out=ps2[:, 0:64], lhsT=h1T[:], rhs=wext2a_s[:],
                             start=True, stop=True)
            st2 = pool.tile([P, 64], f32, tag="b1_st2")
            nc.vector.scalar_tensor_tensor(
                out=st2[:], in0=ps2[:, 0:64], scalar=1.0, in1=brep2a_s[:],
                op0=Alu.bypass, op1=Alu.add)
            vst = pool.tile([P, 2], f32, tag="b1_vst")
            nc.scalar.activation(vst[:, 0:1], st2[:, 33:34], Act.Exp)
            nc.scalar.activation(vst[:, 1:2], st2[:, 33:34], Act.Exp, scale=0.2)
            nc.scalar.activation(st2[:, 33:34], st2[:, 32:33], Act.Exp, scale=0.2)
            nc.scalar.activation(st2[:, 32:33], st2[:, 32:33], Act.Exp)
            rows = min(128, NPER - t * 128)
            nc.sync.dma_start(out=tab2own_d[t * 128:t * 128 + rows, :],
                              in_=st2[:rows, :])
            nc.sync.dma_start(out=vtab2_d[t * 128:t * 128 + rows, 0:2],
                              in_=vst[:rows, 0:2])

        nc.gpsimd.collective_compute(
            "AllGather", Alu.bypass, replica_groups=[list(range(NCORES))],
            ins=[tab2own_d[:]], outs=[tab2_d[:]])

        # ================= small-graph layer 2 =================
        for t in range(8):
            pt = tpool.tile([P, 128], f32, tag="ptr", space="PSUM")
            nc.tensor.transpose(out=pt[:], in_=xg1[t][:], identity=ident_s[:])
            xT = pool.tile([P, 128], f32r, tag="ts2_xT")
            nc.scalar.copy(out=xT[:], in_=pt[:])
            ps2 = npool.tile([P, 256], f32, tag="num", space="PSUM")
            nc.tensor.matmul(out=ps2[:, 0:64], lhsT=xT[:], rhs=wext2b_s[:],
                             start=True, stop=True)
            st2 = pool.tile([P, 64], f32, tag="ts2_st")
            nc.vector.scalar_tensor_tensor(
                out=st2[:], in0=ps2[:, 0:64], scalar=1.0, in1=brep2b_s[:],
                op0=Alu.bypass, op1=Alu.add)
            vst = pool.tile([P, 2], f32, tag="ts2_vst")
            nc.scalar.activation(vst[:, 0:1], st2[:, 33:34], Act.Exp)
            nc.scalar.activation(vst[:, 1:2], st2[:, 33:34], Act.Exp, scale=0.2)
            nc.scalar.activation(st2[:, 33:34], st2[:, 32:33], Act.Exp, scale=0.2)
            nc.scalar.activation(st2[:, 32:33], st2[:, 32:33], Act.Exp)
            nc.sync.dma_start(out=smtab2_d[t * 128:(t + 1) * 128, :], in_=st2[:])
            nc.sync.dma_start(out=smvtab2_d[t * 128:(t + 1) * 128, 0:2],
                              in_=vst[:, 0:2])

        xg2 = []
        X2p2T = cpool.tile([32, G], f32r, tag="X2p2T")
        X2ext2 = []
        for t in range(8):
            ps = edge_gat_tile(
                t, smtab2_d, smvtab2_d, idxsm_s, None,
                dstlsm_s[:, t * NBLK_SM:(t + 1) * NBLK_SM], dglobsm_s,
                NBLK_SM, NBLK_SM, 32, 1, 40, True)
            xo = xout_from_ps(ps, 32, 1, brep2b_s, "sm2")
            keep = spool.tile([P, 32], f32, tag=f"xg2_{t}")
            nc.vector.tensor_copy(out=keep[:], in_=xo[:])
            xg2.append(keep)
            pt = tpool.tile([P, 128], f32, tag="ptr", space="PSUM")
            nc.tensor.transpose(out=pt[:32, :], in_=keep[:], identity=ident_s[:])
            nc.scalar.copy(out=X2p2T[:, t * 128:(t + 1) * 128], in_=pt[:32, :])
            xe = spool.tile([P, 40], f32r, tag=f"X2ext2_{t}")
            nc.scalar.copy(out=xe[:, 0:32], in_=keep[:])
            nc.vector.memset(xe[:, 32:33].bitcast(f32), 1.0)
            nc.vector.memset(xe[:, 33:40].bitcast(f32), 0.0)
            X2ext2.append(xe)

        # ================= big-graph layer 2 =================
        var49b = cpool.tile([P, NT], f32, tag="var49b")
        o_tiles = []
        for t in range(NT):
            ps = edge_gat_tile(
                t, tab2_d, vtab2_d, idx_lo_d, idx_hi_d,
                dstl_s[:, t * NBLK:(t + 1) * NBLK], dglob_d,
                NBLK, NBLK_LO, 32, 1, 40, False)
            xo = xout_from_ps(ps, 32, 1, brep2a_s, "b2")
            grp = group_attn(t, xo, X2p2T[:], X2ext2, 32, 40, "g2")
            o = spool.tile([P, 32], f32, tag=f"o_{t}")
            nc.vector.scalar_tensor_tensor(out=o[:], in0=xo[:], scalar=0.5,
                                           in1=grp[:], op0=Alu.mult, op1=Alu.add)
            mu = pool.tile([P, 1], f32, tag="b2_mu")
            nc.vector.tensor_reduce(out=mu[:], in_=o[:], axis=Ax.X, op=Alu.add)
            nc.vector.tensor_scalar(out=mu[:], in0=mu[:], scalar1=-1.0 / 32,
                                    scalar2=None, op0=Alu.mult)
            nc.vector.tensor_scalar(out=o[:], in0=o[:], scalar1=mu[:],
                                    scalar2=None, op0=Alu.add)
            sq = pool.tile([P, 32], f32, tag="b2_sq")
            nc.vector.tensor_tensor(out=sq[:], in0=o[:], in1=o[:], op=Alu.mult)
            nc.vector.tensor_reduce(out=var49b[:, t:t + 1], in_=sq[:], axis=Ax.X,
                                    op=Alu.add)
            o_tiles.append(o)

        sd49b = cpool.tile([P, NT], f32, tag="sd49b")
        nc.vector.tensor_scalar(out=sd49b[:], in0=var49b[:], scalar1=1.0 / 32,
                                scalar2=LN_EPS, op0=Alu.mult, op1=Alu.add)
        sq49b = cpool.tile([P, NT], f32, tag="sq49b")
        nc.scalar.activation(sq49b[:], sd49b[:], Act.Sqrt)
        rstd49b = cpool.tile([P, NT], f32, tag="rstd49b")
        nc.vector.reciprocal(out=rstd49b[:], in_=sq49b[:])

        for t in range(NT):
            o = o_tiles[t]
            y = pool.tile([P, 32], f32, tag="b2_y")
            nc.vector.scalar_tensor_tensor(
                out=y[:], in0=o[:], scalar=rstd49b[:, t:t + 1], in1=g2rep_s[:],
                op0=Alu.mult, op1=Alu.mult)
            nc.vector.tensor_tensor(out=y[:], in0=y[:], in1=b2rep_s[:],
                                    op=Alu.add)
            yb = pool.tile([P, 32], bf16, tag="b2_yb")
            nc.vector.tensor_copy(out=yb[:], in_=y[:])
            rows = min(128, NPER - t * 128)
            nc.sync.dma_start(out=out_d[t * 128:t * 128 + rows, :],
                              in_=yb[:rows, :])

    nc.compile()
    return nc


# --------------------------------------------------------------------------
# entry point
# --------------------------------------------------------------------------

def kernel(**inputs):
    from concourse.bass_utils import run_bass_kernel_spmd

    shared, per_core, meta = host_prep(inputs)
    nc = build_nc(meta)
    in_maps = []
    for c in range(NCORES):
        m = dict(shared)
        m.update(per_core[c])
        in_maps.append(m)
    res = run_bass_kernel_spmd(nc, in_maps, list(range(NCORES)))
    out = np.concatenate([np.asarray(res.results[c]["out"])
                          for c in range(NCORES)])
    return out.astype(np.float32)


# revision 14
# speedup vs baseline: 2.4523x; 2.4523x over previous
"""Dual-GAT (nn_GAT_48017734369678) on 8 TRN2 NeuronCores via Bass/Tile.

Self-contained: host-side sharding/preprocessing in numpy, device program in
Bass (Tile), executed through run_bass_kernel_spmd on cores 0-7.

The dispatch cost here is dominated by (a) host->device upload bytes over the
axon tunnel (~50MB/s) and (b) STATIC instruction count in the NEFF (~45us per
instruction per dispatch). Both are minimized:
  (a) each core uploads only its own transposed x1 shard (bf16) / Ab rows /
      x2 rows; full copies are assembled on-device with AllGather. Gather
      index tables are uploaded compact ([16, n/16]) and replicated on device.
  (b) every per-tile stage is wrapped in a tc.For_i hardware loop with
      dynamic (register-offset) access patterns, so the program is ~600
      instructions instead of ~24000.

Per-core row spaces are padded to NPAD=6272=49*128 so all loops are uniform;
src node ids are remapped on host into the padded id space, and the padded
output rows are sliced off on host.

Edge aggregation: per-node gather tables in DRAM + dma_gather by src, one-hot
mask matmuls (fp32r) accumulating (numer | softmax-denominator) in PSUM.
Group graph replicated on every core. Identities used:
  exp(LeakyReLU(al+ar)) == max(exp(al)exp(ar), exp(.2al)exp(.2ar))
  segment softmax is shift-invariant (edge scores are O(10): no max needed)
  (A+I)[gidx] row gather folds the group-attention self term exactly.
"""
import sys

sys.path.insert(0, "/opt/trn_rl_repo")

import numpy as np

N, G = 50000, 1024
F_IN, HID, HEADS, NCLS = 128, 32, 4, 32
LN_EPS = 1e-5
NCORES = 8
NPER = N // NCORES            # 6250
NT = (NPER + 127) // 128      # 49 tiles/core
NPAD = NT * 128               # 6272 padded rows/core
NG = NCORES * NPAD            # 50176 padded global rows
SPLIT = 32768                 # int16 gather split (padded id space)
P = 128
SENT = 255.0                  # pad-edge dstlocal sentinel (mask never matches)
TAB1_COLS = 192               # [h(128) | u(4) | u2(4) | junk]  (768B rows)
TAB2_COLS = 64                # [h2(32) | u(1) | u2(1) | junk]  (256B rows)
VT_COLS = 64                  # [v(H) | v2(H) | junk]           (256B rows)
GCAP = 8                      # gather blocks (of 128 idxs) per dma_gather


# --------------------------------------------------------------------------
# host-side preprocessing
# --------------------------------------------------------------------------

def _wrap16(ix):
    """Compact dma_gather idx layout: [16, n/16]; idx i at [i%16, i//16].
    Replication to the 8 groups of 16 partitions happens on device."""
    ix = np.asarray(ix, np.int64)
    n = len(ix)
    assert n % 16 == 0, n
    return np.ascontiguousarray(ix.reshape(n // 16, 16).T.astype(np.int16))


def _segments(src, dst, ntile, split):
    """src already in padded-id space; dst in core-local [0, NPER)."""
    tile = dst // 128
    segs = []
    for t in range(ntile):
        m = tile == t
        s, d = src[m], dst[m] - t * 128
        if split:
            lo = s < SPLIT
            segs.append((s[lo], d[lo], s[~lo], d[~lo]))
        else:
            segs.append((s, d, s[:0], d[:0]))
    return segs


def _flatten(segs, nblk_lo, nblk_hi, ntile):
    nblk = nblk_lo + nblk_hi
    idx_lo, idx_hi, dmod, dglob = [], [], [], []
    for t in range(ntile):
        slo, dlo, shi, dhi = segs[t]
        a = np.zeros(nblk_lo * 128, np.int64); a[:len(slo)] = slo
        b = np.zeros(nblk_hi * 128, np.int64); b[:len(shi)] = shi - SPLIT
        dm = np.full(nblk * 128, SENT, np.float64)
        dm[:len(dlo)] = dlo
        dm[nblk_lo * 128:nblk_lo * 128 + len(dhi)] = dhi
        dg = np.zeros(nblk * 128, np.int64)
        dg[:len(dlo)] = dlo + t * 128
        dg[nblk_lo * 128:nblk_lo * 128 + len(dhi)] = dhi + t * 128
        idx_lo.append(a); idx_hi.append(b); dmod.append(dm); dglob.append(dg)
    idx_lo = np.concatenate(idx_lo) if nblk_lo else np.zeros(0, np.int64)
    idx_hi = np.concatenate(idx_hi) if nblk_hi else np.zeros(0, np.int64)
    dmod = np.concatenate(dmod)
    dglob = np.concatenate(dglob)
    # block layout [128, ntile*nblk]: column t*nblk+b holds block b's dstlocal
    import ml_dtypes
    dmod2 = np.ascontiguousarray(
        dmod.reshape(ntile * nblk, 128).T.astype(ml_dtypes.bfloat16))
    return idx_lo, idx_hi, dmod2, dglob


def _wext(W, a_src, a_dst, b, ncols):
    W = np.asarray(W, np.float32)
    a_src = np.asarray(a_src, np.float32)
    a_dst = np.asarray(a_dst, np.float32)
    b = np.asarray(b, np.float32)
    H, C = a_src.shape
    D = W.shape[1]
    asrc_m = np.zeros((D, H), np.float32)
    adst_m = np.zeros((D, H), np.float32)
    for h in range(H):
        asrc_m[h * C:(h + 1) * C, h] = a_src[h]
        adst_m[h * C:(h + 1) * C, h] = a_dst[h]
    Wx = np.concatenate([W, W @ asrc_m, W @ adst_m], axis=1)
    Wx = np.concatenate(
        [Wx, np.zeros((W.shape[0], ncols - Wx.shape[1]), np.float32)], axis=1)
    brow = np.concatenate([b, b @ asrc_m, b @ adst_m,
                           np.zeros(ncols - D - 2 * H, np.float32)])
    return np.ascontiguousarray(Wx), brow.astype(np.float32)


def host_prep(inputs):
    import ml_dtypes
    bf16 = ml_dtypes.bfloat16
    f32 = np.float32
    x1 = np.asarray(inputs["x1"], f32)
    ei1 = np.asarray(inputs["edge_index1"], np.int64)
    x2 = np.asarray(inputs["x2"], f32)
    ei2 = np.asarray(inputs["edge_index2"], np.int64)
    gidx = np.asarray(inputs["group_index"], np.int64)

    A = np.zeros((G, G), f32)
    u, v = ei2[0], ei2[1]
    np.add.at(A, (u, v), 1.0)
    np.add.at(A, (v, u), (u != v).astype(f32))
    Ap = A + np.eye(G, dtype=f32)
    assert Ap.max() < 256

    src_g, dst_g = ei1[0], ei1[1]
    # remap src node id into the padded-section id space (core*NPAD + local)
    pad_of = lambda ids: (ids // NPER) * NPAD + (ids % NPER)
    core_of = dst_g // NPER
    all_segs = []
    for c in range(NCORES):
        m = core_of == c
        loops = np.arange(c * NPER, (c + 1) * NPER, dtype=np.int64)
        s = pad_of(np.concatenate([src_g[m], loops]))
        d = np.concatenate([dst_g[m], loops]) - c * NPER
        all_segs.append(_segments(s, d, NT, True))
    nblk_lo = max(max((len(t[0]) + 127) // 128 for t in sg) for sg in all_segs)
    nblk_hi = max(max((len(t[2]) + 127) // 128 for t in sg) for sg in all_segs)

    loops2 = np.arange(G, dtype=np.int64)
    s2 = np.concatenate([ei2[0], loops2])
    d2 = np.concatenate([ei2[1], loops2])
    sm_segs = _segments(s2, d2, G // 128, False)
    nblk_sm = max((len(t[0]) + 127) // 128 for t in sm_segs)

    meta = dict(nblk_lo=nblk_lo, nblk_hi=nblk_hi, nblk=nblk_lo + nblk_hi,
                nblk_sm=nblk_sm)

    w1a, b1a = _wext(inputs["W1a"], inputs["a1a_src"], inputs["a1a_dst"],
                     inputs["b1a"], 256)
    w1b, b1b = _wext(inputs["W1b"], inputs["a1b_src"], inputs["a1b_dst"],
                     inputs["b1b"], 256)
    w2a, b2a = _wext(inputs["W2a"], inputs["a2a_src"], inputs["a2a_dst"],
                     inputs["b2a"], 64)
    w2b, b2b = _wext(inputs["W2b"], inputs["a2b_src"], inputs["a2b_dst"],
                     inputs["b2b"], 64)

    i_sm, _, dm_sm, dg_sm = _flatten(sm_segs, nblk_sm, 0, G // 128)

    # [b1a(0:256)|b1b(256:512)|b2a(512:576)|b2b(576:640)|
    #  ln1g(640:768)|ln1b(768:896)|ln2g(896:928)|ln2b(928:960)]
    rowcat = np.concatenate([
        b1a, b1b, b2a, b2b,
        np.asarray(inputs["ln1_g"], f32), np.asarray(inputs["ln1_b"], f32),
        np.asarray(inputs["ln2_g"], f32), np.asarray(inputs["ln2_b"], f32)])
    rowcat16 = np.ascontiguousarray(
        np.broadcast_to(rowcat[None, :], (16, rowcat.shape[0])))

    shared = dict(
        wext1a=np.asarray(w1a, bf16), wext1b=w1b,
        wext2a=w2a, wext2b=w2b,
        rowcat=rowcat16,
        idx_smc=_wrap16(i_sm), dstl_sm=dm_sm, dglob_smc=_wrap16(dg_sm),
    )

    per_core = []
    for c in range(NCORES):
        ilo, ihi, dmod, dglob = _flatten(all_segs[c], nblk_lo, nblk_hi, NT)
        gown = np.concatenate([gidx[c * NPER:(c + 1) * NPER],
                               np.zeros(NPAD - NPER, np.int64)])
        x1sh = np.zeros((P, NPAD), f32)
        x1sh[:, :NPER] = x1[c * NPER:(c + 1) * NPER].T
        per_core.append(dict(
            idx_loc=_wrap16(ilo), idx_hic=_wrap16(ihi),
            dstl=dmod, dglobc=_wrap16(dglob), gidxc=_wrap16(gown),
            x1Tsh=np.asarray(x1sh, bf16),
            x2Tsh=np.ascontiguousarray(x2[c * 128:(c + 1) * 128].T),
            Absh=np.asarray(Ap[c * 128:(c + 1) * 128], bf16),
        ))
    return shared, per_core, meta


# --------------------------------------------------------------------------
# device program
# --------------------------------------------------------------------------

def build_nc(meta):
    import contextlib
    from concourse import bacc, mybir
    from concourse.tile import TileContext
    from concourse.bass import ds, ts

    f32 = mybir.dt.float32
    f32r = mybir.dt.float32r
    bf16 = mybir.dt.bfloat16
    i16 = mybir.dt.int16
    i32 = mybir.dt.int32
    Alu = mybir.AluOpType
    Act = mybir.ActivationFunctionType
    Ax = mybir.AxisListType

    NBLK = meta["nblk"]
    NBLK_LO = meta["nblk_lo"]
    NBLK_HI = meta["nblk_hi"]
    NBLK_SM = meta["nblk_sm"]

    nc = bacc.Bacc(None, target_bir_lowering=False, debug=True)

    dp = lambda n, s, d: nc.declare_dram_parameter(n, list(s), d, isOutput=False)
    x1Tsh_d = dp("x1Tsh", [P, NPAD], bf16)
    x2Tsh_d = dp("x2Tsh", [P, P], f32r)
    Absh_d = dp("Absh", [P, G], bf16)
    wext1a_d = dp("wext1a", [P, 256], bf16)
    wext1b_d = dp("wext1b", [P, 256], f32r)
    wext2a_d = dp("wext2a", [P, 64], f32r)
    wext2b_d = dp("wext2b", [P, 64], f32r)
    rowcat_d = dp("rowcat", [16, 960], f32)
    idx_smc_d = dp("idx_smc", [16, 8 * NBLK_SM * 8], i16)
    dstl_sm_d = dp("dstl_sm", [P, 8 * NBLK_SM], bf16)
    dglob_smc_d = dp("dglob_smc", [16, 8 * NBLK_SM * 8], i16)
    idx_loc_d = dp("idx_loc", [16, NT * NBLK_LO * 8], i16)
    idx_hic_d = dp("idx_hic", [16, NT * NBLK_HI * 8], i16)
    dstl_d = dp("dstl", [P, NT * NBLK], bf16)
    dglobc_d = dp("dglobc", [16, NT * NBLK * 8], i16)
    gidxc_d = dp("gidxc", [16, NT * 8], i16)

    out_d = nc.declare_dram_parameter("out", [NPAD, NCLS], bf16, isOutput=True)

    # AllGather-assembled full tensors (collectives cannot read IO tensors
    # directly, so shards are staged into internal DRAM first)
    x1st_d = nc.dram_tensor("x1st", [P, NPAD], bf16)
    x2st_d = nc.dram_tensor("x2st", [P, P], f32r)
    Abst_d = nc.dram_tensor("Abst", [P, G], bf16)
    x1TG_d = nc.dram_tensor("x1TG", [8 * P, NPAD], bf16, addr_space="Shared")
    x2TG_d = nc.dram_tensor("x2TG", [8 * P, P], f32r, addr_space="Shared")
    AbG_d = nc.dram_tensor("AbG", [G, G], bf16, addr_space="Shared")
    # full-layout (8x replicated) gather index tables, built on device
    idx_lo_d = nc.dram_tensor("idx_lo", [P, NT * NBLK_LO * 8], i16)
    idx_hi_d = nc.dram_tensor("idx_hi", [P, NT * NBLK_HI * 8], i16)
    dglob_d = nc.dram_tensor("dglob", [P, NT * NBLK * 8], i16)

    tab1_d = nc.dram_tensor("tab1", [NG, TAB1_COLS], f32)
    vtab1_d = nc.dram_tensor("vtab1", [NPAD, VT_COLS], f32)
    smtab1_d = nc.dram_tensor("smtab1", [G, TAB1_COLS], f32)
    smvtab1_d = nc.dram_tensor("smvtab1", [G, VT_COLS], f32)
    tab2own_d = nc.dram_tensor("tab2own", [NPAD, TAB2_COLS], f32)
    tab2_d = nc.dram_tensor("tab2", [NG, TAB2_COLS], f32, addr_space="Shared")
    vtab2_d = nc.dram_tensor("vtab2", [NPAD, VT_COLS], f32)
    smtab2_d = nc.dram_tensor("smtab2", [G, TAB2_COLS], f32)
    smvtab2_d = nc.dram_tensor("smvtab2", [G, VT_COLS], f32)

    with TileContext(nc) as tc, contextlib.ExitStack() as ctx:
        pool = ctx.enter_context(tc.tile_pool(name="main", bufs=2))
        cpool = ctx.enter_context(tc.tile_pool(name="consts", bufs=1))
        spool = ctx.enter_context(tc.tile_pool(name="stash", bufs=1))
        gpool = ctx.enter_context(tc.tile_pool(name="gather", bufs=1))
        qpool = ctx.enter_context(tc.tile_pool(name="q", bufs=1))
        ppool = ctx.enter_context(tc.tile_pool(name="psA", bufs=2, space="PSUM"))
        npool = ctx.enter_context(tc.tile_pool(name="psN", bufs=2, space="PSUM"))
        tpool = ctx.enter_context(tc.tile_pool(name="psT", bufs=2, space="PSUM"))
        spsum = ctx.enter_context(tc.tile_pool(name="psS", bufs=1, space="PSUM"))

        # ---- AllGathers: assemble full x1T / x2T / A on device ----
        nc.sync.dma_start(out=x1st_d[:], in_=x1Tsh_d[:])
        nc.sync.dma_start(out=Abst_d[:], in_=Absh_d[:])
        nc.sync.dma_start(out=x2st_d[:], in_=x2Tsh_d[:])
        nc.gpsimd.collective_compute(
            "AllGather", Alu.bypass, replica_groups=[list(range(NCORES))],
            ins=[x1st_d[:]], outs=[x1TG_d[:]])
        nc.gpsimd.collective_compute(
            "AllGather", Alu.bypass, replica_groups=[list(range(NCORES))],
            ins=[Abst_d[:]], outs=[AbG_d[:]])
        nc.gpsimd.collective_compute(
            "AllGather", Alu.bypass, replica_groups=[list(range(NCORES))],
            ins=[x2st_d[:]], outs=[x2TG_d[:]])

        # ---- replicate compact idx tables to full 128-partition layout ----
        for g in range(8):
            nc.sync.dma_start(out=idx_lo_d[16 * g:16 * (g + 1), :],
                              in_=idx_loc_d[:])
            nc.sync.dma_start(out=idx_hi_d[16 * g:16 * (g + 1), :],
                              in_=idx_hic_d[:])
            nc.sync.dma_start(out=dglob_d[16 * g:16 * (g + 1), :],
                              in_=dglobc_d[:])

        def load_const(dram, shape, dtype, tag):
            t = cpool.tile(shape, dtype, tag=tag)
            nc.sync.dma_start(out=t[:], in_=dram[:])
            return t

        def load_rep16(dram, cols, dtype, tag):
            """[16, cols] DRAM -> [128, cols] SBUF, replicated 8x."""
            t = cpool.tile([P, cols], dtype, tag=tag)
            for g in range(8):
                nc.sync.dma_start(out=t[16 * g:16 * (g + 1), :], in_=dram[:])
            return t

        # iota row / per-partition index / identity, generated on device
        iotaI = cpool.tile([P, P], i32, tag="iotaI")
        nc.gpsimd.iota(iotaI[:], pattern=[[1, P]], base=0, channel_multiplier=0)
        iota_s = cpool.tile([P, P], f32, tag="iota")
        nc.vector.tensor_copy(out=iota_s[:], in_=iotaI[:])
        iotaPI = cpool.tile([P, 1], i32, tag="iotaPI")
        nc.gpsimd.iota(iotaPI[:], pattern=[[0, 1]], base=0, channel_multiplier=1)
        iotaP_s = cpool.tile([P, 1], f32, tag="iotaP")
        nc.vector.tensor_copy(out=iotaP_s[:], in_=iotaPI[:])
        ident_s = cpool.tile([P, P], f32, tag="ident")
        nc.vector.tensor_scalar(out=ident_s[:], in0=iota_s[:],
                                scalar1=iotaP_s[:, 0:1], scalar2=None,
                                op0=Alu.is_equal)

        wext1a_s = load_const(wext1a_d, [P, 256], bf16, "wext1a")
        wext1b_s = load_const(wext1b_d, [P, 256], f32r, "wext1b")
        wext2a_s = load_const(wext2a_d, [P, 64], f32r, "wext2a")
        wext2b_s = load_const(wext2b_d, [P, 64], f32r, "wext2b")
        rc_s = load_rep16(rowcat_d, 960, f32, "rowcat")
        brep1a_s = rc_s[:, 0:256]
        brep1b_s = rc_s[:, 256:512]
        brep2a_s = rc_s[:, 512:576]
        brep2b_s = rc_s[:, 576:640]
        g1rep_s = rc_s[:, 640:768]
        b1rep_s = rc_s[:, 768:896]
        g2rep_s = rc_s[:, 896:928]
        b2rep_s = rc_s[:, 928:960]

        idxsm_s = load_rep16(idx_smc_d, 8 * NBLK_SM * 8, i16, "idxsm")
        dglobsm_s = load_rep16(dglob_smc_d, 8 * NBLK_SM * 8, i16, "dglobsm")
        gidx_s = load_rep16(gidxc_d, NT * 8, i16, "gidx")

        def load_bf_as_f32(dram, cols, tag):
            tb = pool.tile([P, cols], bf16, tag=f"{tag}_bf")
            nc.sync.dma_start(out=tb[:], in_=dram[:])
            t = cpool.tile([P, cols], f32, tag=tag)
            nc.vector.tensor_copy(out=t[:], in_=tb[:])
            return t

        dstlsm_s = load_bf_as_f32(dstl_sm_d, 8 * NBLK_SM, "dstlsm")
        dstl_s = load_bf_as_f32(dstl_d, NT * NBLK, "dstl")

        # pre-zero the q-slots so junk pad columns of the fp32r rhs are finite
        zq = qpool.tile([P, NBLK, 256], f32r, tag="q256")
        nc.vector.memset(zq[:].rearrange("p a b -> p (a b)").bitcast(f32), 0.0)
        zq = qpool.tile([P, max(NBLK, NBLK_SM), 40], f32r, tag="q33")
        nc.vector.memset(zq[:].rearrange("p a b -> p (a b)").bitcast(f32), 0.0)

        # ---------------- phase 1: tables ----------------
        # global tab1 (8 sections x 49 tiles) from AllGathered x1TG, bf16
        with tc.For_i(0, NCORES, 1) as sec:
            with tc.For_i(0, NT, 1) as t:
                lhsb = pool.tile([P, 128], bf16, tag="tb_lhsb")
                nc.sync.dma_start(out=lhsb[:],
                                  in_=x1TG_d[ts(sec, P), ts(t, 128)])
                ps = ppool.tile([P, 256], f32, tag="agg", space="PSUM")
                nc.tensor.matmul(out=ps[:], lhsT=lhsb[:], rhs=wext1a_s[:],
                                 start=True, stop=True)
                st = pool.tile([P, 256], f32, tag="tb_st")
                nc.vector.scalar_tensor_tensor(
                    out=st[:], in0=ps[:], scalar=1.0,
                    in1=brep1a_s[:], op0=Alu.bypass, op1=Alu.add)
                nc.scalar.activation(st[:, 132:136], st[:, 128:132],
                                     Act.Exp, scale=0.2)
                nc.scalar.activation(st[:, 128:132], st[:, 128:132], Act.Exp)
                nc.sync.dma_start(
                    out=tab1_d[ds(sec * NPAD + t * 128, 128), :],
                    in_=st[:, 0:TAB1_COLS])

        # own v-table straight from the uploaded shard parameter
        with tc.For_i(0, NT, 1) as t:
            lhsb = pool.tile([P, 128], bf16, tag="vt_lhsb")
            nc.sync.dma_start(out=lhsb[:], in_=x1Tsh_d[:, ts(t, 128)])
            ps = npool.tile([P, 256], f32, tag="num", space="PSUM")
            nc.tensor.matmul(out=ps[:, 0:8], lhsT=lhsb[:],
                             rhs=wext1a_s[:, 128:136], start=True, stop=True)
            st = pool.tile([P, 8], f32, tag="vt_st")
            nc.vector.scalar_tensor_tensor(
                out=st[:], in0=ps[:, 0:8], scalar=1.0,
                in1=rc_s[:, 128:136], op0=Alu.bypass, op1=Alu.add)
            vst = pool.tile([P, 8], f32, tag="vt_vst")
            nc.scalar.activation(vst[:, 0:4], st[:, 4:8], Act.Exp)
            nc.scalar.activation(vst[:, 4:8], st[:, 4:8], Act.Exp, scale=0.2)
            nc.sync.dma_start(out=vtab1_d[ts(t, 128), 0:8], in_=vst[:])

        # small-graph tables from AllGathered x2TG (sections == tiles)
        with tc.For_i(0, 8, 1) as t:
            lhs = pool.tile([P, 128], f32r, tag="sm_lhs")
            nc.sync.dma_start(out=lhs[:], in_=x2TG_d[ts(t, P), :])
            ps = ppool.tile([P, 256], f32, tag="agg", space="PSUM")
            nc.tensor.matmul(out=ps[:], lhsT=lhs[:], rhs=wext1b_s[:],
                             start=True, stop=True)
            st = pool.tile([P, 256], f32, tag="tb_st")
            nc.vector.scalar_tensor_tensor(
                out=st[:], in0=ps[:], scalar=1.0,
                in1=brep1b_s[:], op0=Alu.bypass, op1=Alu.add)
            vst = pool.tile([P, 8], f32, tag="vt_vst")
            nc.scalar.activation(vst[:, 0:4], st[:, 132:136], Act.Exp)
            nc.scalar.activation(vst[:, 4:8], st[:, 132:136], Act.Exp,
                                 scale=0.2)
            nc.sync.dma_start(out=smvtab1_d[ts(t, 128), 0:8], in_=vst[:])
            nc.scalar.activation(st[:, 132:136], st[:, 128:132],
                                 Act.Exp, scale=0.2)
            nc.scalar.activation(st[:, 128:132], st[:, 128:132], Act.Exp)
            nc.sync.dma_start(out=smtab1_d[ts(t, 128), :],
                              in_=st[:, 0:TAB1_COLS])

        # ---------------- edge aggregation (loop body helper) ----------------
        def edge_gat_body(t, tab_dram, vtab_dram, idxlo_src, idxhi_src,
                          dstl_src, dglob_src, nblk, nblk_lo, F, H, rhs_n,
                          idx_in_sbuf, tag):
            """Emits ops for dst-tile t (loop var); returns agg psum
            [(numer F) | (s H)]."""
            tabcols = TAB1_COLS if F == 128 else TAB2_COLS
            gtag = f"g{tabcols}"
            qtag = "q256" if F == 128 else "q33"
            nblk_hi = nblk - nblk_lo
            gt = gpool.tile([P, nblk, tabcols], f32, tag=gtag)
            for g0 in range(0, nblk_lo, GCAP):
                g1 = min(g0 + GCAP, nblk_lo)
                if idx_in_sbuf:
                    iap = idxlo_src[:, ds(t * nblk_lo * 8 + g0 * 8,
                                          (g1 - g0) * 8)]
                else:
                    it = pool.tile([P, (g1 - g0) * 8], i16, tag=f"{tag}_il{g0}")
                    nc.sync.dma_start(
                        out=it[:], in_=idxlo_src[:, ds(t * nblk_lo * 8 + g0 * 8,
                                                       (g1 - g0) * 8)])
                    iap = it[:]
                nc.gpsimd.dma_gather(
                    out_ap=gt[:, g0:g1, :], in_ap=tab_dram[:],
                    idxs_ap=iap, num_idxs=(g1 - g0) * 128,
                    num_idxs_reg=(g1 - g0) * 128, elem_size=tabcols)
            for g0 in range(0, nblk_hi, GCAP):
                g1 = min(g0 + GCAP, nblk_hi)
                if idx_in_sbuf:
                    iap = idxhi_src[:, ds(t * nblk_hi * 8 + g0 * 8,
                                          (g1 - g0) * 8)]
                else:
                    it = pool.tile([P, (g1 - g0) * 8], i16, tag=f"{tag}_ih{g0}")
                    nc.sync.dma_start(
                        out=it[:], in_=idxhi_src[:, ds(t * nblk_hi * 8 + g0 * 8,
                                                       (g1 - g0) * 8)])
                    iap = it[:]
                nc.gpsimd.dma_gather(
                    out_ap=gt[:, nblk_lo + g0:nblk_lo + g1, :],
                    in_ap=tab_dram[SPLIT:, :],
                    idxs_ap=iap, num_idxs=(g1 - g0) * 128,
                    num_idxs_reg=(g1 - g0) * 128, elem_size=tabcols)
            vt = gpool.tile([P, nblk, VT_COLS], f32, tag="v64")
            for g0 in range(0, nblk, GCAP):
                g1 = min(g0 + GCAP, nblk)
                if idx_in_sbuf:
                    iap = dglob_src[:, ds(t * nblk * 8 + g0 * 8, (g1 - g0) * 8)]
                else:
                    it = pool.tile([P, (g1 - g0) * 8], i16, tag=f"{tag}_dg{g0}")
                    nc.sync.dma_start(
                        out=it[:], in_=dglob_src[:, ds(t * nblk * 8 + g0 * 8,
                                                       (g1 - g0) * 8)])
                    iap = it[:]
                nc.gpsimd.dma_gather(
                    out_ap=vt[:, g0:g1, :], in_ap=vtab_dram[:],
                    idxs_ap=iap, num_idxs=(g1 - g0) * 128,
                    num_idxs_reg=(g1 - g0) * 128, elem_size=VT_COLS)
            dl = pool.tile([P, nblk], f32, tag=f"{tag}_dl")
            nc.vector.tensor_copy(out=dl[:], in_=dstl_src[:, ts(t, nblk)])
            mask = qpool.tile([P, nblk, 128], f32r, tag="mask")
            nc.vector.tensor_tensor(
                out=mask[:],
                in0=iota_s[:][:, None, :].to_broadcast([P, nblk, 128]),
                in1=dl[:][:, :, None].to_broadcast([P, nblk, 128]),
                op=Alu.is_equal)
            q = qpool.tile([P, nblk, rhs_n], f32r, tag=qtag)
            m1 = pool.tile([P, nblk, H], f32, tag="pm1")
            m2 = pool.tile([P, nblk, H], f32, tag="pm2")
            nc.vector.tensor_tensor(out=m1[:], in0=gt[:, :, F:F + H],
                                    in1=vt[:, :, 0:H], op=Alu.mult)
            nc.vector.tensor_tensor(out=m2[:], in0=gt[:, :, F + H:F + 2 * H],
                                    in1=vt[:, :, H:2 * H], op=Alu.mult)
            nc.vector.tensor_tensor(out=q[:, :, F:F + H], in0=m1[:],
                                    in1=m2[:], op=Alu.max)
            C = F // H
            for h in range(H):
                nc.vector.tensor_tensor(
                    out=q[:, :, h * C:(h + 1) * C],
                    in0=gt[:, :, h * C:(h + 1) * C],
                    in1=q[:, :, F + h:F + h + 1].to_broadcast([P, nblk, C]),
                    op=Alu.mult)
            ps = ppool.tile([P, 256], f32, tag="agg", space="PSUM")
            for b in range(nblk):
                nc.tensor.matmul(
                    out=ps[:, 0:rhs_n], lhsT=mask[:, b, :], rhs=q[:, b, :],
                    start=(b == 0), stop=(b == nblk - 1))
            return ps

        def xout_from_ps(ps, F, H, brep_s, tag):
            rec = pool.tile([P, H], f32, tag=f"{tag}_rec")
            nc.vector.reciprocal(out=rec[:], in_=ps[:, F:F + H])
            xo = pool.tile([P, F], f32, tag=f"{tag}_xo")
            C = F // H
            for h in range(H):
                nc.vector.tensor_scalar(
                    out=xo[:, h * C:(h + 1) * C], in0=ps[:, h * C:(h + 1) * C],
                    scalar1=rec[:, h:h + 1], scalar2=None, op0=Alu.mult)
            nc.vector.tensor_tensor(out=xo[:], in0=xo[:], in1=brep_s[:, 0:F],
                                    op=Alu.add)
            return xo

        # ---------------- group attention (loop body helper) ----------------
        def group_attn(t, xo, X2pT_ap, X2ext_all, Fs, rhs_n, tag):
            """Returns 0.5*grp tile [P, Fs] f32."""
            pt = tpool.tile([P, 128], f32, tag="ptr", space="PSUM")
            nc.tensor.transpose(out=pt[:Fs, :], in_=xo[:, 0:Fs],
                                identity=ident_s[:])
            xT = pool.tile([P, 128], f32r, tag="ga_xT")
            nc.scalar.copy(out=xT[:Fs, :], in_=pt[:Fs, :])
            pss = spsum.tile([P, 1024], f32, tag="s", space="PSUM")
            nc.tensor.matmul(out=pss[:, 0:512], lhsT=xT[:Fs, :],
                             rhs=X2pT_ap[:, 0:512], start=True, stop=True)
            nc.tensor.matmul(out=pss[:, 512:1024], lhsT=xT[:Fs, :],
                             rhs=X2pT_ap[:, 512:1024], start=True, stop=True)
            mx0 = pool.tile([P, 1], f32, tag="ga_mx0")
            mx1 = pool.tile([P, 1], f32, tag="ga_mx1")
            nc.vector.reduce_max(mx0[:], pss[:, 0:512], axis=Ax.X)
            nc.vector.reduce_max(mx1[:], pss[:, 512:1024], axis=Ax.X)
            negmx = pool.tile([P, 1], f32, tag="ga_negmx")
            nc.vector.tensor_tensor(out=negmx[:], in0=mx0[:], in1=mx1[:],
                                    op=Alu.max)
            nc.vector.tensor_scalar(out=negmx[:], in0=negmx[:], scalar1=-1.0,
                                    scalar2=None, op0=Alu.mult)
            wx = pool.tile([P, G], f32, tag="ga_wx")
            nc.scalar.activation(wx[:, 0:512], pss[:, 0:512], Act.Exp,
                                 bias=negmx[:])
            nc.scalar.activation(wx[:, 512:1024], pss[:, 512:1024], Act.Exp,
                                 bias=negmx[:])
            at = pool.tile([P, 8, 128], bf16, tag="ga_at")
            nc.gpsimd.dma_gather(
                out_ap=at[:], in_ap=AbG_d[:], idxs_ap=gidx_s[:, ts(t, 8)],
                num_idxs=128, num_idxs_reg=128, elem_size=G, transpose=True)
            psn = npool.tile([P, 256], f32, tag="num", space="PSUM")
            for j in range(8):
                wt = tpool.tile([P, 128], f32, tag="ptr", space="PSUM")
                nc.tensor.transpose(out=wt[:], in_=wx[:, j * 128:(j + 1) * 128],
                                    identity=ident_s[:])
                bmt = pool.tile([P, 128], f32r, tag="ga_bmt")
                nc.vector.scalar_tensor_tensor(
                    out=bmt[:], in0=wt[:], scalar=1.0, in1=at[:, j, :],
                    op0=Alu.bypass, op1=Alu.mult)
                nc.tensor.matmul(out=psn[:, 0:rhs_n], lhsT=bmt[:],
                                 rhs=X2ext_all[:, j, :], start=(j == 0),
                                 stop=(j == 7))
            rec = pool.tile([P, 1], f32, tag="ga_grec")
            nc.vector.reciprocal(out=rec[:], in_=psn[:, Fs:Fs + 1])
            grp = pool.tile([P, Fs], f32, tag="ga_grp")
            nc.vector.tensor_scalar(out=grp[:], in0=psn[:, 0:Fs],
                                    scalar1=rec[:], scalar2=0.5, op0=Alu.mult,
                                    op1=Alu.mult)
            return grp

        # ================= small-graph GAT layer 1 =================
        xg1_all = spool.tile([P, 8, 128], f32, tag="xg1")
        X2pT = cpool.tile([P, G], f32r, tag="X2pT")
        X2ext_all = spool.tile([P, 8, 256], f32r, tag="X2ext")
        nc.vector.memset(
            X2ext_all[:].rearrange("p a b -> p (a b)").bitcast(f32), 0.0)
        nc.vector.memset(X2ext_all[:, :, 128:129].bitcast(f32), 1.0)
        with tc.For_i(0, 8, 1) as t:
            ps = edge_gat_body(
                t, smtab1_d, smvtab1_d, idxsm_s, None,
                dstlsm_s, dglobsm_s,
                NBLK_SM, NBLK_SM, 128, 4, 256, True, "sg1")
            xo = xout_from_ps(ps, 128, 4, brep1b_s, "sm1")
            nc.vector.tensor_copy(out=xg1_all[:, t, :], in_=xo[:])
            pt = tpool.tile([P, 128], f32, tag="ptr", space="PSUM")
            nc.tensor.transpose(out=pt[:], in_=xo[:], identity=ident_s[:])
            nc.scalar.copy(out=X2pT[:, ts(t, 128)], in_=pt[:])
            nc.scalar.copy(out=X2ext_all[:, t, 0:128], in_=xo[:])

        # ================= big-graph layer 1 =================
        var49 = cpool.tile([P, NT], f32, tag="var49")
        s1_all = spool.tile([P, NT, 128], f32, tag="s1")
        with tc.For_i(0, NT, 1) as t:
            ps = edge_gat_body(
                t, tab1_d, vtab1_d, idx_lo_d, idx_hi_d,
                dstl_s, dglob_d,
                NBLK, NBLK_LO, 128, 4, 256, False, "bg1")
            xo = xout_from_ps(ps, 128, 4, brep1a_s, "b1")
            grp = group_attn(t, xo, X2pT[:], X2ext_all, 128, 256, "g1")
            s1 = pool.tile([P, 128], f32, tag="b1_s1")
            nc.vector.scalar_tensor_tensor(out=s1[:], in0=xo[:], scalar=0.5,
                                           in1=grp[:], op0=Alu.mult, op1=Alu.add)
            mu = pool.tile([P, 1], f32, tag="b1_mu")
            nc.vector.tensor_reduce(out=mu[:], in_=s1[:], axis=Ax.X, op=Alu.add)
            nc.vector.tensor_scalar(out=mu[:], in0=mu[:], scalar1=-1.0 / 128,
                                    scalar2=None, op0=Alu.mult)
            nc.vector.tensor_scalar(out=s1[:], in0=s1[:], scalar1=mu[:],
                                    scalar2=None, op0=Alu.add)
            nc.vector.tensor_copy(out=s1_all[:, t, :], in_=s1[:])
            sq = pool.tile([P, 128], f32, tag="b1_sq")
            nc.vector.tensor_tensor(out=sq[:], in0=s1[:], in1=s1[:], op=Alu.mult)
            nc.vector.tensor_reduce(out=var49[:, ds(t, 1)], in_=sq[:], axis=Ax.X,
                                    op=Alu.add)

        sd49 = cpool.tile([P, NT], f32, tag="sd49")
        nc.vector.tensor_scalar(out=sd49[:], in0=var49[:], scalar1=1.0 / 128,
                                scalar2=LN_EPS, op0=Alu.mult, op1=Alu.add)
        sq49 = cpool.tile([P, NT], f32, tag="sq49")
        nc.scalar.activation(sq49[:], sd49[:], Act.Sqrt)
        rstd49 = cpool.tile([P, NT], f32, tag="rstd49")
        nc.vector.reciprocal(out=rstd49[:], in_=sq49[:])

        with tc.For_i(0, NT, 1) as t:
            s1 = pool.tile([P, 128], f32, tag="l1_s1")
            nc.vector.tensor_copy(out=s1[:], in_=s1_all[:, t, :])
            rs = pool.tile([P, 1], f32, tag="l1_rs")
            nc.vector.tensor_copy(out=rs[:], in_=rstd49[:, ds(t, 1)])
            y = pool.tile([P, 128], f32, tag="b1_y")
            nc.vector.scalar_tensor_tensor(
                out=y[:], in0=s1[:], scalar=rs[:], in1=g1rep_s[:],
                op0=Alu.mult, op1=Alu.mult)
            nc.vector.tensor_tensor(out=y[:], in0=y[:], in1=b1rep_s[:],
                                    op=Alu.add)
            emin = pool.tile([P, 128], f32, tag="b1_emin")
            nc.vector.tensor_scalar(out=emin[:], in0=y[:], scalar1=0.0,
                                    scalar2=None, op0=Alu.min)
            nc.scalar.activation(emin[:], emin[:], Act.Exp)
            h1 = pool.tile([P, 128], f32, tag="b1_h1")
            nc.vector.tensor_scalar(out=h1[:], in0=y[:], scalar1=0.0,
                                    scalar2=-1.0, op0=Alu.max, op1=Alu.add)
            nc.vector.tensor_tensor(out=h1[:], in0=h1[:], in1=emin[:], op=Alu.add)
            pt = tpool.tile([P, 128], f32, tag="ptr", space="PSUM")
            nc.tensor.transpose(out=pt[:], in_=h1[:], identity=ident_s[:])
            h1T = pool.tile([P, 128], f32r, tag="b1_h1T")
            nc.scalar.copy(out=h1T[:], in_=pt[:])
            ps2 = npool.tile([P, 256], f32, tag="num", space="PSUM")
            nc.tensor.matmul(out=ps2[:, 0:64], lhsT=h1T[:], rhs=wext2a_s[:],
                             start=True, stop=True)
            st2 = pool.tile([P, 64], f32, tag="b1_st2")
            nc.vector.scalar_tensor_tensor(
                out=st2[:], in0=ps2[:, 0:64], scalar=1.0, in1=brep2a_s[:],
                op0=Alu.bypass, op1=Alu.add)
            vst = pool.tile([P, 2], f32, tag="b1_vst")
            nc.scalar.activation(vst[:, 0:1], st2[:, 33:34], Act.Exp)
            nc.scalar.activation(vst[:, 1:2], st2[:, 33:34], Act.Exp, scale=0.2)
            nc.scalar.activation(st2[:, 33:34], st2[:, 32:33], Act.Exp, scale=0.2)
            nc.scalar.activation(st2[:, 32:33], st2[:, 32:33], Act.Exp)
            nc.sync.dma_start(out=tab2own_d[ts(t, 128), :], in_=st2[:])
            nc.sync.dma_start(out=vtab2_d[ts(t, 128), 0:2], in_=vst[:, 0:2])

        nc.gpsimd.collective_compute(
            "AllGather", Alu.bypass, replica_groups=[list(range(NCORES))],
            ins=[tab2own_d[:]], outs=[tab2_d[:]])

        # ================= small-graph layer 2 =================
        with tc.For_i(0, 8, 1) as t:
            xg = pool.tile([P, 128], f32, tag="ts2_xg")
            nc.vector.tensor_copy(out=xg[:], in_=xg1_all[:, t, :])
            pt = tpool.tile([P, 128], f32, tag="ptr", space="PSUM")
            nc.tensor.transpose(out=pt[:], in_=xg[:], identity=ident_s[:])
            xT = pool.tile([P, 128], f32r, tag="ts2_xT")
            nc.scalar.copy(out=xT[:], in_=pt[:])
            ps2 = npool.tile([P, 256], f32, tag="num", space="PSUM")
            nc.tensor.matmul(out=ps2[:, 0:64], lhsT=xT[:], rhs=wext2b_s[:],
                             start=True, stop=True)
            st2 = pool.tile([P, 64], f32, tag="ts2_st")
            nc.vector.scalar_tensor_tensor(
                out=st2[:], in0=ps2[:, 0:64], scalar=1.0, in1=brep2b_s[:],
                op0=Alu.bypass, op1=Alu.add)
            vst = pool.tile([P, 2], f32, tag="ts2_vst")
            nc.scalar.activation(vst[:, 0:1], st2[:, 33:34], Act.Exp)
            nc.scalar.activation(vst[:, 1:2], st2[:, 33:34], Act.Exp, scale=0.2)
            nc.scalar.activation(st2[:, 33:34], st2[:, 32:33], Act.Exp, scale=0.2)
            nc.scalar.activation(st2[:, 32:33], st2[:, 32:33], Act.Exp)
            nc.sync.dma_start(out=smtab2_d[ts(t, 128), :], in_=st2[:])
            nc.sync.dma_start(out=smvtab2_d[ts(t, 128), 0:2], in_=vst[:, 0:2])

        xg2_all = spool.tile([P, 8, 32], f32, tag="xg2")
        X2p2T = cpool.tile([32, G], f32r, tag="X2p2T")
        X2ext2_all = spool.tile([P, 8, 40], f32r, tag="X2ext2")
        nc.vector.memset(
            X2ext2_all[:].rearrange("p a b -> p (a b)").bitcast(f32), 0.0)
        nc.vector.memset(X2ext2_all[:, :, 32:33].bitcast(f32), 1.0)
        with tc.For_i(0, 8, 1) as t:
            ps = edge_gat_body(
                t, smtab2_d, smvtab2_d, idxsm_s, None,
                dstlsm_s, dglobsm_s,
                NBLK_SM, NBLK_SM, 32, 1, 40, True, "sg2")
            xo = xout_from_ps(ps, 32, 1, brep2b_s, "sm2")
            nc.vector.tensor_copy(out=xg2_all[:, t, :], in_=xo[:])
            pt = tpool.tile([P, 128], f32, tag="ptr", space="PSUM")
            nc.tensor.transpose(out=pt[:32, :], in_=xo[:], identity=ident_s[:])
            nc.scalar.copy(out=X2p2T[:, ts(t, 128)], in_=pt[:32, :])
            nc.scalar.copy(out=X2ext2_all[:, t, 0:32], in_=xo[:])

        # ================= big-graph layer 2 =================
        var49b = cpool.tile([P, NT], f32, tag="var49b")
        o_all = spool.tile([P, NT, 32], f32, tag="o")
        with tc.For_i(0, NT, 1) as t:
            ps = edge_gat_body(
                t, tab2_d, vtab2_d, idx_lo_d, idx_hi_d,
                dstl_s, dglob_d,
                NBLK, NBLK_LO, 32, 1, 40, False, "bg2")
            xo = xout_from_ps(ps, 32, 1, brep2a_s, "b2")
            grp = group_attn(t, xo, X2p2T[:], X2ext2_all, 32, 40, "g2")
            o = pool.tile([P, 32], f32, tag="b2_o")
            nc.vector.scalar_tensor_tensor(out=o[:], in0=xo[:], scalar=0.5,
                                           in1=grp[:], op0=Alu.mult, op1=Alu.add)
            mu = pool.tile([P, 1], f32, tag="b2_mu")
            nc.vector.tensor_reduce(out=mu[:], in_=o[:], axis=Ax.X, op=Alu.add)
            nc.vector.tensor_scalar(out=mu[:], in0=mu[:], scalar1=-1.0 / 32,
                                    scalar2=None, op0=Alu.mult)
            nc.vector.tensor_scalar(out=o[:], in0=o[:], scalar1=mu[:],
                                    scalar2=None, op0=Alu.add)
            nc.vector.tensor_copy(out=o_all[:, t, :], in_=o[:])
            sq = pool.tile([P, 32], f32, tag="b2_sq")
            nc.vector.tensor_tensor(out=sq[:], in0=o[:], in1=o[:], op=Alu.mult)
            nc.vector.tensor_reduce(out=var49b[:, ds(t, 1)], in_=sq[:],
                                    axis=Ax.X, op=Alu.add)

        sd49b = cpool.tile([P, NT], f32, tag="sd49b")
        nc.vector.tensor_scalar(out=sd49b[:], in0=var49b[:], scalar1=1.0 / 32,
                                scalar2=LN_EPS, op0=Alu.mult, op1=Alu.add)
        sq49b = cpool.tile([P, NT], f32, tag="sq49b")
        nc.scalar.activation(sq49b[:], sd49b[:], Act.Sqrt)
        rstd49b = cpool.tile([P, NT], f32, tag="rstd49b")
        nc.vector.reciprocal(out=rstd49b[:], in_=sq49b[:])

        with tc.For_i(0, NT, 1) as t:
            o = pool.tile([P, 32], f32, tag="l2_o")
            nc.vector.tensor_copy(out=o[:], in_=o_all[:, t, :])
            rs = pool.tile([P, 1], f32, tag="l2_rs")
            nc.vector.tensor_copy(out=rs[:], in_=rstd49b[:, ds(t, 1)])
            y = pool.tile([P, 32], f32, tag="b2_y")
            nc.vector.scalar_tensor_tensor(
                out=y[:], in0=o[:], scalar=rs[:], in1=g2rep_s[:],
                op0=Alu.mult, op1=Alu.mult)
            nc.vector.tensor_tensor(out=y[:], in0=y[:], in1=b2rep_s[:],
                                    op=Alu.add)
            yb = pool.tile([P, 32], bf16, tag="b2_yb")
            nc.vector.tensor_copy(out=yb[:], in_=y[:])
            nc.sync.dma_start(out=out_d[ts(t, 128), :], in_=yb[:])

    nc.compile()
    return nc


# --------------------------------------------------------------------------
# entry point
# --------------------------------------------------------------------------

def kernel(**inputs):
    from concourse.bass_utils import run_bass_kernel_spmd

    shared, per_core, meta = host_prep(inputs)
    nc = build_nc(meta)
    in_maps = []
    for c in range(NCORES):
        m = dict(shared)
        m.update(per_core[c])
        in_maps.append(m)
    res = run_bass_kernel_spmd(nc, in_maps, list(range(NCORES)))
    out = np.concatenate([np.asarray(res.results[c]["out"])[:NPER]
                          for c in range(NCORES)])
    return out.astype(np.float32)


# revision 16
# speedup vs baseline: 2.7924x; 1.1387x over previous
"""Dual-GAT (nn_GAT_48017734369678) on 8 TRN2 NeuronCores via Bass/Tile.

Self-contained: host-side sharding/preprocessing in numpy, device program in
Bass (Tile), executed through run_bass_kernel_spmd on cores 0-7.

The dispatch cost here is dominated by (a) host->device upload bytes over the
axon tunnel (~50MB/s) and (b) STATIC instruction count in the NEFF (~45us per
instruction per dispatch). Both are minimized:
  (a) each core uploads only its own transposed x1 shard (bf16) / Ab rows /
      x2 rows; full copies are assembled on-device with AllGather. Gather
      index tables are uploaded compact ([16, n/16]) and replicated on device.
  (b) every per-tile stage is wrapped in a tc.For_i hardware loop with
      dynamic (register-offset) access patterns, so the program is ~600
      instructions instead of ~24000.

Per-core row spaces are padded to NPAD=6272=49*128 so all loops are uniform;
src node ids are remapped on host into the padded id space, and the padded
output rows are sliced off on host.

Edge aggregation: per-node gather tables in DRAM + dma_gather by src, one-hot
mask matmuls (fp32r) accumulating (numer | softmax-denominator) in PSUM.
Group graph replicated on every core. Identities used:
  exp(LeakyReLU(al+ar)) == max(exp(al)exp(ar), exp(.2al)exp(.2ar))
  segment softmax is shift-invariant (edge scores are O(10): no max needed)
  (A+I)[gidx] row gather folds the group-attention self term exactly.
"""
import sys

sys.path.insert(0, "/opt/trn_rl_repo")

import numpy as np

N, G = 50000, 1024
F_IN, HID, HEADS, NCLS = 128, 32, 4, 32
LN_EPS = 1e-5
NCORES = 8
NPER = N // NCORES            # 6250
NT = (NPER + 127) // 128      # 49 tiles/core
NPAD = NT * 128               # 6272 padded rows/core
NG = NCORES * NPAD            # 50176 padded global rows
SPLIT = 32768                 # int16 gather split (padded id space)
P = 128
SENT = 255.0                  # pad-edge dstlocal sentinel (mask never matches)
TAB1_COLS = 192               # [h(128) | u(4) | u2(4) | junk]  (768B rows)
TAB2_COLS = 64                # [h2(32) | u(1) | u2(1) | junk]  (256B rows)
VT_COLS = 64                  # [v(H) | v2(H) | junk]           (256B rows)
GCAP = 8                      # gather blocks (of 128 idxs) per dma_gather


# --------------------------------------------------------------------------
# host-side preprocessing
# --------------------------------------------------------------------------

def _wrap16(ix):
    """Compact dma_gather idx layout: [16, n/16]; idx i at [i%16, i//16].
    Replication to the 8 groups of 16 partitions happens on device."""
    ix = np.asarray(ix, np.int64)
    n = len(ix)
    assert n % 16 == 0, n
    return np.ascontiguousarray(ix.reshape(n // 16, 16).T.astype(np.int16))


def _segments(src, dst, ntile, split):
    """src already in padded-id space; dst in core-local [0, NPER)."""
    tile = dst // 128
    segs = []
    for t in range(ntile):
        m = tile == t
        s, d = src[m], dst[m] - t * 128
        if split:
            lo = s < SPLIT
            segs.append((s[lo], d[lo], s[~lo], d[~lo]))
        else:
            segs.append((s, d, s[:0], d[:0]))
    return segs


def _flatten(segs, nblk_lo, nblk_hi, ntile):
    nblk = nblk_lo + nblk_hi
    idx_lo, idx_hi, dmod, dglob = [], [], [], []
    for t in range(ntile):
        slo, dlo, shi, dhi = segs[t]
        a = np.zeros(nblk_lo * 128, np.int64); a[:len(slo)] = slo
        b = np.zeros(nblk_hi * 128, np.int64); b[:len(shi)] = shi - SPLIT
        dm = np.full(nblk * 128, SENT, np.float64)
        dm[:len(dlo)] = dlo
        dm[nblk_lo * 128:nblk_lo * 128 + len(dhi)] = dhi
        dg = np.zeros(nblk * 128, np.int64)
        dg[:len(dlo)] = dlo + t * 128
        dg[nblk_lo * 128:nblk_lo * 128 + len(dhi)] = dhi + t * 128
        idx_lo.append(a); idx_hi.append(b); dmod.append(dm); dglob.append(dg)
    idx_lo = np.concatenate(idx_lo) if nblk_lo else np.zeros(0, np.int64)
    idx_hi = np.concatenate(idx_hi) if nblk_hi else np.zeros(0, np.int64)
    dmod = np.concatenate(dmod)
    dglob = np.concatenate(dglob)
    # block layout [128, ntile*nblk]: column t*nblk+b holds block b's dstlocal
    dmod2 = np.ascontiguousarray(
        dmod.reshape(ntile * nblk, 128).T.astype(np.uint8))
    return idx_lo, idx_hi, dmod2, dglob


def _wext(W, a_src, a_dst, b, ncols):
    W = np.asarray(W, np.float32)
    a_src = np.asarray(a_src, np.float32)
    a_dst = np.asarray(a_dst, np.float32)
    b = np.asarray(b, np.float32)
    H, C = a_src.shape
    D = W.shape[1]
    asrc_m = np.zeros((D, H), np.float32)
    adst_m = np.zeros((D, H), np.float32)
    for h in range(H):
        asrc_m[h * C:(h + 1) * C, h] = a_src[h]
        adst_m[h * C:(h + 1) * C, h] = a_dst[h]
    Wx = np.concatenate([W, W @ asrc_m, W @ adst_m], axis=1)
    Wx = np.concatenate(
        [Wx, np.zeros((W.shape[0], ncols - Wx.shape[1]), np.float32)], axis=1)
    brow = np.concatenate([b, b @ asrc_m, b @ adst_m,
                           np.zeros(ncols - D - 2 * H, np.float32)])
    return np.ascontiguousarray(Wx), brow.astype(np.float32)


def host_prep(inputs):
    import ml_dtypes
    bf16 = ml_dtypes.bfloat16
    f32 = np.float32
    x1 = np.asarray(inputs["x1"], f32)
    ei1 = np.asarray(inputs["edge_index1"], np.int64)
    x2 = np.asarray(inputs["x2"], f32)
    ei2 = np.asarray(inputs["edge_index2"], np.int64)
    gidx = np.asarray(inputs["group_index"], np.int64)

    A = np.zeros((G, G), f32)
    u, v = ei2[0], ei2[1]
    np.add.at(A, (u, v), 1.0)
    np.add.at(A, (v, u), (u != v).astype(f32))
    Ap = A + np.eye(G, dtype=f32)
    assert Ap.max() < 256

    src_g, dst_g = ei1[0], ei1[1]
    # remap src node id into the padded-section id space (core*NPAD + local)
    pad_of = lambda ids: (ids // NPER) * NPAD + (ids % NPER)
    core_of = dst_g // NPER
    all_segs = []
    for c in range(NCORES):
        m = core_of == c
        loops = np.arange(c * NPER, (c + 1) * NPER, dtype=np.int64)
        s = pad_of(np.concatenate([src_g[m], loops]))
        d = np.concatenate([dst_g[m], loops]) - c * NPER
        all_segs.append(_segments(s, d, NT, True))
    nblk_lo = max(max((len(t[0]) + 127) // 128 for t in sg) for sg in all_segs)
    nblk_hi = max(max((len(t[2]) + 127) // 128 for t in sg) for sg in all_segs)

    loops2 = np.arange(G, dtype=np.int64)
    s2 = np.concatenate([ei2[0], loops2])
    d2 = np.concatenate([ei2[1], loops2])
    sm_segs = _segments(s2, d2, G // 128, False)
    nblk_sm = max((len(t[0]) + 127) // 128 for t in sm_segs)

    meta = dict(nblk_lo=nblk_lo, nblk_hi=nblk_hi, nblk=nblk_lo + nblk_hi,
                nblk_sm=nblk_sm)

    w1a, b1a = _wext(inputs["W1a"], inputs["a1a_src"], inputs["a1a_dst"],
                     inputs["b1a"], 256)
    w1b, b1b = _wext(inputs["W1b"], inputs["a1b_src"], inputs["a1b_dst"],
                     inputs["b1b"], 256)
    w2a, b2a = _wext(inputs["W2a"], inputs["a2a_src"], inputs["a2a_dst"],
                     inputs["b2a"], 64)
    w2b, b2b = _wext(inputs["W2b"], inputs["a2b_src"], inputs["a2b_dst"],
                     inputs["b2b"], 64)

    i_sm, _, dm_sm, dg_sm = _flatten(sm_segs, nblk_sm, 0, G // 128)

    # [b1a(0:256)|b1b(256:512)|b2a(512:576)|b2b(576:640)|
    #  ln1g(640:768)|ln1b(768:896)|ln2g(896:928)|ln2b(928:960)]
    rowcat = np.concatenate([
        b1a, b1b, b2a, b2b,
        np.asarray(inputs["ln1_g"], f32), np.asarray(inputs["ln1_b"], f32),
        np.asarray(inputs["ln2_g"], f32), np.asarray(inputs["ln2_b"], f32)])
    rowcat16 = np.ascontiguousarray(
        np.broadcast_to(rowcat[None, :], (16, rowcat.shape[0])))

    shared = dict(wext1a=np.asarray(w1a, bf16))
    # identical-on-every-core arrays are uploaded as 1/8-row shards and
    # AllGathered on device
    i_smw = _wrap16(i_sm)
    dg_smw = _wrap16(dg_sm)
    sh_slices = dict(wext1b=w1b, wext2a=w2a, wext2b=w2b, rowcat=rowcat16,
                     idx_smc=i_smw, dstl_sm=dm_sm, dglob_smc=dg_smw)

    per_core = []
    for c in range(NCORES):
        ilo, ihi, dmod, dglob = _flatten(all_segs[c], nblk_lo, nblk_hi, NT)
        gown = np.concatenate([gidx[c * NPER:(c + 1) * NPER],
                               np.zeros(NPAD - NPER, np.int64)])
        x1sh = np.zeros((P, NPAD), f32)
        x1sh[:, :NPER] = x1[c * NPER:(c + 1) * NPER].T
        shsh = {k + "_sh": np.ascontiguousarray(
                    a[c * (a.shape[0] // 8):(c + 1) * (a.shape[0] // 8)])
                for k, a in sh_slices.items()}
        per_core.append(dict(
            **shsh,
            idx_loc=_wrap16(ilo), idx_hic=_wrap16(ihi),
            dstl=dmod, dglobc=_wrap16(dglob), gidxc=_wrap16(gown),
            x1Tsh=np.asarray(x1sh, bf16),
            x2Tsh=np.ascontiguousarray(x2[c * 128:(c + 1) * 128].T),
            Absh=np.asarray(Ap[c * 128:(c + 1) * 128], bf16),
        ))
    return shared, per_core, meta


# --------------------------------------------------------------------------
# device program
# --------------------------------------------------------------------------

def build_nc(meta):
    import contextlib
    from concourse import bacc, mybir
    from concourse.tile import TileContext
    from concourse.bass import ds, ts

    f32 = mybir.dt.float32
    f32r = mybir.dt.float32r
    bf16 = mybir.dt.bfloat16
    i16 = mybir.dt.int16
    i32 = mybir.dt.int32
    u8 = mybir.dt.uint8
    Alu = mybir.AluOpType
    Act = mybir.ActivationFunctionType
    Ax = mybir.AxisListType

    NBLK = meta["nblk"]
    NBLK_LO = meta["nblk_lo"]
    NBLK_HI = meta["nblk_hi"]
    NBLK_SM = meta["nblk_sm"]

    nc = bacc.Bacc(None, target_bir_lowering=False, debug=True)

    dp = lambda n, s, d: nc.declare_dram_parameter(n, list(s), d, isOutput=False)
    x1Tsh_d = dp("x1Tsh", [P, NPAD], bf16)
    x2Tsh_d = dp("x2Tsh", [P, P], f32r)
    Absh_d = dp("Absh", [P, G], bf16)
    wext1a_d = dp("wext1a", [P, 256], bf16)
    wext1b_sh_d = dp("wext1b_sh", [16, 256], f32r)
    wext2a_sh_d = dp("wext2a_sh", [16, 64], f32r)
    wext2b_sh_d = dp("wext2b_sh", [16, 64], f32r)
    rowcat_sh_d = dp("rowcat_sh", [2, 960], f32)
    idx_smc_sh_d = dp("idx_smc_sh", [2, 8 * NBLK_SM * 8], i16)
    dstl_sm_sh_d = dp("dstl_sm_sh", [16, 8 * NBLK_SM], u8)
    dglob_smc_sh_d = dp("dglob_smc_sh", [2, 8 * NBLK_SM * 8], i16)
    idx_loc_d = dp("idx_loc", [16, NT * NBLK_LO * 8], i16)
    idx_hic_d = dp("idx_hic", [16, NT * NBLK_HI * 8], i16)
    dstl_d = dp("dstl", [P, NT * NBLK], u8)
    dglobc_d = dp("dglobc", [16, NT * NBLK * 8], i16)
    gidxc_d = dp("gidxc", [16, NT * 8], i16)

    out_d = nc.declare_dram_parameter("out", [NPAD, NCLS], bf16, isOutput=True)

    # AllGather-assembled full tensors (collectives cannot read IO tensors
    # directly, so shards are staged into internal DRAM first)
    x1st_d = nc.dram_tensor("x1st", [P, NPAD], bf16)
    x2st_d = nc.dram_tensor("x2st", [P, P], f32r)
    Abst_d = nc.dram_tensor("Abst", [P, G], bf16)
    shstage = {}
    shfull = {}
    for nm, dram, full_rows in [
            ("wext1b", wext1b_sh_d, P), ("wext2a", wext2a_sh_d, P),
            ("wext2b", wext2b_sh_d, P), ("rowcat", rowcat_sh_d, 16),
            ("idx_smc", idx_smc_sh_d, 16), ("dstl_sm", dstl_sm_sh_d, P),
            ("dglob_smc", dglob_smc_sh_d, 16)]:
        shp = list(dram.shape)
        shstage[nm] = nc.dram_tensor(nm + "_st", shp, dram.dtype)
        shfull[nm] = nc.dram_tensor(nm + "_G", [full_rows, shp[1]],
                                    dram.dtype, addr_space="Shared")
    x1TG_d = nc.dram_tensor("x1TG", [8 * P, NPAD], bf16, addr_space="Shared")
    x2TG_d = nc.dram_tensor("x2TG", [8 * P, P], f32r, addr_space="Shared")
    AbG_d = nc.dram_tensor("AbG", [G, G], bf16, addr_space="Shared")
    # full-layout (8x replicated) gather index tables, built on device
    idx_lo_d = nc.dram_tensor("idx_lo", [P, NT * NBLK_LO * 8], i16)
    idx_hi_d = nc.dram_tensor("idx_hi", [P, NT * NBLK_HI * 8], i16)
    dglob_d = nc.dram_tensor("dglob", [P, NT * NBLK * 8], i16)

    tab1_d = nc.dram_tensor("tab1", [NG, TAB1_COLS], f32)
    vtab1_d = nc.dram_tensor("vtab1", [NPAD, VT_COLS], f32)
    smtab1_d = nc.dram_tensor("smtab1", [G, TAB1_COLS], f32)
    smvtab1_d = nc.dram_tensor("smvtab1", [G, VT_COLS], f32)
    tab2own_d = nc.dram_tensor("tab2own", [NPAD, TAB2_COLS], f32)
    tab2_d = nc.dram_tensor("tab2", [NG, TAB2_COLS], f32, addr_space="Shared")
    vtab2_d = nc.dram_tensor("vtab2", [NPAD, VT_COLS], f32)
    smtab2_d = nc.dram_tensor("smtab2", [G, TAB2_COLS], f32)
    smvtab2_d = nc.dram_tensor("smvtab2", [G, VT_COLS], f32)

    with TileContext(nc) as tc, contextlib.ExitStack() as ctx:
        pool = ctx.enter_context(tc.tile_pool(name="main", bufs=2))
        cpool = ctx.enter_context(tc.tile_pool(name="consts", bufs=1))
        spool = ctx.enter_context(tc.tile_pool(name="stash", bufs=1))
        gpool = ctx.enter_context(tc.tile_pool(name="gather", bufs=1))
        qpool = ctx.enter_context(tc.tile_pool(name="q", bufs=1))
        ppool = ctx.enter_context(tc.tile_pool(name="psA", bufs=2, space="PSUM"))
        npool = ctx.enter_context(tc.tile_pool(name="psN", bufs=2, space="PSUM"))
        tpool = ctx.enter_context(tc.tile_pool(name="psT", bufs=2, space="PSUM"))
        spsum = ctx.enter_context(tc.tile_pool(name="psS", bufs=1, space="PSUM"))

        # ---- AllGathers: assemble full x1T / x2T / A on device ----
        nc.sync.dma_start(out=x1st_d[:], in_=x1Tsh_d[:])
        nc.sync.dma_start(out=Abst_d[:], in_=Absh_d[:])
        nc.sync.dma_start(out=x2st_d[:], in_=x2Tsh_d[:])
        nc.gpsimd.collective_compute(
            "AllGather", Alu.bypass, replica_groups=[list(range(NCORES))],
            ins=[x1st_d[:]], outs=[x1TG_d[:]])
        nc.gpsimd.collective_compute(
            "AllGather", Alu.bypass, replica_groups=[list(range(NCORES))],
            ins=[Abst_d[:]], outs=[AbG_d[:]])
        nc.gpsimd.collective_compute(
            "AllGather", Alu.bypass, replica_groups=[list(range(NCORES))],
            ins=[x2st_d[:]], outs=[x2TG_d[:]])
        for nm, dram in [("wext1b", wext1b_sh_d), ("wext2a", wext2a_sh_d),
                         ("wext2b", wext2b_sh_d), ("rowcat", rowcat_sh_d),
                         ("idx_smc", idx_smc_sh_d), ("dstl_sm", dstl_sm_sh_d),
                         ("dglob_smc", dglob_smc_sh_d)]:
            nc.sync.dma_start(out=shstage[nm][:], in_=dram[:])
            nc.gpsimd.collective_compute(
                "AllGather", Alu.bypass, replica_groups=[list(range(NCORES))],
                ins=[shstage[nm][:]], outs=[shfull[nm][:]])

        # ---- replicate compact idx tables to full 128-partition layout ----
        for g in range(8):
            nc.sync.dma_start(out=idx_lo_d[16 * g:16 * (g + 1), :],
                              in_=idx_loc_d[:])
            nc.sync.dma_start(out=idx_hi_d[16 * g:16 * (g + 1), :],
                              in_=idx_hic_d[:])
            nc.sync.dma_start(out=dglob_d[16 * g:16 * (g + 1), :],
                              in_=dglobc_d[:])

        def load_const(dram, shape, dtype, tag):
            t = cpool.tile(shape, dtype, tag=tag)
            nc.sync.dma_start(out=t[:], in_=dram[:])
            return t

        def load_rep16(dram, cols, dtype, tag):
            """[16, cols] DRAM -> [128, cols] SBUF, replicated 8x."""
            t = cpool.tile([P, cols], dtype, tag=tag)
            for g in range(8):
                nc.sync.dma_start(out=t[16 * g:16 * (g + 1), :], in_=dram[:])
            return t

        # iota row / per-partition index / identity, generated on device
        iotaI = cpool.tile([P, P], i32, tag="iotaI")
        nc.gpsimd.iota(iotaI[:], pattern=[[1, P]], base=0, channel_multiplier=0)
        iota_s = cpool.tile([P, P], f32, tag="iota")
        nc.vector.tensor_copy(out=iota_s[:], in_=iotaI[:])
        iotaPI = cpool.tile([P, 1], i32, tag="iotaPI")
        nc.gpsimd.iota(iotaPI[:], pattern=[[0, 1]], base=0, channel_multiplier=1)
        iotaP_s = cpool.tile([P, 1], f32, tag="iotaP")
        nc.vector.tensor_copy(out=iotaP_s[:], in_=iotaPI[:])
        ident_s = cpool.tile([P, P], f32, tag="ident")
        nc.vector.tensor_scalar(out=ident_s[:], in0=iota_s[:],
                                scalar1=iotaP_s[:, 0:1], scalar2=None,
                                op0=Alu.is_equal)

        wext1a_s = load_const(wext1a_d, [P, 256], bf16, "wext1a")
        wext1b_s = load_const(shfull["wext1b"], [P, 256], f32r, "wext1b")
        wext2a_s = load_const(shfull["wext2a"], [P, 64], f32r, "wext2a")
        wext2b_s = load_const(shfull["wext2b"], [P, 64], f32r, "wext2b")
        rc_s = load_rep16(shfull["rowcat"], 960, f32, "rowcat")
        brep1a_s = rc_s[:, 0:256]
        brep1b_s = rc_s[:, 256:512]
        brep2a_s = rc_s[:, 512:576]
        brep2b_s = rc_s[:, 576:640]
        g1rep_s = rc_s[:, 640:768]
        b1rep_s = rc_s[:, 768:896]
        g2rep_s = rc_s[:, 896:928]
        b2rep_s = rc_s[:, 928:960]

        idxsm_s = load_rep16(shfull["idx_smc"], 8 * NBLK_SM * 8, i16, "idxsm")
        dglobsm_s = load_rep16(shfull["dglob_smc"], 8 * NBLK_SM * 8, i16,
                               "dglobsm")
        gidx_s = load_rep16(gidxc_d, NT * 8, i16, "gidx")

        def load_u8_as_f32(dram, cols, tag):
            tb = pool.tile([P, cols], u8, tag=f"{tag}_u8")
            nc.sync.dma_start(out=tb[:], in_=dram[:])
            t = cpool.tile([P, cols], f32, tag=tag)
            nc.vector.tensor_copy(out=t[:], in_=tb[:])
            return t

        dstlsm_s = load_u8_as_f32(shfull["dstl_sm"], 8 * NBLK_SM, "dstlsm")
        dstl_s = load_u8_as_f32(dstl_d, NT * NBLK, "dstl")

        # pre-zero the q-slots so junk pad columns of the fp32r rhs are finite
        zq = qpool.tile([P, NBLK, 256], f32r, tag="q256")
        nc.vector.memset(zq[:].rearrange("p a b -> p (a b)").bitcast(f32), 0.0)
        zq = qpool.tile([P, max(NBLK, NBLK_SM), 40], f32r, tag="q33")
        nc.vector.memset(zq[:].rearrange("p a b -> p (a b)").bitcast(f32), 0.0)

        # ---------------- phase 1: tables ----------------
        # global tab1 (8 sections x 49 tiles) from AllGathered x1TG, bf16
        with tc.For_i(0, NT, 1) as t:
            for sec in range(NCORES):
                lhsb = pool.tile([P, 128], bf16, tag="tb_lhsb")
                nc.sync.dma_start(out=lhsb[:],
                                  in_=x1TG_d[sec * P:(sec + 1) * P,
                                             ts(t, 128)])
                ps = ppool.tile([P, 256], f32, tag="agg", space="PSUM")
                nc.tensor.matmul(out=ps[:], lhsT=lhsb[:], rhs=wext1a_s[:],
                                 start=True, stop=True)
                st = pool.tile([P, 256], f32, tag="tb_st")
                nc.vector.scalar_tensor_tensor(
                    out=st[:], in0=ps[:], scalar=1.0,
                    in1=brep1a_s[:], op0=Alu.bypass, op1=Alu.add)
                nc.scalar.activation(st[:, 132:136], st[:, 128:132],
                                     Act.Exp, scale=0.2)
                nc.scalar.activation(st[:, 128:132], st[:, 128:132], Act.Exp)
                nc.sync.dma_start(
                    out=tab1_d[ds(t * 128 + sec * NPAD, 128), :],
                    in_=st[:, 0:TAB1_COLS])

        # small-graph tables from AllGathered x2TG (sections == tiles)
        with tc.For_i(0, 8, 1) as t:
            lhs = pool.tile([P, 128], f32r, tag="sm_lhs")
            nc.sync.dma_start(out=lhs[:], in_=x2TG_d[ts(t, P), :])
            ps = ppool.tile([P, 256], f32, tag="agg", space="PSUM")
            nc.tensor.matmul(out=ps[:], lhsT=lhs[:], rhs=wext1b_s[:],
                             start=True, stop=True)
            st = pool.tile([P, 256], f32, tag="tb_st")
            nc.vector.scalar_tensor_tensor(
                out=st[:], in0=ps[:], scalar=1.0,
                in1=brep1b_s[:], op0=Alu.bypass, op1=Alu.add)
            vst = pool.tile([P, 8], f32, tag="vt_vst")
            nc.scalar.activation(vst[:, 0:4], st[:, 132:136], Act.Exp)
            nc.scalar.activation(vst[:, 4:8], st[:, 132:136], Act.Exp,
                                 scale=0.2)
            nc.sync.dma_start(out=smvtab1_d[ts(t, 128), 0:8], in_=vst[:])
            nc.scalar.activation(st[:, 132:136], st[:, 128:132],
                                 Act.Exp, scale=0.2)
            nc.scalar.activation(st[:, 128:132], st[:, 128:132], Act.Exp)
            nc.sync.dma_start(out=smtab1_d[ts(t, 128), :],
                              in_=st[:, 0:TAB1_COLS])

        # ---------------- edge aggregation (loop body helper) ----------------
        def edge_gat_body(t, tab_dram, vtab_dram, idxlo_src, idxhi_src,
                          dstl_src, dglob_src, nblk, nblk_lo, F, H, rhs_n,
                          idx_in_sbuf, tag):
            """Emits ops for dst-tile t (loop var); returns agg psum
            [(numer F) | (s H)]."""
            tabcols = TAB1_COLS if F == 128 else TAB2_COLS
            gtag = f"g{tabcols}"
            qtag = "q256" if F == 128 else "q33"
            nblk_hi = nblk - nblk_lo
            gt = gpool.tile([P, nblk, tabcols], f32, tag=gtag)
            for g0 in range(0, nblk_lo, GCAP):
                g1 = min(g0 + GCAP, nblk_lo)
                if idx_in_sbuf:
                    iap = idxlo_src[:, ds(t * nblk_lo * 8 + g0 * 8,
                                          (g1 - g0) * 8)]
                else:
                    it = pool.tile([P, (g1 - g0) * 8], i16, tag=f"{tag}_il{g0}")
                    nc.sync.dma_start(
                        out=it[:], in_=idxlo_src[:, ds(t * nblk_lo * 8 + g0 * 8,
                                                       (g1 - g0) * 8)])
                    iap = it[:]
                nc.gpsimd.dma_gather(
                    out_ap=gt[:, g0:g1, :], in_ap=tab_dram[:],
                    idxs_ap=iap, num_idxs=(g1 - g0) * 128,
                    num_idxs_reg=(g1 - g0) * 128, elem_size=tabcols)
            for g0 in range(0, nblk_hi, GCAP):
                g1 = min(g0 + GCAP, nblk_hi)
                if idx_in_sbuf:
                    iap = idxhi_src[:, ds(t * nblk_hi * 8 + g0 * 8,
                                          (g1 - g0) * 8)]
                else:
                    it = pool.tile([P, (g1 - g0) * 8], i16, tag=f"{tag}_ih{g0}")
                    nc.sync.dma_start(
                        out=it[:], in_=idxhi_src[:, ds(t * nblk_hi * 8 + g0 * 8,
                                                       (g1 - g0) * 8)])
                    iap = it[:]
                nc.gpsimd.dma_gather(
                    out_ap=gt[:, nblk_lo + g0:nblk_lo + g1, :],
                    in_ap=tab_dram[SPLIT:, :],
                    idxs_ap=iap, num_idxs=(g1 - g0) * 128,
                    num_idxs_reg=(g1 - g0) * 128, elem_size=tabcols)
            vt = gpool.tile([P, nblk, VT_COLS], f32, tag="v64")
            for g0 in range(0, nblk, GCAP):
                g1 = min(g0 + GCAP, nblk)
                if idx_in_sbuf:
                    iap = dglob_src[:, ds(t * nblk * 8 + g0 * 8, (g1 - g0) * 8)]
                else:
                    it = pool.tile([P, (g1 - g0) * 8], i16, tag=f"{tag}_dg{g0}")
                    nc.sync.dma_start(
                        out=it[:], in_=dglob_src[:, ds(t * nblk * 8 + g0 * 8,
                                                       (g1 - g0) * 8)])
                    iap = it[:]
                nc.gpsimd.dma_gather(
                    out_ap=vt[:, g0:g1, :], in_ap=vtab_dram[:],
                    idxs_ap=iap, num_idxs=(g1 - g0) * 128,
                    num_idxs_reg=(g1 - g0) * 128, elem_size=VT_COLS)
            dl = pool.tile([P, nblk], f32, tag=f"{tag}_dl")
            nc.vector.tensor_copy(out=dl[:], in_=dstl_src[:, ts(t, nblk)])
            mask = qpool.tile([P, nblk, 128], f32r, tag="mask")
            nc.vector.tensor_tensor(
                out=mask[:],
                in0=iota_s[:][:, None, :].to_broadcast([P, nblk, 128]),
                in1=dl[:][:, :, None].to_broadcast([P, nblk, 128]),
                op=Alu.is_equal)
            q = qpool.tile([P, nblk, rhs_n], f32r, tag=qtag)
            m1 = pool.tile([P, nblk, H], f32, tag="pm1")
            m2 = pool.tile([P, nblk, H], f32, tag="pm2")
            nc.vector.tensor_tensor(out=m1[:], in0=gt[:, :, F:F + H],
                                    in1=vt[:, :, 0:H], op=Alu.mult)
            nc.vector.tensor_tensor(out=m2[:], in0=gt[:, :, F + H:F + 2 * H],
                                    in1=vt[:, :, H:2 * H], op=Alu.mult)
            nc.vector.tensor_tensor(out=q[:, :, F:F + H], in0=m1[:],
                                    in1=m2[:], op=Alu.max)
            C = F // H
            for h in range(H):
                nc.vector.tensor_tensor(
                    out=q[:, :, h * C:(h + 1) * C],
                    in0=gt[:, :, h * C:(h + 1) * C],
                    in1=q[:, :, F + h:F + h + 1].to_broadcast([P, nblk, C]),
                    op=Alu.mult)
            ps = ppool.tile([P, 256], f32, tag="agg", space="PSUM")
            for b in range(nblk):
                nc.tensor.matmul(
                    out=ps[:, 0:rhs_n], lhsT=mask[:, b, :], rhs=q[:, b, :],
                    start=(b == 0), stop=(b == nblk - 1))
            return ps

        def xout_from_ps(ps, F, H, brep_s, tag):
            rec = pool.tile([P, H], f32, tag=f"{tag}_rec")
            nc.vector.reciprocal(out=rec[:], in_=ps[:, F:F + H])
            xo = pool.tile([P, F], f32, tag=f"{tag}_xo")
            C = F // H
            for h in range(H):
                nc.vector.tensor_scalar(
                    out=xo[:, h * C:(h + 1) * C], in0=ps[:, h * C:(h + 1) * C],
                    scalar1=rec[:, h:h + 1], scalar2=None, op0=Alu.mult)
            nc.vector.tensor_tensor(out=xo[:], in0=xo[:], in1=brep_s[:, 0:F],
                                    op=Alu.add)
            return xo

        # ---------------- group attention (loop body helper) ----------------
        def group_attn(t, xo, X2pT_ap, X2ext_all, Fs, rhs_n, tag):
            """Returns 0.5*grp tile [P, Fs] f32."""
            pt = tpool.tile([P, 128], f32, tag="ptr", space="PSUM")
            nc.tensor.transpose(out=pt[:Fs, :], in_=xo[:, 0:Fs],
                                identity=ident_s[:])
            xT = pool.tile([P, 128], f32r, tag="ga_xT")
            nc.scalar.copy(out=xT[:Fs, :], in_=pt[:Fs, :])
            pss = spsum.tile([P, 1024], f32, tag="s", space="PSUM")
            nc.tensor.matmul(out=pss[:, 0:512], lhsT=xT[:Fs, :],
                             rhs=X2pT_ap[:, 0:512], start=True, stop=True)
            nc.tensor.matmul(out=pss[:, 512:1024], lhsT=xT[:Fs, :],
                             rhs=X2pT_ap[:, 512:1024], start=True, stop=True)
            mx0 = pool.tile([P, 1], f32, tag="ga_mx0")
            mx1 = pool.tile([P, 1], f32, tag="ga_mx1")
            nc.vector.reduce_max(mx0[:], pss[:, 0:512], axis=Ax.X)
            nc.vector.reduce_max(mx1[:], pss[:, 512:1024], axis=Ax.X)
            negmx = pool.tile([P, 1], f32, tag="ga_negmx")
            nc.vector.tensor_tensor(out=negmx[:], in0=mx0[:], in1=mx1[:],
                                    op=Alu.max)
            nc.vector.tensor_scalar(out=negmx[:], in0=negmx[:], scalar1=-1.0,
                                    scalar2=None, op0=Alu.mult)
            wx = pool.tile([P, G], f32, tag="ga_wx")
            nc.scalar.activation(wx[:, 0:512], pss[:, 0:512], Act.Exp,
                                 bias=negmx[:])
            nc.scalar.activation(wx[:, 512:1024], pss[:, 512:1024], Act.Exp,
                                 bias=negmx[:])
            at = pool.tile([P, 8, 128], bf16, tag="ga_at")
            nc.gpsimd.dma_gather(
                out_ap=at[:], in_ap=AbG_d[:], idxs_ap=gidx_s[:, ts(t, 8)],
                num_idxs=128, num_idxs_reg=128, elem_size=G, transpose=True)
            psn = npool.tile([P, 256], f32, tag="num", space="PSUM")
            for j in range(8):
                wt = tpool.tile([P, 128], f32, tag="ptr", space="PSUM")
                nc.tensor.transpose(out=wt[:], in_=wx[:, j * 128:(j + 1) * 128],
                                    identity=ident_s[:])
                bmt = pool.tile([P, 128], f32r, tag="ga_bmt")
                nc.vector.scalar_tensor_tensor(
                    out=bmt[:], in0=wt[:], scalar=1.0, in1=at[:, j, :],
                    op0=Alu.bypass, op1=Alu.mult)
                nc.tensor.matmul(out=psn[:, 0:rhs_n], lhsT=bmt[:],
                                 rhs=X2ext_all[:, j, :], start=(j == 0),
                                 stop=(j == 7))
            rec = pool.tile([P, 1], f32, tag="ga_grec")
            nc.vector.reciprocal(out=rec[:], in_=psn[:, Fs:Fs + 1])
            grp = pool.tile([P, Fs], f32, tag="ga_grp")
            nc.vector.tensor_scalar(out=grp[:], in0=psn[:, 0:Fs],
                                    scalar1=rec[:], scalar2=0.5, op0=Alu.mult,
                                    op1=Alu.mult)
            return grp

        # ================= small-graph GAT layer 1 =================
        xg1_all = spool.tile([P, 8, 128], f32, tag="xg1")
        X2pT = cpool.tile([P, G], f32r, tag="X2pT")
        X2ext_all = spool.tile([P, 8, 256], f32r, tag="X2ext")
        nc.vector.memset(
            X2ext_all[:].rearrange("p a b -> p (a b)").bitcast(f32), 0.0)
        nc.vector.memset(X2ext_all[:, :, 128:129].bitcast(f32), 1.0)
        with tc.For_i(0, 8, 1) as t:
            ps = edge_gat_body(
                t, smtab1_d, smvtab1_d, idxsm_s, None,
                dstlsm_s, dglobsm_s,
                NBLK_SM, NBLK_SM, 128, 4, 256, True, "sg1")
            xo = xout_from_ps(ps, 128, 4, brep1b_s, "sm1")
            nc.vector.tensor_copy(out=xg1_all[:, t, :], in_=xo[:])
            pt = tpool.tile([P, 128], f32, tag="ptr", space="PSUM")
            nc.tensor.transpose(out=pt[:], in_=xo[:], identity=ident_s[:])
            nc.scalar.copy(out=X2pT[:, ts(t, 128)], in_=pt[:])
            nc.scalar.copy(out=X2ext_all[:, t, 0:128], in_=xo[:])

        # ================= big-graph layer 1 =================
        var49 = cpool.tile([P, NT], f32, tag="var49")
        s1_all = spool.tile([P, NT, 128], f32, tag="s1")
        with tc.For_i(0, NT, 1) as t:
            # build this tile's v-table rows (dsts of tile t are within tile t)
            lhsb = pool.tile([P, 128], bf16, tag="vt_lhsb")
            nc.sync.dma_start(out=lhsb[:], in_=x1Tsh_d[:, ts(t, 128)])
            psv = npool.tile([P, 256], f32, tag="num", space="PSUM")
            nc.tensor.matmul(out=psv[:, 0:8], lhsT=lhsb[:],
                             rhs=wext1a_s[:, 128:136], start=True, stop=True)
            stv = pool.tile([P, 8], f32, tag="vt_st")
            nc.vector.scalar_tensor_tensor(
                out=stv[:], in0=psv[:, 0:8], scalar=1.0,
                in1=rc_s[:, 128:136], op0=Alu.bypass, op1=Alu.add)
            vst = pool.tile([P, 8], f32, tag="vt_vst")
            nc.scalar.activation(vst[:, 0:4], stv[:, 4:8], Act.Exp)
            nc.scalar.activation(vst[:, 4:8], stv[:, 4:8], Act.Exp, scale=0.2)
            nc.sync.dma_start(out=vtab1_d[ts(t, 128), 0:8], in_=vst[:])
            ps = edge_gat_body(
                t, tab1_d, vtab1_d, idx_lo_d, idx_hi_d,
                dstl_s, dglob_d,
                NBLK, NBLK_LO, 128, 4, 256, False, "bg1")
            xo = xout_from_ps(ps, 128, 4, brep1a_s, "b1")
            grp = group_attn(t, xo, X2pT[:], X2ext_all, 128, 256, "g1")
            s1 = pool.tile([P, 128], f32, tag="b1_s1")
            nc.vector.scalar_tensor_tensor(out=s1[:], in0=xo[:], scalar=0.5,
                                           in1=grp[:], op0=Alu.mult, op1=Alu.add)
            mu = pool.tile([P, 1], f32, tag="b1_mu")
            nc.vector.tensor_reduce(out=mu[:], in_=s1[:], axis=Ax.X, op=Alu.add)
            nc.vector.tensor_scalar(out=mu[:], in0=mu[:], scalar1=-1.0 / 128,
                                    scalar2=None, op0=Alu.mult)
            nc.vector.tensor_scalar(out=s1[:], in0=s1[:], scalar1=mu[:],
                                    scalar2=None, op0=Alu.add)
            nc.vector.tensor_copy(out=s1_all[:, t, :], in_=s1[:])
            sq = pool.tile([P, 128], f32, tag="b1_sq")
            nc.vector.tensor_tensor(out=sq[:], in0=s1[:], in1=s1[:], op=Alu.mult)
            nc.vector.tensor_reduce(out=var49[:, ds(t, 1)], in_=sq[:], axis=Ax.X,
                                    op=Alu.add)

        sd49 = cpool.tile([P, NT], f32, tag="sd49")
        nc.vector.tensor_scalar(out=sd49[:], in0=var49[:], scalar1=1.0 / 128,
                                scalar2=LN_EPS, op0=Alu.mult, op1=Alu.add)
        sq49 = cpool.tile([P, NT], f32, tag="sq49")
        nc.scalar.activation(sq49[:], sd49[:], Act.Sqrt)
        rstd49 = cpool.tile([P, NT], f32, tag="rstd49")
        nc.vector.reciprocal(out=rstd49[:], in_=sq49[:])

        with tc.For_i(0, NT, 1) as t:
            s1 = pool.tile([P, 128], f32, tag="l1_s1")
            nc.vector.tensor_copy(out=s1[:], in_=s1_all[:, t, :])
            rs = pool.tile([P, 1], f32, tag="l1_rs")
            nc.vector.tensor_copy(out=rs[:], in_=rstd49[:, ds(t, 1)])
            y = pool.tile([P, 128], f32, tag="b1_y")
            nc.vector.scalar_tensor_tensor(
                out=y[:], in0=s1[:], scalar=rs[:], in1=g1rep_s[:],
                op0=Alu.mult, op1=Alu.mult)
            nc.vector.tensor_tensor(out=y[:], in0=y[:], in1=b1rep_s[:],
                                    op=Alu.add)
            emin = pool.tile([P, 128], f32, tag="b1_emin")
            nc.vector.tensor_scalar(out=emin[:], in0=y[:], scalar1=0.0,
                                    scalar2=None, op0=Alu.min)
            nc.scalar.activation(emin[:], emin[:], Act.Exp)
            h1 = pool.tile([P, 128], f32, tag="b1_h1")
            nc.vector.tensor_scalar(out=h1[:], in0=y[:], scalar1=0.0,
                                    scalar2=-1.0, op0=Alu.max, op1=Alu.add)
            nc.vector.tensor_tensor(out=h1[:], in0=h1[:], in1=emin[:], op=Alu.add)
            pt = tpool.tile([P, 128], f32, tag="ptr", space="PSUM")
            nc.tensor.transpose(out=pt[:], in_=h1[:], identity=ident_s[:])
            h1T = pool.tile([P, 128], f32r, tag="b1_h1T")
            nc.scalar.copy(out=h1T[:], in_=pt[:])
            ps2 = npool.tile([P, 256], f32, tag="num", space="PSUM")
            nc.tensor.matmul(out=ps2[:, 0:64], lhsT=h1T[:], rhs=wext2a_s[:],
                             start=True, stop=True)
            st2 = pool.tile([P, 64], f32, tag="b1_st2")
            nc.vector.scalar_tensor_tensor(
                out=st2[:], in0=ps2[:, 0:64], scalar=1.0, in1=brep2a_s[:],
                op0=Alu.bypass, op1=Alu.add)
            vst = pool.tile([P, 2], f32, tag="b1_vst")
            nc.scalar.activation(vst[:, 0:1], st2[:, 33:34], Act.Exp)
            nc.scalar.activation(vst[:, 1:2], st2[:, 33:34], Act.Exp, scale=0.2)
            nc.scalar.activation(st2[:, 33:34], st2[:, 32:33], Act.Exp, scale=0.2)
            nc.scalar.activation(st2[:, 32:33], st2[:, 32:33], Act.Exp)
            nc.sync.dma_start(out=tab2own_d[ts(t, 128), :], in_=st2[:])
            nc.sync.dma_start(out=vtab2_d[ts(t, 128), 0:2], in_=vst[:, 0:2])

        nc.gpsimd.collective_compute(
            "AllGather", Alu.bypass, replica_groups=[list(range(NCORES))],
            ins=[tab2own_d[:]], outs=[tab2_d[:]])

        # ================= small-graph layer 2 =================
        with tc.For_i(0, 8, 1) as t:
            xg = pool.tile([P, 128], f32, tag="ts2_xg")
            nc.vector.tensor_copy(out=xg[:], in_=xg1_all[:, t, :])
            pt = tpool.tile([P, 128], f32, tag="ptr", space="PSUM")
            nc.tensor.transpose(out=pt[:], in_=xg[:], identity=ident_s[:])
            xT = pool.tile([P, 128], f32r, tag="ts2_xT")
            nc.scalar.copy(out=xT[:], in_=pt[:])
            ps2 = npool.tile([P, 256], f32, tag="num", space="PSUM")
            nc.tensor.matmul(out=ps2[:, 0:64], lhsT=xT[:], rhs=wext2b_s[:],
                             start=True, stop=True)
            st2 = pool.tile([P, 64], f32, tag="ts2_st")
            nc.vector.scalar_tensor_tensor(
                out=st2[:], in0=ps2[:, 0:64], scalar=1.0, in1=brep2b_s[:],
                op0=Alu.bypass, op1=Alu.add)
            vst = pool.tile([P, 2], f32, tag="ts2_vst")
            nc.scalar.activation(vst[:, 0:1], st2[:, 33:34], Act.Exp)
            nc.scalar.activation(vst[:, 1:2], st2[:, 33:34], Act.Exp, scale=0.2)
            nc.scalar.activation(st2[:, 33:34], st2[:, 32:33], Act.Exp, scale=0.2)
            nc.scalar.activation(st2[:, 32:33], st2[:, 32:33], Act.Exp)
            nc.sync.dma_start(out=smtab2_d[ts(t, 128), :], in_=st2[:])
            nc.sync.dma_start(out=smvtab2_d[ts(t, 128), 0:2], in_=vst[:, 0:2])

        xg2_all = spool.tile([P, 8, 32], f32, tag="xg2")
        X2p2T = cpool.tile([32, G], f32r, tag="X2p2T")
        X2ext2_all = spool.tile([P, 8, 40], f32r, tag="X2ext2")
        nc.vector.memset(
            X2ext2_all[:].rearrange("p a b -> p (a b)").bitcast(f32), 0.0)
        nc.vector.memset(X2ext2_all[:, :, 32:33].bitcast(f32), 1.0)
        with tc.For_i(0, 8, 1) as t:
            ps = edge_gat_body(
                t, smtab2_d, smvtab2_d, idxsm_s, None,
                dstlsm_s, dglobsm_s,
                NBLK_SM, NBLK_SM, 32, 1, 40, True, "sg2")
            xo = xout_from_ps(ps, 32, 1, brep2b_s, "sm2")
            nc.vector.tensor_copy(out=xg2_all[:, t, :], in_=xo[:])
            pt = tpool.tile([P, 128], f32, tag="ptr", space="PSUM")
            nc.tensor.transpose(out=pt[:32, :], in_=xo[:], identity=ident_s[:])
            nc.scalar.copy(out=X2p2T[:, ts(t, 128)], in_=pt[:32, :])
            nc.scalar.copy(out=X2ext2_all[:, t, 0:32], in_=xo[:])

        # ================= big-graph layer 2 =================
        var49b = cpool.tile([P, NT], f32, tag="var49b")
        o_all = spool.tile([P, NT, 32], f32, tag="o")
        with tc.For_i(0, NT, 1) as t:
            ps = edge_gat_body(
                t, tab2_d, vtab2_d, idx_lo_d, idx_hi_d,
                dstl_s, dglob_d,
                NBLK, NBLK_LO, 32, 1, 40, False, "bg2")
            xo = xout_from_ps(ps, 32, 1, brep2a_s, "b2")
            grp = group_attn(t, xo, X2p2T[:], X2ext2_all, 32, 40, "g2")
            o = pool.tile([P, 32], f32, tag="b2_o")
            nc.vector.scalar_tensor_tensor(out=o[:], in0=xo[:], scalar=0.5,
                                           in1=grp[:], op0=Alu.mult, op1=Alu.add)
            mu = pool.tile([P, 1], f32, tag="b2_mu")
            nc.vector.tensor_reduce(out=mu[:], in_=o[:], axis=Ax.X, op=Alu.add)
            nc.vector.tensor_scalar(out=mu[:], in0=mu[:], scalar1=-1.0 / 32,
                                    scalar2=None, op0=Alu.mult)
            nc.vector.tensor_scalar(out=o[:], in0=o[:], scalar1=mu[:],
                                    scalar2=None, op0=Alu.add)
            nc.vector.tensor_copy(out=o_all[:, t, :], in_=o[:])
            sq = pool.tile([P, 32], f32, tag="b2_sq")
            nc.vector.tensor_tensor(out=sq[:], in0=o[:], in1=o[:], op=Alu.mult)
            nc.vector.tensor_reduce(out=var49b[:, ds(t, 1)], in_=sq[:],
                                    axis=Ax.X, op=Alu.add)

        sd49b = cpool.tile([P, NT], f32, tag="sd49b")
        nc.vector.tensor_scalar(out=sd49b[:], in0=var49b[:], scalar1=1.0 / 32,
                                scalar2=LN_EPS, op0=Alu.mult, op1=Alu.add)
        sq49b = cpool.tile([P, NT], f32, tag="sq49b")
        nc.scalar.activation(sq49b[:], sd49b[:], Act.Sqrt)
        rstd49b = cpool.tile([P, NT], f32, tag="rstd49b")
        nc.vector.reciprocal(out=rstd49b[:], in_=sq49b[:])

        with tc.For_i(0, NT, 1) as t:
            o = pool.tile([P, 32], f32, tag="l2_o")
            nc.vector.tensor_copy(out=o[:], in_=o_all[:, t, :])
            rs = pool.tile([P, 1], f32, tag="l2_rs")
            nc.vector.tensor_copy(out=rs[:], in_=rstd49b[:, ds(t, 1)])
            y = pool.tile([P, 32], f32, tag="b2_y")
            nc.vector.scalar_tensor_tensor(
                out=y[:], in0=o[:], scalar=rs[:], in1=g2rep_s[:],
                op0=Alu.mult, op1=Alu.mult)
            nc.vector.tensor_tensor(out=y[:], in0=y[:], in1=b2rep_s[:],
                                    op=Alu.add)
            yb = pool.tile([P, 32], bf16, tag="b2_yb")
            nc.vector.tensor_copy(out=yb[:], in_=y[:])
            nc.sync.dma_start(out=out_d[ts(t, 128), :], in_=yb[:])

    nc.compile()
    return nc


# --------------------------------------------------------------------------
# entry point
# --------------------------------------------------------------------------

def kernel(**inputs):
    from concourse.bass_utils import run_bass_kernel_spmd

    shared, per_core, meta = host_prep(inputs)
    nc = build_nc(meta)
    in_maps = []
    for c in range(NCORES):
        m = dict(shared)
        m.update(per_core[c])
        in_maps.append(m)
    res = run_bass_kernel_spmd(nc, in_maps, list(range(NCORES)))
    out = np.concatenate([np.asarray(res.results[c]["out"])[:NPER]
                          for c in range(NCORES)])
    return out.astype(np.float32)


# revision 18
# speedup vs baseline: 3.0589x; 1.0955x over previous
"""Dual-GAT (nn_GAT_48017734369678) on 8 TRN2 NeuronCores via Bass/Tile.

Self-contained: host-side sharding/preprocessing in numpy, device program in
Bass (Tile), executed through run_bass_kernel_spmd on cores 0-7.

The dispatch cost here is dominated by (a) host->device upload bytes over the
axon tunnel (~50MB/s) and (b) STATIC instruction count in the NEFF (~45us per
instruction per dispatch). Both are minimized:
  (a) each core uploads only its own transposed x1 shard (bf16) / Ab rows /
      x2 rows; full copies are assembled on-device with AllGather. Gather
      index tables are uploaded compact ([16, n/16]) and replicated on device.
  (b) every per-tile stage is wrapped in a tc.For_i hardware loop with
      dynamic (register-offset) access patterns, so the program is ~600
      instructions instead of ~24000.

Per-core row spaces are padded to NPAD=6272=49*128 so all loops are uniform;
src node ids are remapped on host into the padded id space, and the padded
output rows are sliced off on host.

Edge aggregation: per-node gather tables in DRAM + dma_gather by src, one-hot
mask matmuls (fp32r) accumulating (numer | softmax-denominator) in PSUM.
Group graph replicated on every core. Identities used:
  exp(LeakyReLU(al+ar)) == max(exp(al)exp(ar), exp(.2al)exp(.2ar))
  segment softmax is shift-invariant (edge scores are O(10): no max needed)
  (A+I)[gidx] row gather folds the group-attention self term exactly.
"""
import sys

sys.path.insert(0, "/opt/trn_rl_repo")

import numpy as np

N, G = 50000, 1024
F_IN, HID, HEADS, NCLS = 128, 32, 4, 32
LN_EPS = 1e-5
NCORES = 8
NPER = N // NCORES            # 6250
NT = (NPER + 127) // 128      # 49 tiles/core
NPAD = NT * 128               # 6272 padded rows/core
NG = NCORES * NPAD            # 50176 padded global rows
SPLIT = 32768                 # int16 gather split (padded id space)
P = 128
SENT = 255.0                  # pad-edge dstlocal sentinel (mask never matches)
TAB1_COLS = 192               # [h(128) | u(4) | u2(4) | junk]  (768B rows)
TAB2_COLS = 64                # [h2(32) | u(1) | u2(1) | junk]  (256B rows)
VT_COLS = 64                  # [v(H) | v2(H) | junk]           (256B rows)
GCAP = 8                      # gather blocks (of 128 idxs) per dma_gather


# --------------------------------------------------------------------------
# host-side preprocessing
# --------------------------------------------------------------------------

def _wrap16(ix):
    """Compact dma_gather idx layout: [16, n/16]; idx i at [i%16, i//16].
    Replication to the 8 groups of 16 partitions happens on device."""
    ix = np.asarray(ix, np.int64)
    n = len(ix)
    assert n % 16 == 0, n
    return np.ascontiguousarray(ix.reshape(n // 16, 16).T.astype(np.int16))


def _segments(src, dst, ntile, split):
    """src already in padded-id space; dst in core-local [0, NPER)."""
    tile = dst // 128
    segs = []
    for t in range(ntile):
        m = tile == t
        s, d = src[m], dst[m] - t * 128
        if split:
            lo = s < SPLIT
            segs.append((s[lo], d[lo], s[~lo], d[~lo]))
        else:
            segs.append((s, d, s[:0], d[:0]))
    return segs


def _flatten(segs, nblk_lo, nblk_hi, ntile, dg_pad=0):
    nblk = nblk_lo + nblk_hi
    idx_lo, idx_hi, dmod, dglob = [], [], [], []
    for t in range(ntile):
        slo, dlo, shi, dhi = segs[t]
        a = np.zeros(nblk_lo * 128, np.int64); a[:len(slo)] = slo
        b = np.zeros(nblk_hi * 128, np.int64); b[:len(shi)] = shi - SPLIT
        dm = np.full(nblk * 128, SENT, np.float64)
        dm[:len(dlo)] = dlo
        dm[nblk_lo * 128:nblk_lo * 128 + len(dhi)] = dhi
        dg = np.full(nblk * 128, dg_pad, np.int64)
        dg[:len(dlo)] = dlo + t * 128
        dg[nblk_lo * 128:nblk_lo * 128 + len(dhi)] = dhi + t * 128
        idx_lo.append(a); idx_hi.append(b); dmod.append(dm); dglob.append(dg)
    idx_lo = np.concatenate(idx_lo) if nblk_lo else np.zeros(0, np.int64)
    idx_hi = np.concatenate(idx_hi) if nblk_hi else np.zeros(0, np.int64)
    dmod = np.concatenate(dmod)
    dglob = np.concatenate(dglob)
    # block layout [128, ntile*nblk]: column t*nblk+b holds block b's dstlocal
    dmod2 = np.ascontiguousarray(
        dmod.reshape(ntile * nblk, 128).T.astype(np.uint8))
    return idx_lo, idx_hi, dmod2, dglob


def _wext(W, a_src, a_dst, b, ncols):
    W = np.asarray(W, np.float32)
    a_src = np.asarray(a_src, np.float32)
    a_dst = np.asarray(a_dst, np.float32)
    b = np.asarray(b, np.float32)
    H, C = a_src.shape
    D = W.shape[1]
    asrc_m = np.zeros((D, H), np.float32)
    adst_m = np.zeros((D, H), np.float32)
    for h in range(H):
        asrc_m[h * C:(h + 1) * C, h] = a_src[h]
        adst_m[h * C:(h + 1) * C, h] = a_dst[h]
    Wx = np.concatenate([W, W @ asrc_m, W @ adst_m], axis=1)
    Wx = np.concatenate(
        [Wx, np.zeros((W.shape[0], ncols - Wx.shape[1]), np.float32)], axis=1)
    brow = np.concatenate([b, b @ asrc_m, b @ adst_m,
                           np.zeros(ncols - D - 2 * H, np.float32)])
    return np.ascontiguousarray(Wx), brow.astype(np.float32)


def host_prep(inputs):
    import ml_dtypes
    bf16 = ml_dtypes.bfloat16
    f32 = np.float32
    x1 = np.asarray(inputs["x1"], f32)
    ei1 = np.asarray(inputs["edge_index1"], np.int64)
    x2 = np.asarray(inputs["x2"], f32)
    ei2 = np.asarray(inputs["edge_index2"], np.int64)
    gidx = np.asarray(inputs["group_index"], np.int64)

    A = np.zeros((G, G), f32)
    u, v = ei2[0], ei2[1]
    np.add.at(A, (u, v), 1.0)
    np.add.at(A, (v, u), (u != v).astype(f32))
    Ap = A + np.eye(G, dtype=f32)
    assert Ap.max() < 256

    src_g, dst_g = ei1[0], ei1[1]
    # remap src node id into the padded-section id space (core*NPAD + local)
    pad_of = lambda ids: (ids // NPER) * NPAD + (ids % NPER)
    core_of = dst_g // NPER
    all_segs = []
    for c in range(NCORES):
        m = core_of == c
        loops = np.arange(c * NPER, (c + 1) * NPER, dtype=np.int64)
        s = pad_of(np.concatenate([src_g[m], loops]))
        d = np.concatenate([dst_g[m], loops]) - c * NPER
        all_segs.append(_segments(s, d, NT, True))
    nblk_lo = max(max((len(t[0]) + 127) // 128 for t in sg) for sg in all_segs)
    nblk_hi = max(max((len(t[2]) + 127) // 128 for t in sg) for sg in all_segs)

    loops2 = np.arange(G, dtype=np.int64)
    s2 = np.concatenate([ei2[0], loops2])
    d2 = np.concatenate([ei2[1], loops2])
    sm_segs = _segments(s2, d2, G // 128, False)
    nblk_sm = max((len(t[0]) + 127) // 128 for t in sm_segs)

    meta = dict(nblk_lo=nblk_lo, nblk_hi=nblk_hi, nblk=nblk_lo + nblk_hi,
                nblk_sm=nblk_sm)

    w1a, b1a = _wext(inputs["W1a"], inputs["a1a_src"], inputs["a1a_dst"],
                     inputs["b1a"], 256)
    w1b, b1b = _wext(inputs["W1b"], inputs["a1b_src"], inputs["a1b_dst"],
                     inputs["b1b"], 256)
    w2a, b2a = _wext(inputs["W2a"], inputs["a2a_src"], inputs["a2a_dst"],
                     inputs["b2a"], 64)
    w2b, b2b = _wext(inputs["W2b"], inputs["a2b_src"], inputs["a2b_dst"],
                     inputs["b2b"], 64)

    i_sm, _, dm_sm, dg_sm = _flatten(sm_segs, nblk_sm, 0, G // 128)

    # [b1a(0:256)|b1b(256:512)|b2a(512:576)|b2b(576:640)|
    #  ln1g(640:768)|ln1b(768:896)|ln2g(896:928)|ln2b(928:960)]
    rowcat = np.concatenate([
        b1a, b1b, b2a, b2b,
        np.asarray(inputs["ln1_g"], f32), np.asarray(inputs["ln1_b"], f32),
        np.asarray(inputs["ln2_g"], f32), np.asarray(inputs["ln2_b"], f32)])
    rowcat16 = np.ascontiguousarray(
        np.broadcast_to(rowcat[None, :], (16, rowcat.shape[0])))

    shared = dict(wext1a=np.asarray(w1a, bf16))
    # identical-on-every-core arrays are uploaded as 1/8-row shards and
    # AllGathered on device
    i_smw = _wrap16(i_sm)
    dg_smw = _wrap16(dg_sm)
    sh_slices = dict(wext1b=w1b, wext2a=w2a, wext2b=w2b, rowcat=rowcat16,
                     idx_smc=i_smw, dstl_sm=dm_sm, dglob_smc=dg_smw)

    per_core = []
    for c in range(NCORES):
        ilo, ihi, dmod, dglob = _flatten(all_segs[c], nblk_lo, nblk_hi, NT,
                                         dg_pad=NPAD)
        gown = np.concatenate([gidx[c * NPER:(c + 1) * NPER],
                               np.zeros(NPAD - NPER, np.int64)])
        x1sh = np.zeros((P, NPAD), f32)
        x1sh[:, :NPER] = x1[c * NPER:(c + 1) * NPER].T
        shsh = {k + "_sh": np.ascontiguousarray(
                    a[c * (a.shape[0] // 8):(c + 1) * (a.shape[0] // 8)])
                for k, a in sh_slices.items()}
        per_core.append(dict(
            **shsh,
            idx_loc=_wrap16(ilo), idx_hic=_wrap16(ihi),
            dglobc=_wrap16(dglob), gidxc=_wrap16(gown),
            x1Tsh=np.asarray(x1sh, bf16),
            x2Tsh=np.ascontiguousarray(x2[c * 128:(c + 1) * 128].T),
            Absh=np.asarray(Ap[c * 128:(c + 1) * 128], bf16),
        ))
    return shared, per_core, meta


# --------------------------------------------------------------------------
# device program
# --------------------------------------------------------------------------

def build_nc(meta):
    import contextlib
    from concourse import bacc, mybir
    from concourse.tile import TileContext
    from concourse.bass import ds, ts

    f32 = mybir.dt.float32
    f32r = mybir.dt.float32r
    bf16 = mybir.dt.bfloat16
    i16 = mybir.dt.int16
    i32 = mybir.dt.int32
    u8 = mybir.dt.uint8
    Alu = mybir.AluOpType
    Act = mybir.ActivationFunctionType
    Ax = mybir.AxisListType

    NBLK = meta["nblk"]
    NBLK_LO = meta["nblk_lo"]
    NBLK_HI = meta["nblk_hi"]
    NBLK_SM = meta["nblk_sm"]

    nc = bacc.Bacc(None, target_bir_lowering=False, debug=True)

    dp = lambda n, s, d: nc.declare_dram_parameter(n, list(s), d, isOutput=False)
    x1Tsh_d = dp("x1Tsh", [P, NPAD], bf16)
    x2Tsh_d = dp("x2Tsh", [P, P], f32r)
    Absh_d = dp("Absh", [P, G], bf16)
    wext1a_d = dp("wext1a", [P, 256], bf16)
    wext1b_sh_d = dp("wext1b_sh", [16, 256], f32r)
    wext2a_sh_d = dp("wext2a_sh", [16, 64], f32r)
    wext2b_sh_d = dp("wext2b_sh", [16, 64], f32r)
    rowcat_sh_d = dp("rowcat_sh", [2, 960], f32)
    idx_smc_sh_d = dp("idx_smc_sh", [2, 8 * NBLK_SM * 8], i16)
    dstl_sm_sh_d = dp("dstl_sm_sh", [16, 8 * NBLK_SM], u8)
    dglob_smc_sh_d = dp("dglob_smc_sh", [2, 8 * NBLK_SM * 8], i16)
    idx_loc_d = dp("idx_loc", [16, NT * NBLK_LO * 8], i16)
    idx_hic_d = dp("idx_hic", [16, NT * NBLK_HI * 8], i16)
    dglobc_d = dp("dglobc", [16, NT * NBLK * 8], i16)
    gidxc_d = dp("gidxc", [16, NT * 8], i16)

    out_d = nc.declare_dram_parameter("out", [NPAD, NCLS], bf16, isOutput=True)

    # AllGather-assembled full tensors (collectives cannot read IO tensors
    # directly, so shards are staged into internal DRAM first)
    x1st_d = nc.dram_tensor("x1st", [P, NPAD], bf16)
    x2st_d = nc.dram_tensor("x2st", [P, P], f32r)
    Abst_d = nc.dram_tensor("Abst", [P, G], bf16)
    shstage = {}
    shfull = {}
    for nm, dram, full_rows in [
            ("wext1b", wext1b_sh_d, P), ("wext2a", wext2a_sh_d, P),
            ("wext2b", wext2b_sh_d, P), ("rowcat", rowcat_sh_d, 16),
            ("idx_smc", idx_smc_sh_d, 16), ("dstl_sm", dstl_sm_sh_d, P),
            ("dglob_smc", dglob_smc_sh_d, 16)]:
        shp = list(dram.shape)
        shstage[nm] = nc.dram_tensor(nm + "_st", shp, dram.dtype)
        shfull[nm] = nc.dram_tensor(nm + "_G", [full_rows, shp[1]],
                                    dram.dtype, addr_space="Shared")
    x1TG_d = nc.dram_tensor("x1TG", [8 * P, NPAD], bf16, addr_space="Shared")
    x2TG_d = nc.dram_tensor("x2TG", [8 * P, P], f32r, addr_space="Shared")
    AbG_d = nc.dram_tensor("AbG", [G, G], bf16, addr_space="Shared")
    # full-layout (8x replicated) gather index tables, built on device
    idx_lo_d = nc.dram_tensor("idx_lo", [P, NT * NBLK_LO * 8], i16)
    idx_hi_d = nc.dram_tensor("idx_hi", [P, NT * NBLK_HI * 8], i16)
    dglob_d = nc.dram_tensor("dglob", [P, NT * NBLK * 8], i16)

    tab1_d = nc.dram_tensor("tab1", [NG, TAB1_COLS], f32)
    # one extra 128-row tile: row NPAD is the pad-slot target (zeroed)
    vtab1_d = nc.dram_tensor("vtab1", [NPAD + 128, VT_COLS], f32)
    smtab1_d = nc.dram_tensor("smtab1", [G, TAB1_COLS], f32)
    smvtab1_d = nc.dram_tensor("smvtab1", [G, VT_COLS], f32)
    tab2own_d = nc.dram_tensor("tab2own", [NPAD, TAB2_COLS], f32)
    tab2_d = nc.dram_tensor("tab2", [NG, TAB2_COLS], f32, addr_space="Shared")
    vtab2_d = nc.dram_tensor("vtab2", [NPAD + 128, VT_COLS], f32)
    smtab2_d = nc.dram_tensor("smtab2", [G, TAB2_COLS], f32)
    smvtab2_d = nc.dram_tensor("smvtab2", [G, VT_COLS], f32)

    with TileContext(nc) as tc, contextlib.ExitStack() as ctx:
        pool = ctx.enter_context(tc.tile_pool(name="main", bufs=2))
        cpool = ctx.enter_context(tc.tile_pool(name="consts", bufs=1))
        spool = ctx.enter_context(tc.tile_pool(name="stash", bufs=1))
        gpool = ctx.enter_context(tc.tile_pool(name="gather", bufs=1))
        qpool = ctx.enter_context(tc.tile_pool(name="q", bufs=1))
        ppool = ctx.enter_context(tc.tile_pool(name="psA", bufs=2, space="PSUM"))
        npool = ctx.enter_context(tc.tile_pool(name="psN", bufs=2, space="PSUM"))
        tpool = ctx.enter_context(tc.tile_pool(name="psT", bufs=2, space="PSUM"))
        spsum = ctx.enter_context(tc.tile_pool(name="psS", bufs=1, space="PSUM"))

        # ---- AllGathers: assemble full x1T / x2T / A on device ----
        nc.sync.dma_start(out=x1st_d[:], in_=x1Tsh_d[:])
        nc.sync.dma_start(out=Abst_d[:], in_=Absh_d[:])
        nc.sync.dma_start(out=x2st_d[:], in_=x2Tsh_d[:])
        nc.gpsimd.collective_compute(
            "AllGather", Alu.bypass, replica_groups=[list(range(NCORES))],
            ins=[x1st_d[:]], outs=[x1TG_d[:]])
        nc.gpsimd.collective_compute(
            "AllGather", Alu.bypass, replica_groups=[list(range(NCORES))],
            ins=[Abst_d[:]], outs=[AbG_d[:]])
        nc.gpsimd.collective_compute(
            "AllGather", Alu.bypass, replica_groups=[list(range(NCORES))],
            ins=[x2st_d[:]], outs=[x2TG_d[:]])
        for nm, dram in [("wext1b", wext1b_sh_d), ("wext2a", wext2a_sh_d),
                         ("wext2b", wext2b_sh_d), ("rowcat", rowcat_sh_d),
                         ("idx_smc", idx_smc_sh_d), ("dstl_sm", dstl_sm_sh_d),
                         ("dglob_smc", dglob_smc_sh_d)]:
            nc.sync.dma_start(out=shstage[nm][:], in_=dram[:])
            nc.gpsimd.collective_compute(
                "AllGather", Alu.bypass, replica_groups=[list(range(NCORES))],
                ins=[shstage[nm][:]], outs=[shfull[nm][:]])

        # ---- replicate compact idx tables to full 128-partition layout ----
        for g in range(8):
            nc.sync.dma_start(out=idx_lo_d[16 * g:16 * (g + 1), :],
                              in_=idx_loc_d[:])
            nc.sync.dma_start(out=idx_hi_d[16 * g:16 * (g + 1), :],
                              in_=idx_hic_d[:])
            nc.sync.dma_start(out=dglob_d[16 * g:16 * (g + 1), :],
                              in_=dglobc_d[:])

        def load_const(dram, shape, dtype, tag):
            t = cpool.tile(shape, dtype, tag=tag)
            nc.sync.dma_start(out=t[:], in_=dram[:])
            return t

        def load_rep16(dram, cols, dtype, tag):
            """[16, cols] DRAM -> [128, cols] SBUF, replicated 8x."""
            t = cpool.tile([P, cols], dtype, tag=tag)
            for g in range(8):
                nc.sync.dma_start(out=t[16 * g:16 * (g + 1), :], in_=dram[:])
            return t

        # iota row / per-partition index / identity, generated on device
        iotaI = cpool.tile([P, P], i32, tag="iotaI")
        nc.gpsimd.iota(iotaI[:], pattern=[[1, P]], base=0, channel_multiplier=0)
        iota_s = cpool.tile([P, P], f32, tag="iota")
        nc.vector.tensor_copy(out=iota_s[:], in_=iotaI[:])
        iotaPI = cpool.tile([P, 1], i32, tag="iotaPI")
        nc.gpsimd.iota(iotaPI[:], pattern=[[0, 1]], base=0, channel_multiplier=1)
        iotaP_s = cpool.tile([P, 1], f32, tag="iotaP")
        nc.vector.tensor_copy(out=iotaP_s[:], in_=iotaPI[:])
        ident_s = cpool.tile([P, P], f32, tag="ident")
        nc.vector.tensor_scalar(out=ident_s[:], in0=iota_s[:],
                                scalar1=iotaP_s[:, 0:1], scalar2=None,
                                op0=Alu.is_equal)

        wext1a_s = load_const(wext1a_d, [P, 256], bf16, "wext1a")
        wext1b_s = load_const(shfull["wext1b"], [P, 256], f32r, "wext1b")
        wext2a_s = load_const(shfull["wext2a"], [P, 64], f32r, "wext2a")
        wext2b_s = load_const(shfull["wext2b"], [P, 64], f32r, "wext2b")
        rc_s = load_rep16(shfull["rowcat"], 960, f32, "rowcat")
        brep1a_s = rc_s[:, 0:256]
        brep1b_s = rc_s[:, 256:512]
        brep2a_s = rc_s[:, 512:576]
        brep2b_s = rc_s[:, 576:640]
        g1rep_s = rc_s[:, 640:768]
        b1rep_s = rc_s[:, 768:896]
        g2rep_s = rc_s[:, 896:928]
        b2rep_s = rc_s[:, 928:960]

        idxsm_s = load_rep16(shfull["idx_smc"], 8 * NBLK_SM * 8, i16, "idxsm")
        dglobsm_s = load_rep16(shfull["dglob_smc"], 8 * NBLK_SM * 8, i16,
                               "dglobsm")
        gidx_s = load_rep16(gidxc_d, NT * 8, i16, "gidx")

        def load_u8_as_f32(dram, cols, tag):
            tb = pool.tile([P, cols], u8, tag=f"{tag}_u8")
            nc.sync.dma_start(out=tb[:], in_=dram[:])
            t = cpool.tile([P, cols], f32, tag=tag)
            nc.vector.tensor_copy(out=t[:], in_=tb[:])
            return t

        dstlsm_s = load_u8_as_f32(shfull["dstl_sm"], 8 * NBLK_SM, "dstlsm")
        # derive big-graph dstl from the wrapped dglob idx table:
        # block-layout [p, c] = wrap16-layout [p, 8c + p//16]
        dgw = cpool.tile([P, NT * NBLK * 8], i16, tag="dgw")
        nc.sync.dma_start(out=dgw[:], in_=dglob_d[:])
        dgv = dgw[:].rearrange("p (c e) -> p c e", e=8)
        dsti = cpool.tile([P, NT * NBLK], i16, tag="dsti")
        for g in range(8):
            nc.sync.dma_start(out=dsti[16 * g:16 * (g + 1), :],
                              in_=dgv[16 * g:16 * (g + 1), :, g])
        dstl_s = cpool.tile([P, NT * NBLK], f32, tag="dstl")
        nc.vector.tensor_copy(out=dstl_s[:], in_=dsti[:])
        # per-tile base offsets (t*128) for the in-loop subtract
        tbI = cpool.tile([P, NT], i32, tag="tbI")
        nc.gpsimd.iota(tbI[:], pattern=[[128, NT]], base=0,
                       channel_multiplier=0)
        tbase_s = cpool.tile([P, NT], f32, tag="tbase")
        nc.vector.tensor_copy(out=tbase_s[:], in_=tbI[:])
        # zero the vtab pad-slot tile (row NPAD target of dglob pads)
        zv = cpool.tile([P, VT_COLS], f32, tag="zv")
        nc.vector.memset(zv[:], 0.0)
        nc.sync.dma_start(out=vtab1_d[NPAD:NPAD + 128, :], in_=zv[:])
        nc.sync.dma_start(out=vtab2_d[NPAD:NPAD + 128, :], in_=zv[:])

        # pre-zero the q-slots so junk pad columns of the fp32r rhs are finite
        zq = qpool.tile([P, NBLK, 256], f32r, tag="q256")
        nc.vector.memset(zq[:].rearrange("p a b -> p (a b)").bitcast(f32), 0.0)
        zq = qpool.tile([P, max(NBLK, NBLK_SM), 40], f32r, tag="q33")
        nc.vector.memset(zq[:].rearrange("p a b -> p (a b)").bitcast(f32), 0.0)

        # ---------------- phase 1: tables ----------------
        # global tab1 (8 sections x 49 tiles) from AllGathered x1TG, bf16
        with tc.For_i(0, NT, 1) as t:
            for sec in range(NCORES):
                lhsb = pool.tile([P, 128], bf16, tag="tb_lhsb")
                nc.sync.dma_start(out=lhsb[:],
                                  in_=x1TG_d[sec * P:(sec + 1) * P,
                                             ts(t, 128)])
                ps = ppool.tile([P, 256], f32, tag="agg", space="PSUM")
                nc.tensor.matmul(out=ps[:], lhsT=lhsb[:], rhs=wext1a_s[:],
                                 start=True, stop=True)
                st = pool.tile([P, 256], f32, tag="tb_st")
                nc.vector.scalar_tensor_tensor(
                    out=st[:], in0=ps[:], scalar=1.0,
                    in1=brep1a_s[:], op0=Alu.bypass, op1=Alu.add)
                nc.scalar.activation(st[:, 132:136], st[:, 128:132],
                                     Act.Exp, scale=0.2)
                nc.scalar.activation(st[:, 128:132], st[:, 128:132], Act.Exp)
                nc.sync.dma_start(
                    out=tab1_d[ds(t * 128 + sec * NPAD, 128), :],
                    in_=st[:, 0:TAB1_COLS])

        # small-graph tables from AllGathered x2TG (sections == tiles)
        for t in range(8):
            lhs = pool.tile([P, 128], f32r, tag="sm_lhs")
            nc.sync.dma_start(out=lhs[:], in_=x2TG_d[t * P:(t + 1) * P, :])
            ps = ppool.tile([P, 256], f32, tag="agg", space="PSUM")
            nc.tensor.matmul(out=ps[:], lhsT=lhs[:], rhs=wext1b_s[:],
                             start=True, stop=True)
            st = pool.tile([P, 256], f32, tag="tb_st")
            nc.vector.scalar_tensor_tensor(
                out=st[:], in0=ps[:], scalar=1.0,
                in1=brep1b_s[:], op0=Alu.bypass, op1=Alu.add)
            vst = pool.tile([P, 8], f32, tag="vt_vst")
            nc.scalar.activation(vst[:, 0:4], st[:, 132:136], Act.Exp)
            nc.scalar.activation(vst[:, 4:8], st[:, 132:136], Act.Exp,
                                 scale=0.2)
            nc.sync.dma_start(out=smvtab1_d[t * 128:(t + 1) * 128, 0:8],
                              in_=vst[:])
            nc.scalar.activation(st[:, 132:136], st[:, 128:132],
                                 Act.Exp, scale=0.2)
            nc.scalar.activation(st[:, 128:132], st[:, 128:132], Act.Exp)
            nc.sync.dma_start(out=smtab1_d[t * 128:(t + 1) * 128, :],
                              in_=st[:, 0:TAB1_COLS])

        # ---------------- edge aggregation (loop body helper) ----------------
        def edge_gat_body(t, tab_dram, vtab_dram, idxlo_src, idxhi_src,
                          dstl_src, dglob_src, nblk, nblk_lo, F, H, rhs_n,
                          idx_in_sbuf, tag, tbase=None):
            """Emits ops for dst-tile t (loop var); returns agg psum
            [(numer F) | (s H)]."""
            tabcols = TAB1_COLS if F == 128 else TAB2_COLS
            gtag = f"g{tabcols}"
            qtag = "q256" if F == 128 else "q33"
            nblk_hi = nblk - nblk_lo
            gt = gpool.tile([P, nblk, tabcols], f32, tag=gtag)
            for g0 in range(0, nblk_lo, GCAP):
                g1 = min(g0 + GCAP, nblk_lo)
                if idx_in_sbuf:
                    iap = idxlo_src[:, ds(t * nblk_lo * 8 + g0 * 8,
                                          (g1 - g0) * 8)]
                else:
                    it = pool.tile([P, (g1 - g0) * 8], i16, tag=f"{tag}_il{g0}")
                    nc.sync.dma_start(
                        out=it[:], in_=idxlo_src[:, ds(t * nblk_lo * 8 + g0 * 8,
                                                       (g1 - g0) * 8)])
                    iap = it[:]
                nc.gpsimd.dma_gather(
                    out_ap=gt[:, g0:g1, :], in_ap=tab_dram[:],
                    idxs_ap=iap, num_idxs=(g1 - g0) * 128,
                    num_idxs_reg=(g1 - g0) * 128, elem_size=tabcols)
            for g0 in range(0, nblk_hi, GCAP):
                g1 = min(g0 + GCAP, nblk_hi)
                if idx_in_sbuf:
                    iap = idxhi_src[:, ds(t * nblk_hi * 8 + g0 * 8,
                                          (g1 - g0) * 8)]
                else:
                    it = pool.tile([P, (g1 - g0) * 8], i16, tag=f"{tag}_ih{g0}")
                    nc.sync.dma_start(
                        out=it[:], in_=idxhi_src[:, ds(t * nblk_hi * 8 + g0 * 8,
                                                       (g1 - g0) * 8)])
                    iap = it[:]
                nc.gpsimd.dma_gather(
                    out_ap=gt[:, nblk_lo + g0:nblk_lo + g1, :],
                    in_ap=tab_dram[SPLIT:, :],
                    idxs_ap=iap, num_idxs=(g1 - g0) * 128,
                    num_idxs_reg=(g1 - g0) * 128, elem_size=tabcols)
            vt = gpool.tile([P, nblk, VT_COLS], f32, tag="v64")
            for g0 in range(0, nblk, GCAP):
                g1 = min(g0 + GCAP, nblk)
                if idx_in_sbuf:
                    iap = dglob_src[:, ds(t * nblk * 8 + g0 * 8, (g1 - g0) * 8)]
                else:
                    it = pool.tile([P, (g1 - g0) * 8], i16, tag=f"{tag}_dg{g0}")
                    nc.sync.dma_start(
                        out=it[:], in_=dglob_src[:, ds(t * nblk * 8 + g0 * 8,
                                                       (g1 - g0) * 8)])
                    iap = it[:]
                nc.gpsimd.dma_gather(
                    out_ap=vt[:, g0:g1, :], in_ap=vtab_dram[:],
                    idxs_ap=iap, num_idxs=(g1 - g0) * 128,
                    num_idxs_reg=(g1 - g0) * 128, elem_size=VT_COLS)
            dl = pool.tile([P, nblk], f32, tag=f"{tag}_dl")
            if tbase is None:
                nc.vector.tensor_copy(out=dl[:], in_=dstl_src[:, ts(t, nblk)])
            else:
                tb = pool.tile([P, 1], f32, tag=f"{tag}_tb")
                nc.vector.tensor_copy(out=tb[:], in_=tbase[:, ds(t, 1)])
                nc.vector.tensor_scalar(
                    out=dl[:], in0=dstl_src[:, ts(t, nblk)], scalar1=tb[:, 0:1],
                    scalar2=None, op0=Alu.subtract)
            mask = qpool.tile([P, nblk, 128], f32r, tag="mask")
            nc.vector.tensor_tensor(
                out=mask[:],
                in0=iota_s[:][:, None, :].to_broadcast([P, nblk, 128]),
                in1=dl[:][:, :, None].to_broadcast([P, nblk, 128]),
                op=Alu.is_equal)
            q = qpool.tile([P, nblk, rhs_n], f32r, tag=qtag)
            m1 = pool.tile([P, nblk, H], f32, tag="pm1")
            m2 = pool.tile([P, nblk, H], f32, tag="pm2")
            nc.vector.tensor_tensor(out=m1[:], in0=gt[:, :, F:F + H],
                                    in1=vt[:, :, 0:H], op=Alu.mult)
            nc.vector.tensor_tensor(out=m2[:], in0=gt[:, :, F + H:F + 2 * H],
                                    in1=vt[:, :, H:2 * H], op=Alu.mult)
            nc.vector.tensor_tensor(out=q[:, :, F:F + H], in0=m1[:],
                                    in1=m2[:], op=Alu.max)
            C = F // H
            for h in range(H):
                nc.vector.tensor_tensor(
                    out=q[:, :, h * C:(h + 1) * C],
                    in0=gt[:, :, h * C:(h + 1) * C],
                    in1=q[:, :, F + h:F + h + 1].to_broadcast([P, nblk, C]),
                    op=Alu.mult)
            ps = ppool.tile([P, 256], f32, tag="agg", space="PSUM")
            for b in range(nblk):
                nc.tensor.matmul(
                    out=ps[:, 0:rhs_n], lhsT=mask[:, b, :], rhs=q[:, b, :],
                    start=(b == 0), stop=(b == nblk - 1))
            return ps

        def xout_from_ps(ps, F, H, brep_s, tag):
            rec = pool.tile([P, H], f32, tag=f"{tag}_rec")
            nc.vector.reciprocal(out=rec[:], in_=ps[:, F:F + H])
            xo = pool.tile([P, F], f32, tag=f"{tag}_xo")
            C = F // H
            for h in range(H):
                nc.vector.tensor_scalar(
                    out=xo[:, h * C:(h + 1) * C], in0=ps[:, h * C:(h + 1) * C],
                    scalar1=rec[:, h:h + 1], scalar2=None, op0=Alu.mult)
            nc.vector.tensor_tensor(out=xo[:], in0=xo[:], in1=brep_s[:, 0:F],
                                    op=Alu.add)
            return xo

        # ---------------- group attention (loop body helper) ----------------
        def group_attn(t, xo, X2pT_ap, X2ext_all, Fs, rhs_n, tag):
            """Returns 0.5*grp tile [P, Fs] f32."""
            pt = tpool.tile([P, 128], f32, tag="ptr", space="PSUM")
            nc.tensor.transpose(out=pt[:Fs, :], in_=xo[:, 0:Fs],
                                identity=ident_s[:])
            xT = pool.tile([P, 128], f32r, tag="ga_xT")
            nc.scalar.copy(out=xT[:Fs, :], in_=pt[:Fs, :])
            pss = spsum.tile([P, 1024], f32, tag="s", space="PSUM")
            nc.tensor.matmul(out=pss[:, 0:512], lhsT=xT[:Fs, :],
                             rhs=X2pT_ap[:, 0:512], start=True, stop=True)
            nc.tensor.matmul(out=pss[:, 512:1024], lhsT=xT[:Fs, :],
                             rhs=X2pT_ap[:, 512:1024], start=True, stop=True)
            mx0 = pool.tile([P, 1], f32, tag="ga_mx0")
            mx1 = pool.tile([P, 1], f32, tag="ga_mx1")
            nc.vector.reduce_max(mx0[:], pss[:, 0:512], axis=Ax.X)
            nc.vector.reduce_max(mx1[:], pss[:, 512:1024], axis=Ax.X)
            negmx = pool.tile([P, 1], f32, tag="ga_negmx")
            nc.vector.tensor_tensor(out=negmx[:], in0=mx0[:], in1=mx1[:],
                                    op=Alu.max)
            nc.vector.tensor_scalar(out=negmx[:], in0=negmx[:], scalar1=-1.0,
                                    scalar2=None, op0=Alu.mult)
            wx = pool.tile([P, G], f32, tag="ga_wx")
            nc.scalar.activation(wx[:, 0:512], pss[:, 0:512], Act.Exp,
                                 bias=negmx[:])
            nc.scalar.activation(wx[:, 512:1024], pss[:, 512:1024], Act.Exp,
                                 bias=negmx[:])
            at = pool.tile([P, 8, 128], bf16, tag="ga_at")
            nc.gpsimd.dma_gather(
                out_ap=at[:], in_ap=AbG_d[:], idxs_ap=gidx_s[:, ts(t, 8)],
                num_idxs=128, num_idxs_reg=128, elem_size=G, transpose=True)
            psn = npool.tile([P, 256], f32, tag="num", space="PSUM")
            for j in range(8):
                wt = tpool.tile([P, 128], f32, tag="ptr", space="PSUM")
                nc.tensor.transpose(out=wt[:], in_=wx[:, j * 128:(j + 1) * 128],
                                    identity=ident_s[:])
                bmt = pool.tile([P, 128], f32r, tag="ga_bmt")
                nc.vector.scalar_tensor_tensor(
                    out=bmt[:], in0=wt[:], scalar=1.0, in1=at[:, j, :],
                    op0=Alu.bypass, op1=Alu.mult)
                nc.tensor.matmul(out=psn[:, 0:rhs_n], lhsT=bmt[:],
                                 rhs=X2ext_all[:, j, :], start=(j == 0),
                                 stop=(j == 7))
            rec = pool.tile([P, 1], f32, tag="ga_grec")
            nc.vector.reciprocal(out=rec[:], in_=psn[:, Fs:Fs + 1])
            grp = pool.tile([P, Fs], f32, tag="ga_grp")
            nc.vector.tensor_scalar(out=grp[:], in0=psn[:, 0:Fs],
                                    scalar1=rec[:], scalar2=0.5, op0=Alu.mult,
                                    op1=Alu.mult)
            return grp

        # ================= small-graph GAT layer 1 =================
        xg1_all = spool.tile([P, 8, 128], f32, tag="xg1")
        X2pT = cpool.tile([P, G], f32r, tag="X2pT")
        X2ext_all = spool.tile([P, 8, 256], f32r, tag="X2ext")
        nc.vector.memset(
            X2ext_all[:].rearrange("p a b -> p (a b)").bitcast(f32), 0.0)
        nc.vector.memset(X2ext_all[:, :, 128:129].bitcast(f32), 1.0)
        with tc.For_i(0, 8, 1) as t:
            ps = edge_gat_body(
                t, smtab1_d, smvtab1_d, idxsm_s, None,
                dstlsm_s, dglobsm_s,
                NBLK_SM, NBLK_SM, 128, 4, 256, True, "sg1")
            xo = xout_from_ps(ps, 128, 4, brep1b_s, "sm1")
            nc.vector.tensor_copy(out=xg1_all[:, t, :], in_=xo[:])
            pt = tpool.tile([P, 128], f32, tag="ptr", space="PSUM")
            nc.tensor.transpose(out=pt[:], in_=xo[:], identity=ident_s[:])
            nc.scalar.copy(out=X2pT[:, ts(t, 128)], in_=pt[:])
            nc.scalar.copy(out=X2ext_all[:, t, 0:128], in_=xo[:])

        # ================= big-graph layer 1 =================
        var49 = cpool.tile([P, NT], f32, tag="var49")
        s1_all = spool.tile([P, NT, 128], f32, tag="s1")
        with tc.For_i(0, NT, 1) as t:
            # build this tile's v-table rows (dsts of tile t are within tile t)
            lhsb = pool.tile([P, 128], bf16, tag="vt_lhsb")
            nc.sync.dma_start(out=lhsb[:], in_=x1Tsh_d[:, ts(t, 128)])
            psv = npool.tile([P, 256], f32, tag="num", space="PSUM")
            nc.tensor.matmul(out=psv[:, 0:8], lhsT=lhsb[:],
                             rhs=wext1a_s[:, 128:136], start=True, stop=True)
            stv = pool.tile([P, 8], f32, tag="vt_st")
            nc.vector.scalar_tensor_tensor(
                out=stv[:], in0=psv[:, 0:8], scalar=1.0,
                in1=rc_s[:, 128:136], op0=Alu.bypass, op1=Alu.add)
            vst = pool.tile([P, 8], f32, tag="vt_vst")
            nc.scalar.activation(vst[:, 0:4], stv[:, 4:8], Act.Exp)
            nc.scalar.activation(vst[:, 4:8], stv[:, 4:8], Act.Exp, scale=0.2)
            nc.sync.dma_start(out=vtab1_d[ts(t, 128), 0:8], in_=vst[:])
            ps = edge_gat_body(
                t, tab1_d, vtab1_d, idx_lo_d, idx_hi_d,
                dstl_s, dglob_d,
                NBLK, NBLK_LO, 128, 4, 256, False, "bg1", tbase=tbase_s)
            xo = xout_from_ps(ps, 128, 4, brep1a_s, "b1")
            grp = group_attn(t, xo, X2pT[:], X2ext_all, 128, 256, "g1")
            s1 = pool.tile([P, 128], f32, tag="b1_s1")
            nc.vector.scalar_tensor_tensor(out=s1[:], in0=xo[:], scalar=0.5,
                                           in1=grp[:], op0=Alu.mult, op1=Alu.add)
            mu = pool.tile([P, 1], f32, tag="b1_mu")
            nc.vector.tensor_reduce(out=mu[:], in_=s1[:], axis=Ax.X, op=Alu.add)
            nc.vector.tensor_scalar(out=mu[:], in0=mu[:], scalar1=-1.0 / 128,
                                    scalar2=None, op0=Alu.mult)
            nc.vector.tensor_scalar(out=s1[:], in0=s1[:], scalar1=mu[:],
                                    scalar2=None, op0=Alu.add)
            nc.vector.tensor_copy(out=s1_all[:, t, :], in_=s1[:])
            sq = pool.tile([P, 128], f32, tag="b1_sq")
            nc.vector.tensor_tensor(out=sq[:], in0=s1[:], in1=s1[:], op=Alu.mult)
            nc.vector.tensor_reduce(out=var49[:, ds(t, 1)], in_=sq[:], axis=Ax.X,
                                    op=Alu.add)

        sd49 = cpool.tile([P, NT], f32, tag="sd49")
        nc.vector.tensor_scalar(out=sd49[:], in0=var49[:], scalar1=1.0 / 128,
                                scalar2=LN_EPS, op0=Alu.mult, op1=Alu.add)
        sq49 = cpool.tile([P, NT], f32, tag="sq49")
        nc.scalar.activation(sq49[:], sd49[:], Act.Sqrt)
        rstd49 = cpool.tile([P, NT], f32, tag="rstd49")
        nc.vector.reciprocal(out=rstd49[:], in_=sq49[:])

        with tc.For_i(0, NT, 1) as t:
            s1 = pool.tile([P, 128], f32, tag="l1_s1")
            nc.vector.tensor_copy(out=s1[:], in_=s1_all[:, t, :])
            rs = pool.tile([P, 1], f32, tag="l1_rs")
            nc.vector.tensor_copy(out=rs[:], in_=rstd49[:, ds(t, 1)])
            y = pool.tile([P, 128], f32, tag="b1_y")
            nc.vector.scalar_tensor_tensor(
                out=y[:], in0=s1[:], scalar=rs[:], in1=g1rep_s[:],
                op0=Alu.mult, op1=Alu.mult)
            nc.vector.tensor_tensor(out=y[:], in0=y[:], in1=b1rep_s[:],
                                    op=Alu.add)
            emin = pool.tile([P, 128], f32, tag="b1_emin")
            nc.vector.tensor_scalar(out=emin[:], in0=y[:], scalar1=0.0,
                                    scalar2=None, op0=Alu.min)
            nc.scalar.activation(emin[:], emin[:], Act.Exp)
            h1 = pool.tile([P, 128], f32, tag="b1_h1")
            nc.vector.tensor_scalar(out=h1[:], in0=y[:], scalar1=0.0,
                                    scalar2=-1.0, op0=Alu.max, op1=Alu.add)
            nc.vector.tensor_tensor(out=h1[:], in0=h1[:], in1=emin[:], op=Alu.add)
            pt = tpool.tile([P, 128], f32, tag="ptr", space="PSUM")
            nc.tensor.transpose(out=pt[:], in_=h1[:], identity=ident_s[:])
            h1T = pool.tile([P, 128], f32r, tag="b1_h1T")
            nc.scalar.copy(out=h1T[:], in_=pt[:])
            ps2 = npool.tile([P, 256], f32, tag="num", space="PSUM")
            nc.tensor.matmul(out=ps2[:, 0:64], lhsT=h1T[:], rhs=wext2a_s[:],
                             start=True, stop=True)
            st2 = pool.tile([P, 64], f32, tag="b1_st2")
            nc.vector.scalar_tensor_tensor(
                out=st2[:], in0=ps2[:, 0:64], scalar=1.0, in1=brep2a_s[:],
                op0=Alu.bypass, op1=Alu.add)
            vst = pool.tile([P, 2], f32, tag="b1_vst")
            nc.scalar.activation(vst[:, 0:1], st2[:, 33:34], Act.Exp)
            nc.scalar.activation(vst[:, 1:2], st2[:, 33:34], Act.Exp, scale=0.2)
            nc.scalar.activation(st2[:, 33:34], st2[:, 32:33], Act.Exp, scale=0.2)
            nc.scalar.activation(st2[:, 32:33], st2[:, 32:33], Act.Exp)
            nc.sync.dma_start(out=tab2own_d[ts(t, 128), :], in_=st2[:])
            nc.sync.dma_start(out=vtab2_d[ts(t, 128), 0:2], in_=vst[:, 0:2])

        nc.gpsimd.collective_compute(
            "AllGather", Alu.bypass, replica_groups=[list(range(NCORES))],
            ins=[tab2own_d[:]], outs=[tab2_d[:]])

        # ================= small-graph layer 2 =================
        with tc.For_i(0, 8, 1) as t:
            xg = pool.tile([P, 128], f32, tag="ts2_xg")
            nc.vector.tensor_copy(out=xg[:], in_=xg1_all[:, t, :])
            pt = tpool.tile([P, 128], f32, tag="ptr", space="PSUM")
            nc.tensor.transpose(out=pt[:], in_=xg[:], identity=ident_s[:])
            xT = pool.tile([P, 128], f32r, tag="ts2_xT")
            nc.scalar.copy(out=xT[:], in_=pt[:])
            ps2 = npool.tile([P, 256], f32, tag="num", space="PSUM")
            nc.tensor.matmul(out=ps2[:, 0:64], lhsT=xT[:], rhs=wext2b_s[:],
                             start=True, stop=True)
            st2 = pool.tile([P, 64], f32, tag="ts2_st")
            nc.vector.scalar_tensor_tensor(
                out=st2[:], in0=ps2[:, 0:64], scalar=1.0, in1=brep2b_s[:],
                op0=Alu.bypass, op1=Alu.add)
            vst = pool.tile([P, 2], f32, tag="ts2_vst")
            nc.scalar.activation(vst[:, 0:1], st2[:, 33:34], Act.Exp)
            nc.scalar.activation(vst[:, 1:2], st2[:, 33:34], Act.Exp, scale=0.2)
            nc.scalar.activation(st2[:, 33:34], st2[:, 32:33], Act.Exp, scale=0.2)
            nc.scalar.activation(st2[:, 32:33], st2[:, 32:33], Act.Exp)
            nc.sync.dma_start(out=smtab2_d[ts(t, 128), :], in_=st2[:])
            nc.sync.dma_start(out=smvtab2_d[ts(t, 128), 0:2], in_=vst[:, 0:2])

        xg2_all = spool.tile([P, 8, 32], f32, tag="xg2")
        X2p2T = cpool.tile([32, G], f32r, tag="X2p2T")
        X2ext2_all = spool.tile([P, 8, 40], f32r, tag="X2ext2")
        nc.vector.memset(
            X2ext2_all[:].rearrange("p a b -> p (a b)").bitcast(f32), 0.0)
        nc.vector.memset(X2ext2_all[:, :, 32:33].bitcast(f32), 1.0)
        with tc.For_i(0, 8, 1) as t:
            ps = edge_gat_body(
                t, smtab2_d, smvtab2_d, idxsm_s, None,
                dstlsm_s, dglobsm_s,
                NBLK_SM, NBLK_SM, 32, 1, 40, True, "sg2")
            xo = xout_from_ps(ps, 32, 1, brep2b_s, "sm2")
            nc.vector.tensor_copy(out=xg2_all[:, t, :], in_=xo[:])
            pt = tpool.tile([P, 128], f32, tag="ptr", space="PSUM")
            nc.tensor.transpose(out=pt[:32, :], in_=xo[:], identity=ident_s[:])
            nc.scalar.copy(out=X2p2T[:, ts(t, 128)], in_=pt[:32, :])
            nc.scalar.copy(out=X2ext2_all[:, t, 0:32], in_=xo[:])

        # ================= big-graph layer 2 =================
        var49b = cpool.tile([P, NT], f32, tag="var49b")
        o_all = spool.tile([P, NT, 32], f32, tag="o")
        with tc.For_i(0, NT, 1) as t:
            ps = edge_gat_body(
                t, tab2_d, vtab2_d, idx_lo_d, idx_hi_d,
                dstl_s, dglob_d,
                NBLK, NBLK_LO, 32, 1, 40, False, "bg2", tbase=tbase_s)
            xo = xout_from_ps(ps, 32, 1, brep2a_s, "b2")
            grp = group_attn(t, xo, X2p2T[:], X2ext2_all, 32, 40, "g2")
            o = pool.tile([P, 32], f32, tag="b2_o")
            nc.vector.scalar_tensor_tensor(out=o[:], in0=xo[:], scalar=0.5,
                                           in1=grp[:], op0=Alu.mult, op1=Alu.add)
            mu = pool.tile([P, 1], f32, tag="b2_mu")
            nc.vector.tensor_reduce(out=mu[:], in_=o[:], axis=Ax.X, op=Alu.add)
            nc.vector.tensor_scalar(out=mu[:], in0=mu[:], scalar1=-1.0 / 32,
                                    scalar2=None, op0=Alu.mult)
            nc.vector.tensor_scalar(out=o[:], in0=o[:], scalar1=mu[:],
                                    scalar2=None, op0=Alu.add)
            nc.vector.tensor_copy(out=o_all[:, t, :], in_=o[:])
            sq = pool.tile([P, 32], f32, tag="b2_sq")
            nc.vector.tensor_tensor(out=sq[:], in0=o[:], in1=o[:], op=Alu.mult)
            nc.vector.tensor_reduce(out=var49b[:, ds(t, 1)], in_=sq[:],
                                    axis=Ax.X, op=Alu.add)

        sd49b = cpool.tile([P, NT], f32, tag="sd49b")
        nc.vector.tensor_scalar(out=sd49b[:], in0=var49b[:], scalar1=1.0 / 32,
                                scalar2=LN_EPS, op0=Alu.mult, op1=Alu.add)
        sq49b = cpool.tile([P, NT], f32, tag="sq49b")
        nc.scalar.activation(sq49b[:], sd49b[:], Act.Sqrt)
        rstd49b = cpool.tile([P, NT], f32, tag="rstd49b")
        nc.vector.reciprocal(out=rstd49b[:], in_=sq49b[:])

        with tc.For_i(0, NT, 1) as t:
            o = pool.tile([P, 32], f32, tag="l2_o")
            nc.vector.tensor_copy(out=o[:], in_=o_all[:, t, :])
            rs = pool.tile([P, 1], f32, tag="l2_rs")
            nc.vector.tensor_copy(out=rs[:], in_=rstd49b[:, ds(t, 1)])
            y = pool.tile([P, 32], f32, tag="b2_y")
            nc.vector.scalar_tensor_tensor(
                out=y[:], in0=o[:], scalar=rs[:], in1=g2rep_s[:],
                op0=Alu.mult, op1=Alu.mult)
            nc.vector.tensor_tensor(out=y[:], in0=y[:], in1=b2rep_s[:],
                                    op=Alu.add)
            yb = pool.tile([P, 32], bf16, tag="b2_yb")
            nc.vector.tensor_copy(out=yb[:], in_=y[:])
            nc.sync.dma_start(out=out_d[ts(t, 128), :], in_=yb[:])

    nc.compile()
    return nc


# --------------------------------------------------------------------------
# entry point
# --------------------------------------------------------------------------

def kernel(**inputs):
    from concourse.bass_utils import run_bass_kernel_spmd

    shared, per_core, meta = host_prep(inputs)
    nc = build_nc(meta)
    in_maps = []
    for c in range(NCORES):
        m = dict(shared)
        m.update(per_core[c])
        in_maps.append(m)
    res = run_bass_kernel_spmd(nc, in_maps, list(range(NCORES)))
    out = np.concatenate([np.asarray(res.results[c]["out"])[:NPER]
                          for c in range(NCORES)])
    return out.astype(np.float32)


# revision 19
# speedup vs baseline: 3.1945x; 1.0443x over previous
"""Dual-GAT (nn_GAT_48017734369678) on 8 TRN2 NeuronCores via Bass/Tile.

Self-contained: host-side sharding/preprocessing in numpy, device program in
Bass (Tile), executed through run_bass_kernel_spmd on cores 0-7.

The dispatch cost here is dominated by (a) host->device upload bytes over the
axon tunnel (~50MB/s) and (b) STATIC instruction count in the NEFF (~45us per
instruction per dispatch). Both are minimized:
  (a) each core uploads only its own transposed x1 shard (bf16) / Ab rows /
      x2 rows; full copies are assembled on-device with AllGather. Gather
      index tables are uploaded compact ([16, n/16]) and replicated on device.
  (b) every per-tile stage is wrapped in a tc.For_i hardware loop with
      dynamic (register-offset) access patterns, so the program is ~600
      instructions instead of ~24000.

Per-core row spaces are padded to NPAD=6272=49*128 so all loops are uniform;
src node ids are remapped on host into the padded id space, and the padded
output rows are sliced off on host.

Edge aggregation: per-node gather tables in DRAM + dma_gather by src, one-hot
mask matmuls (fp32r) accumulating (numer | softmax-denominator) in PSUM.
Group graph replicated on every core. Identities used:
  exp(LeakyReLU(al+ar)) == max(exp(al)exp(ar), exp(.2al)exp(.2ar))
  segment softmax is shift-invariant (edge scores are O(10): no max needed)
  (A+I)[gidx] row gather folds the group-attention self term exactly.
"""
import sys

sys.path.insert(0, "/opt/trn_rl_repo")

import numpy as np

N, G = 50000, 1024
F_IN, HID, HEADS, NCLS = 128, 32, 4, 32
LN_EPS = 1e-5
NCORES = 8
NPER = N // NCORES            # 6250
NT = (NPER + 127) // 128      # 49 tiles/core
NPAD = NT * 128               # 6272 padded rows/core
NG = NCORES * NPAD            # 50176 padded global rows
SPLIT = 32768                 # int16 gather split (padded id space)
P = 128
SENT = 255.0                  # pad-edge dstlocal sentinel (mask never matches)
TAB1_COLS = 192               # [h(128) | u(4) | u2(4) | junk]  (768B rows)
TAB2_COLS = 64                # [h2(32) | u(1) | u2(1) | junk]  (256B rows)
VT_COLS = 64                  # [v(H) | v2(H) | junk]           (256B rows)
GCAP = 8                      # gather blocks (of 128 idxs) per dma_gather


# --------------------------------------------------------------------------
# host-side preprocessing
# --------------------------------------------------------------------------

def _wrap16(ix):
    """Compact dma_gather idx layout: [16, n/16]; idx i at [i%16, i//16].
    Replication to the 8 groups of 16 partitions happens on device."""
    ix = np.asarray(ix, np.int64)
    n = len(ix)
    assert n % 16 == 0, n
    return np.ascontiguousarray(ix.reshape(n // 16, 16).T.astype(np.int16))


def _segments(src, dst, ntile, split):
    """src already in padded-id space; dst in core-local [0, NPER)."""
    tile = dst // 128
    segs = []
    for t in range(ntile):
        m = tile == t
        s, d = src[m], dst[m] - t * 128
        if split:
            lo = s < SPLIT
            segs.append((s[lo], d[lo], s[~lo], d[~lo]))
        else:
            segs.append((s, d, s[:0], d[:0]))
    return segs


def _flatten(segs, nblk_lo, nblk_hi, ntile, dg_pad=0):
    nblk = nblk_lo + nblk_hi
    idx_lo, idx_hi, dmod, dglob = [], [], [], []
    for t in range(ntile):
        slo, dlo, shi, dhi = segs[t]
        a = np.zeros(nblk_lo * 128, np.int64); a[:len(slo)] = slo
        b = np.zeros(nblk_hi * 128, np.int64); b[:len(shi)] = shi - SPLIT
        dm = np.full(nblk * 128, SENT, np.float64)
        dm[:len(dlo)] = dlo
        dm[nblk_lo * 128:nblk_lo * 128 + len(dhi)] = dhi
        dg = np.full(nblk * 128, dg_pad, np.int64)
        dg[:len(dlo)] = dlo + t * 128
        dg[nblk_lo * 128:nblk_lo * 128 + len(dhi)] = dhi + t * 128
        idx_lo.append(a); idx_hi.append(b); dmod.append(dm); dglob.append(dg)
    idx_lo = np.concatenate(idx_lo) if nblk_lo else np.zeros(0, np.int64)
    idx_hi = np.concatenate(idx_hi) if nblk_hi else np.zeros(0, np.int64)
    dmod = np.concatenate(dmod)
    dglob = np.concatenate(dglob)
    # block layout [128, ntile*nblk]: column t*nblk+b holds block b's dstlocal
    dmod2 = np.ascontiguousarray(
        dmod.reshape(ntile * nblk, 128).T.astype(np.uint8))
    return idx_lo, idx_hi, dmod2, dglob


def _wext(W, a_src, a_dst, b, ncols):
    W = np.asarray(W, np.float32)
    a_src = np.asarray(a_src, np.float32)
    a_dst = np.asarray(a_dst, np.float32)
    b = np.asarray(b, np.float32)
    H, C = a_src.shape
    D = W.shape[1]
    asrc_m = np.zeros((D, H), np.float32)
    adst_m = np.zeros((D, H), np.float32)
    for h in range(H):
        asrc_m[h * C:(h + 1) * C, h] = a_src[h]
        adst_m[h * C:(h + 1) * C, h] = a_dst[h]
    Wx = np.concatenate([W, W @ asrc_m, W @ adst_m], axis=1)
    Wx = np.concatenate(
        [Wx, np.zeros((W.shape[0], ncols - Wx.shape[1]), np.float32)], axis=1)
    brow = np.concatenate([b, b @ asrc_m, b @ adst_m,
                           np.zeros(ncols - D - 2 * H, np.float32)])
    return np.ascontiguousarray(Wx), brow.astype(np.float32)


def host_prep(inputs):
    import ml_dtypes
    bf16 = ml_dtypes.bfloat16
    f32 = np.float32
    x1 = np.asarray(inputs["x1"], f32)
    ei1 = np.asarray(inputs["edge_index1"], np.int64)
    x2 = np.asarray(inputs["x2"], f32)
    ei2 = np.asarray(inputs["edge_index2"], np.int64)
    gidx = np.asarray(inputs["group_index"], np.int64)

    A = np.zeros((G, G), f32)
    u, v = ei2[0], ei2[1]
    np.add.at(A, (u, v), 1.0)
    np.add.at(A, (v, u), (u != v).astype(f32))
    Ap = A + np.eye(G, dtype=f32)
    assert Ap.max() < 256

    src_g, dst_g = ei1[0], ei1[1]
    # remap src node id into the padded-section id space (core*NPAD + local)
    pad_of = lambda ids: (ids // NPER) * NPAD + (ids % NPER)
    core_of = dst_g // NPER
    all_segs = []
    for c in range(NCORES):
        m = core_of == c
        loops = np.arange(c * NPER, (c + 1) * NPER, dtype=np.int64)
        s = pad_of(np.concatenate([src_g[m], loops]))
        d = np.concatenate([dst_g[m], loops]) - c * NPER
        all_segs.append(_segments(s, d, NT, True))
    nblk_lo = max(max((len(t[0]) + 127) // 128 for t in sg) for sg in all_segs)
    nblk_hi = max(max((len(t[2]) + 127) // 128 for t in sg) for sg in all_segs)

    loops2 = np.arange(G, dtype=np.int64)
    s2 = np.concatenate([ei2[0], loops2])
    d2 = np.concatenate([ei2[1], loops2])
    sm_segs = _segments(s2, d2, G // 128, False)
    nblk_sm = max((len(t[0]) + 127) // 128 for t in sm_segs)

    meta = dict(nblk_lo=nblk_lo, nblk_hi=nblk_hi, nblk=nblk_lo + nblk_hi,
                nblk_sm=nblk_sm)

    w1a, b1a = _wext(inputs["W1a"], inputs["a1a_src"], inputs["a1a_dst"],
                     inputs["b1a"], 256)
    w1b, b1b = _wext(inputs["W1b"], inputs["a1b_src"], inputs["a1b_dst"],
                     inputs["b1b"], 256)
    w2a, b2a = _wext(inputs["W2a"], inputs["a2a_src"], inputs["a2a_dst"],
                     inputs["b2a"], 64)
    w2b, b2b = _wext(inputs["W2b"], inputs["a2b_src"], inputs["a2b_dst"],
                     inputs["b2b"], 64)

    i_sm, _, dm_sm, dg_sm = _flatten(sm_segs, nblk_sm, 0, G // 128)

    # [b1a(0:256)|b1b(256:512)|b2a(512:576)|b2b(576:640)|
    #  ln1g(640:768)|ln1b(768:896)|ln2g(896:928)|ln2b(928:960)]
    rowcat = np.concatenate([
        b1a, b1b, b2a, b2b,
        np.asarray(inputs["ln1_g"], f32), np.asarray(inputs["ln1_b"], f32),
        np.asarray(inputs["ln2_g"], f32), np.asarray(inputs["ln2_b"], f32)])
    rowcat16 = np.ascontiguousarray(
        np.broadcast_to(rowcat[None, :], (16, rowcat.shape[0])))

    shared = dict(wext1a=np.asarray(w1a, bf16))
    # identical-on-every-core arrays are uploaded as 1/8-row shards and
    # AllGathered on device
    i_smw = _wrap16(i_sm)
    dg_smw = _wrap16(dg_sm)
    sh_slices = dict(wext1b=w1b, wext2a=w2a, wext2b=w2b, rowcat=rowcat16,
                     idx_smc=i_smw, dstl_sm=dm_sm, dglob_smc=dg_smw)

    per_core = []
    for c in range(NCORES):
        ilo, ihi, dmod, dglob = _flatten(all_segs[c], nblk_lo, nblk_hi, NT,
                                         dg_pad=NPAD)
        gown = np.concatenate([gidx[c * NPER:(c + 1) * NPER],
                               np.zeros(NPAD - NPER, np.int64)])
        x1sh = np.zeros((P, NPAD), f32)
        x1sh[:, :NPER] = x1[c * NPER:(c + 1) * NPER].T
        shsh = {k + "_sh": np.ascontiguousarray(
                    a[c * (a.shape[0] // 8):(c + 1) * (a.shape[0] // 8)])
                for k, a in sh_slices.items()}
        per_core.append(dict(
            **shsh,
            idx_loc=_wrap16(ilo), idx_hic=_wrap16(ihi),
            dglobc=_wrap16(dglob), gidxc=_wrap16(gown),
            x1Tsh=np.asarray(x1sh, bf16),
            x2Tsh=np.ascontiguousarray(x2[c * 128:(c + 1) * 128].T),
            Absh=np.asarray(Ap[c * 128:(c + 1) * 128], bf16),
        ))
    return shared, per_core, meta


# --------------------------------------------------------------------------
# device program
# --------------------------------------------------------------------------

def build_nc(meta):
    import contextlib
    from concourse import bacc, mybir
    from concourse.tile import TileContext
    from concourse.bass import ds, ts

    f32 = mybir.dt.float32
    f32r = mybir.dt.float32r
    bf16 = mybir.dt.bfloat16
    i16 = mybir.dt.int16
    i32 = mybir.dt.int32
    u8 = mybir.dt.uint8
    Alu = mybir.AluOpType
    Act = mybir.ActivationFunctionType
    Ax = mybir.AxisListType

    NBLK = meta["nblk"]
    NBLK_LO = meta["nblk_lo"]
    NBLK_HI = meta["nblk_hi"]
    NBLK_SM = meta["nblk_sm"]

    nc = bacc.Bacc(None, target_bir_lowering=False, debug=True)

    dp = lambda n, s, d: nc.declare_dram_parameter(n, list(s), d, isOutput=False)
    x1Tsh_d = dp("x1Tsh", [P, NPAD], bf16)
    x2Tsh_d = dp("x2Tsh", [P, P], f32r)
    Absh_d = dp("Absh", [P, G], bf16)
    wext1a_d = dp("wext1a", [P, 256], bf16)
    wext1b_sh_d = dp("wext1b_sh", [16, 256], f32r)
    wext2a_sh_d = dp("wext2a_sh", [16, 64], f32r)
    wext2b_sh_d = dp("wext2b_sh", [16, 64], f32r)
    rowcat_sh_d = dp("rowcat_sh", [2, 960], f32)
    idx_smc_sh_d = dp("idx_smc_sh", [2, 8 * NBLK_SM * 8], i16)
    dstl_sm_sh_d = dp("dstl_sm_sh", [16, 8 * NBLK_SM], u8)
    dglob_smc_sh_d = dp("dglob_smc_sh", [2, 8 * NBLK_SM * 8], i16)
    idx_loc_d = dp("idx_loc", [16, NT * NBLK_LO * 8], i16)
    idx_hic_d = dp("idx_hic", [16, NT * NBLK_HI * 8], i16)
    dglobc_d = dp("dglobc", [16, NT * NBLK * 8], i16)
    gidxc_d = dp("gidxc", [16, NT * 8], i16)

    out_d = nc.declare_dram_parameter("out", [NPAD, NCLS], bf16, isOutput=True)

    # AllGather-assembled full tensors (collectives cannot read IO tensors
    # directly, so shards are staged into internal DRAM first)
    x1st_d = nc.dram_tensor("x1st", [P, NPAD], bf16)
    x2st_d = nc.dram_tensor("x2st", [P, P], f32r)
    Abst_d = nc.dram_tensor("Abst", [P, G], bf16)
    packBst_d = nc.dram_tensor("packBst", [1, boff], u8)
    packBG_d = nc.dram_tensor("packBG", [8, boff], u8, addr_space="Shared")
    shfull = {}
    for nm, dt_, rr, cc in B_SPEC:
        shfull[nm] = nc.dram_tensor(nm + "_G", [rr, cc], dt_)
    x1TG_d = nc.dram_tensor("x1TG", [8 * P, NPAD], bf16, addr_space="Shared")
    x2TG_d = nc.dram_tensor("x2TG", [8 * P, P], f32r, addr_space="Shared")
    AbG_d = nc.dram_tensor("AbG", [G, G], bf16, addr_space="Shared")
    # full-layout (8x replicated) gather index tables, built on device
    idx_lo_d = nc.dram_tensor("idx_lo", [P, NT * NBLK_LO * 8], i16)
    idx_hi_d = nc.dram_tensor("idx_hi", [P, NT * NBLK_HI * 8], i16)
    dglob_d = nc.dram_tensor("dglob", [P, NT * NBLK * 8], i16)

    tab1_d = nc.dram_tensor("tab1", [NG, TAB1_COLS], f32)
    # one extra 128-row tile: row NPAD is the pad-slot target (zeroed)
    vtab1_d = nc.dram_tensor("vtab1", [NPAD + 128, VT_COLS], f32)
    smtab1_d = nc.dram_tensor("smtab1", [G, TAB1_COLS], f32)
    smvtab1_d = nc.dram_tensor("smvtab1", [G, VT_COLS], f32)
    tab2own_d = nc.dram_tensor("tab2own", [NPAD, TAB2_COLS], f32)
    tab2_d = nc.dram_tensor("tab2", [NG, TAB2_COLS], f32, addr_space="Shared")
    vtab2_d = nc.dram_tensor("vtab2", [NPAD + 128, VT_COLS], f32)
    smtab2_d = nc.dram_tensor("smtab2", [G, TAB2_COLS], f32)
    smvtab2_d = nc.dram_tensor("smvtab2", [G, VT_COLS], f32)

    with TileContext(nc) as tc, contextlib.ExitStack() as ctx:
        pool = ctx.enter_context(tc.tile_pool(name="main", bufs=2))
        cpool = ctx.enter_context(tc.tile_pool(name="consts", bufs=1))
        spool = ctx.enter_context(tc.tile_pool(name="stash", bufs=1))
        gpool = ctx.enter_context(tc.tile_pool(name="gather", bufs=1))
        qpool = ctx.enter_context(tc.tile_pool(name="q", bufs=1))
        ppool = ctx.enter_context(tc.tile_pool(name="psA", bufs=2, space="PSUM"))
        npool = ctx.enter_context(tc.tile_pool(name="psN", bufs=2, space="PSUM"))
        tpool = ctx.enter_context(tc.tile_pool(name="psT", bufs=2, space="PSUM"))
        spsum = ctx.enter_context(tc.tile_pool(name="psS", bufs=1, space="PSUM"))

        # ---- AllGathers: assemble full x1T / x2T / A on device ----
        nc.sync.dma_start(out=x1st_d[:], in_=x1Tsh_ap)
        nc.sync.dma_start(out=Abst_d[:], in_=Absh_ap)
        nc.sync.dma_start(out=x2st_d[:], in_=x2Tsh_ap)
        nc.gpsimd.collective_compute(
            "AllGather", Alu.bypass, replica_groups=[list(range(NCORES))],
            ins=[x1st_d[:]], outs=[x1TG_d[:]])
        nc.gpsimd.collective_compute(
            "AllGather", Alu.bypass, replica_groups=[list(range(NCORES))],
            ins=[Abst_d[:]], outs=[AbG_d[:]])
        nc.gpsimd.collective_compute(
            "AllGather", Alu.bypass, replica_groups=[list(range(NCORES))],
            ins=[x2st_d[:]], outs=[x2TG_d[:]])
        nc.sync.dma_start(out=packBst_d[:], in_=packB_d[:])
        nc.gpsimd.collective_compute(
            "AllGather", Alu.bypass, replica_groups=[list(range(NCORES))],
            ins=[packBst_d[:]], outs=[packBG_d[:]])
        for nm, dt_, rr, cc in B_SPEC:
            sz = rr * cc * mybir.dt.size(dt_) // 8
            o0 = B_OFF[nm]
            nc.sync.dma_start(
                out=shfull[nm][:].rearrange("(a r) c -> a (r c)", a=8),
                in_=packBG_d[:, o0:o0 + sz].bitcast(dt_))

        # ---- replicate compact idx tables to full 128-partition layout ----
        for g in range(8):
            nc.sync.dma_start(out=idx_lo_d[16 * g:16 * (g + 1), :],
                              in_=packA_d[:, A_LO:A_HI])
            nc.sync.dma_start(out=idx_hi_d[16 * g:16 * (g + 1), :],
                              in_=packA_d[:, A_HI:A_DG])
            nc.sync.dma_start(out=dglob_d[16 * g:16 * (g + 1), :],
                              in_=packA_d[:, A_DG:A_GI])

        def load_const(dram, shape, dtype, tag):
            t = cpool.tile(shape, dtype, tag=tag)
            nc.sync.dma_start(out=t[:], in_=dram[:])
            return t

        def load_rep16(dram, cols, dtype, tag):
            """[16, cols] DRAM -> [128, cols] SBUF, replicated 8x."""
            t = cpool.tile([P, cols], dtype, tag=tag)
            for g in range(8):
                nc.sync.dma_start(out=t[16 * g:16 * (g + 1), :], in_=dram[:])
            return t

        # iota row / per-partition index / identity, generated on device
        iotaI = cpool.tile([P, P], i32, tag="iotaI")
        nc.gpsimd.iota(iotaI[:], pattern=[[1, P]], base=0, channel_multiplier=0)
        iota_s = cpool.tile([P, P], f32, tag="iota")
        nc.vector.tensor_copy(out=iota_s[:], in_=iotaI[:])
        iotaPI = cpool.tile([P, 1], i32, tag="iotaPI")
        nc.gpsimd.iota(iotaPI[:], pattern=[[0, 1]], base=0, channel_multiplier=1)
        iotaP_s = cpool.tile([P, 1], f32, tag="iotaP")
        nc.vector.tensor_copy(out=iotaP_s[:], in_=iotaPI[:])
        ident_s = cpool.tile([P, P], f32, tag="ident")
        nc.vector.tensor_scalar(out=ident_s[:], in0=iota_s[:],
                                scalar1=iotaP_s[:, 0:1], scalar2=None,
                                op0=Alu.is_equal)

        wext1a_s = cpool.tile([P, 256], bf16, tag="wext1a")
        nc.sync.dma_start(out=wext1a_s[:], in_=wext1a_ap)
        wext1b_s = load_const(shfull["wext1b"], [P, 256], f32r, "wext1b")
        wext2a_s = load_const(shfull["wext2a"], [P, 64], f32r, "wext2a")
        wext2b_s = load_const(shfull["wext2b"], [P, 64], f32r, "wext2b")
        rc_s = load_rep16(shfull["rowcat"], 960, f32, "rowcat")
        brep1a_s = rc_s[:, 0:256]
        brep1b_s = rc_s[:, 256:512]
        brep2a_s = rc_s[:, 512:576]
        brep2b_s = rc_s[:, 576:640]
        g1rep_s = rc_s[:, 640:768]
        b1rep_s = rc_s[:, 768:896]
        g2rep_s = rc_s[:, 896:928]
        b2rep_s = rc_s[:, 928:960]

        idxsm_s = load_rep16(shfull["idx_smc"], 8 * NBLK_SM * 8, i16, "idxsm")
        dglobsm_s = load_rep16(shfull["dglob_smc"], 8 * NBLK_SM * 8, i16,
                               "dglobsm")
        gidx_s = cpool.tile([P, NT * 8], i16, tag="gidx")
        for g in range(8):
            nc.sync.dma_start(out=gidx_s[16 * g:16 * (g + 1), :],
                              in_=packA_d[:, A_GI:A_END])

        def load_u8_as_f32(dram, cols, tag):
            tb = pool.tile([P, cols], u8, tag=f"{tag}_u8")
            nc.sync.dma_start(out=tb[:], in_=dram[:])
            t = cpool.tile([P, cols], f32, tag=tag)
            nc.vector.tensor_copy(out=t[:], in_=tb[:])
            return t

        dstlsm_s = load_u8_as_f32(shfull["dstl_sm"], 8 * NBLK_SM, "dstlsm")
        
        # derive big-graph dstl from the wrapped dglob idx table:
        # block-layout [p, c] = wrap16-layout [p, 8c + p//16]
        dgw = cpool.tile([P, NT * NBLK * 8], i16, tag="dgw")
        nc.sync.dma_start(out=dgw[:], in_=dglob_d[:])
        dgv = dgw[:].rearrange("p (c e) -> p c e", e=8)
        dsti = cpool.tile([P, NT * NBLK], i16, tag="dsti")
        for g in range(8):
            nc.sync.dma_start(out=dsti[16 * g:16 * (g + 1), :],
                              in_=dgv[16 * g:16 * (g + 1), :, g])
        dstl_s = cpool.tile([P, NT * NBLK], f32, tag="dstl")
        nc.vector.tensor_copy(out=dstl_s[:], in_=dsti[:])
        # per-tile base offsets (t*128) for the in-loop subtract
        tbI = cpool.tile([P, NT], i32, tag="tbI")
        nc.gpsimd.iota(tbI[:], pattern=[[128, NT]], base=0,
                       channel_multiplier=0)
        tbase_s = cpool.tile([P, NT], f32, tag="tbase")
        nc.vector.tensor_copy(out=tbase_s[:], in_=tbI[:])
        # zero the vtab pad-slot tile (row NPAD target of dglob pads)
        zv = cpool.tile([P, VT_COLS], f32, tag="zv")
        nc.vector.memset(zv[:], 0.0)
        nc.sync.dma_start(out=vtab1_d[NPAD:NPAD + 128, :], in_=zv[:])
        nc.sync.dma_start(out=vtab2_d[NPAD:NPAD + 128, :], in_=zv[:])

        # pre-zero the q-slots so junk pad columns of the fp32r rhs are finite
        zq = qpool.tile([P, NBLK, 256], f32r, tag="q256")
        nc.vector.memset(zq[:].rearrange("p a b -> p (a b)").bitcast(f32), 0.0)
        zq = qpool.tile([P, max(NBLK, NBLK_SM), 40], f32r, tag="q33")
        nc.vector.memset(zq[:].rearrange("p a b -> p (a b)").bitcast(f32), 0.0)

        # ---------------- phase 1: tables ----------------
        # global tab1 (8 sections x 49 tiles) from AllGathered x1TG, bf16
        with tc.For_i(0, NT, 1) as t:
            for sec in range(NCORES):
                lhsb = pool.tile([P, 128], bf16, tag="tb_lhsb")
                nc.sync.dma_start(out=lhsb[:],
                                  in_=x1TG_d[sec * P:(sec + 1) * P,
                                             ts(t, 128)])
                ps = ppool.tile([P, 256], f32, tag="agg", space="PSUM")
                nc.tensor.matmul(out=ps[:], lhsT=lhsb[:], rhs=wext1a_s[:],
                                 start=True, stop=True)
                st = pool.tile([P, 256], f32, tag="tb_st")
                nc.vector.scalar_tensor_tensor(
                    out=st[:], in0=ps[:], scalar=1.0,
                    in1=brep1a_s[:], op0=Alu.bypass, op1=Alu.add)
                nc.scalar.activation(st[:, 132:136], st[:, 128:132],
                                     Act.Exp, scale=0.2)
                nc.scalar.activation(st[:, 128:132], st[:, 128:132], Act.Exp)
                nc.sync.dma_start(
                    out=tab1_d[ds(t * 128 + sec * NPAD, 128), :],
                    in_=st[:, 0:TAB1_COLS])

        # small-graph tables from AllGathered x2TG (sections == tiles)
        for t in range(8):
            lhs = pool.tile([P, 128], f32r, tag="sm_lhs")
            nc.sync.dma_start(out=lhs[:], in_=x2TG_d[t * P:(t + 1) * P, :])
            ps = ppool.tile([P, 256], f32, tag="agg", space="PSUM")
            nc.tensor.matmul(out=ps[:], lhsT=lhs[:], rhs=wext1b_s[:],
                             start=True, stop=True)
            st = pool.tile([P, 256], f32, tag="tb_st")
            nc.vector.scalar_tensor_tensor(
                out=st[:], in0=ps[:], scalar=1.0,
                in1=brep1b_s[:], op0=Alu.bypass, op1=Alu.add)
            vst = pool.tile([P, 8], f32, tag="vt_vst")
            nc.scalar.activation(vst[:, 0:4], st[:, 132:136], Act.Exp)
            nc.scalar.activation(vst[:, 4:8], st[:, 132:136], Act.Exp,
                                 scale=0.2)
            nc.sync.dma_start(out=smvtab1_d[t * 128:(t + 1) * 128, 0:8],
                              in_=vst[:])
            nc.scalar.activation(st[:, 132:136], st[:, 128:132],
                                 Act.Exp, scale=0.2)
            nc.scalar.activation(st[:, 128:132], st[:, 128:132], Act.Exp)
            nc.sync.dma_start(out=smtab1_d[t * 128:(t + 1) * 128, :],
                              in_=st[:, 0:TAB1_COLS])

        # ---------------- edge aggregation (loop body helper) ----------------
        def edge_gat_body(t, tab_dram, vtab_dram, idxlo_src, idxhi_src,
                          dstl_src, dglob_src, nblk, nblk_lo, F, H, rhs_n,
                          idx_in_sbuf, tag, tbase=None):
            """Emits ops for dst-tile t (loop var); returns agg psum
            [(numer F) | (s H)]."""
            tabcols = TAB1_COLS if F == 128 else TAB2_COLS
            gtag = f"g{tabcols}"
            qtag = "q256" if F == 128 else "q33"
            nblk_hi = nblk - nblk_lo
            gt = gpool.tile([P, nblk, tabcols], f32, tag=gtag)
            for g0 in range(0, nblk_lo, GCAP):
                g1 = min(g0 + GCAP, nblk_lo)
                if idx_in_sbuf:
                    iap = idxlo_src[:, ds(t * nblk_lo * 8 + g0 * 8,
                                          (g1 - g0) * 8)]
                else:
                    it = pool.tile([P, (g1 - g0) * 8], i16, tag=f"{tag}_il{g0}")
                    nc.sync.dma_start(
                        out=it[:], in_=idxlo_src[:, ds(t * nblk_lo * 8 + g0 * 8,
                                                       (g1 - g0) * 8)])
                    iap = it[:]
                nc.gpsimd.dma_gather(
                    out_ap=gt[:, g0:g1, :], in_ap=tab_dram[:],
                    idxs_ap=iap, num_idxs=(g1 - g0) * 128,
                    num_idxs_reg=(g1 - g0) * 128, elem_size=tabcols)
            for g0 in range(0, nblk_hi, GCAP):
                g1 = min(g0 + GCAP, nblk_hi)
                if idx_in_sbuf:
                    iap = idxhi_src[:, ds(t * nblk_hi * 8 + g0 * 8,
                                          (g1 - g0) * 8)]
                else:
                    it = pool.tile([P, (g1 - g0) * 8], i16, tag=f"{tag}_ih{g0}")
                    nc.sync.dma_start(
                        out=it[:], in_=idxhi_src[:, ds(t * nblk_hi * 8 + g0 * 8,
                                                       (g1 - g0) * 8)])
                    iap = it[:]
                nc.gpsimd.dma_gather(
                    out_ap=gt[:, nblk_lo + g0:nblk_lo + g1, :],
                    in_ap=tab_dram[SPLIT:, :],
                    idxs_ap=iap, num_idxs=(g1 - g0) * 128,
                    num_idxs_reg=(g1 - g0) * 128, elem_size=tabcols)
            vt = gpool.tile([P, nblk, VT_COLS], f32, tag="v64")
            for g0 in range(0, nblk, GCAP):
                g1 = min(g0 + GCAP, nblk)
                if idx_in_sbuf:
                    iap = dglob_src[:, ds(t * nblk * 8 + g0 * 8, (g1 - g0) * 8)]
                else:
                    it = pool.tile([P, (g1 - g0) * 8], i16, tag=f"{tag}_dg{g0}")
                    nc.sync.dma_start(
                        out=it[:], in_=dglob_src[:, ds(t * nblk * 8 + g0 * 8,
                                                       (g1 - g0) * 8)])
                    iap = it[:]
                nc.gpsimd.dma_gather(
                    out_ap=vt[:, g0:g1, :], in_ap=vtab_dram[:],
                    idxs_ap=iap, num_idxs=(g1 - g0) * 128,
                    num_idxs_reg=(g1 - g0) * 128, elem_size=VT_COLS)
            dl = pool.tile([P, nblk], f32, tag=f"{tag}_dl")
            if tbase is None:
                nc.vector.tensor_copy(out=dl[:], in_=dstl_src[:, ts(t, nblk)])
            else:
                tb = pool.tile([P, 1], f32, tag=f"{tag}_tb")
                nc.vector.tensor_copy(out=tb[:], in_=tbase[:, ds(t, 1)])
                nc.vector.tensor_scalar(
                    out=dl[:], in0=dstl_src[:, ts(t, nblk)], scalar1=tb[:, 0:1],
                    scalar2=None, op0=Alu.subtract)
            mask = qpool.tile([P, nblk, 128], f32r, tag="mask")
            nc.vector.tensor_tensor(
                out=mask[:],
                in0=iota_s[:][:, None, :].to_broadcast([P, nblk, 128]),
                in1=dl[:][:, :, None].to_broadcast([P, nblk, 128]),
                op=Alu.is_equal)
            q = qpool.tile([P, nblk, rhs_n], f32r, tag=qtag)
            m1 = pool.tile([P, nblk, H], f32, tag="pm1")
            m2 = pool.tile([P, nblk, H], f32, tag="pm2")
            nc.vector.tensor_tensor(out=m1[:], in0=gt[:, :, F:F + H],
                                    in1=vt[:, :, 0:H], op=Alu.mult)
            nc.vector.tensor_tensor(out=m2[:], in0=gt[:, :, F + H:F + 2 * H],
                                    in1=vt[:, :, H:2 * H], op=Alu.mult)
            nc.vector.tensor_tensor(out=q[:, :, F:F + H], in0=m1[:],
                                    in1=m2[:], op=Alu.max)
            C = F // H
            for h in range(H):
                nc.vector.tensor_tensor(
                    out=q[:, :, h * C:(h + 1) * C],
                    in0=gt[:, :, h * C:(h + 1) * C],
                    in1=q[:, :, F + h:F + h + 1].to_broadcast([P, nblk, C]),
                    op=Alu.mult)
            ps = ppool.tile([P, 256], f32, tag="agg", space="PSUM")
            for b in range(nblk):
                nc.tensor.matmul(
                    out=ps[:, 0:rhs_n], lhsT=mask[:, b, :], rhs=q[:, b, :],
                    start=(b == 0), stop=(b == nblk - 1))
            return ps

        def xout_from_ps(ps, F, H, brep_s, tag):
            rec = pool.tile([P, H], f32, tag=f"{tag}_rec")
            nc.vector.reciprocal(out=rec[:], in_=ps[:, F:F + H])
            xo = pool.tile([P, F], f32, tag=f"{tag}_xo")
            C = F // H
            for h in range(H):
                nc.vector.tensor_scalar(
                    out=xo[:, h * C:(h + 1) * C], in0=ps[:, h * C:(h + 1) * C],
                    scalar1=rec[:, h:h + 1], scalar2=None, op0=Alu.mult)
            nc.vector.tensor_tensor(out=xo[:], in0=xo[:], in1=brep_s[:, 0:F],
                                    op=Alu.add)
            return xo

        # ---------------- group attention (loop body helper) ----------------
        def group_attn(t, xo, X2pT_ap, X2ext_all, Fs, rhs_n, tag):
            """Returns 0.5*grp tile [P, Fs] f32."""
            pt = tpool.tile([P, 128], f32, tag="ptr", space="PSUM")
            nc.tensor.transpose(out=pt[:Fs, :], in_=xo[:, 0:Fs],
                                identity=ident_s[:])
            xT = pool.tile([P, 128], f32r, tag="ga_xT")
            nc.scalar.copy(out=xT[:Fs, :], in_=pt[:Fs, :])
            pss = spsum.tile([P, 1024], f32, tag="s", space="PSUM")
            nc.tensor.matmul(out=pss[:, 0:512], lhsT=xT[:Fs, :],
                             rhs=X2pT_ap[:, 0:512], start=True, stop=True)
            nc.tensor.matmul(out=pss[:, 512:1024], lhsT=xT[:Fs, :],
                             rhs=X2pT_ap[:, 512:1024], start=True, stop=True)
            mx0 = pool.tile([P, 1], f32, tag="ga_mx0")
            mx1 = pool.tile([P, 1], f32, tag="ga_mx1")
            nc.vector.reduce_max(mx0[:], pss[:, 0:512], axis=Ax.X)
            nc.vector.reduce_max(mx1[:], pss[:, 512:1024], axis=Ax.X)
            negmx = pool.tile([P, 1], f32, tag="ga_negmx")
            nc.vector.tensor_tensor(out=negmx[:], in0=mx0[:], in1=mx1[:],
                                    op=Alu.max)
            nc.vector.tensor_scalar(out=negmx[:], in0=negmx[:], scalar1=-1.0,
                                    scalar2=None, op0=Alu.mult)
            wx = pool.tile([P, G], f32, tag="ga_wx")
            nc.scalar.activation(wx[:, 0:512], pss[:, 0:512], Act.Exp,
                                 bias=negmx[:])
            nc.scalar.activation(wx[:, 512:1024], pss[:, 512:1024], Act.Exp,
                                 bias=negmx[:])
            at = pool.tile([P, 8, 128], bf16, tag="ga_at")
            nc.gpsimd.dma_gather(
                out_ap=at[:], in_ap=AbG_d[:], idxs_ap=gidx_s[:, ts(t, 8)],
                num_idxs=128, num_idxs_reg=128, elem_size=G, transpose=True)
            psn = npool.tile([P, 256], f32, tag="num", space="PSUM")
            for j in range(8):
                wt = tpool.tile([P, 128], f32, tag="ptr", space="PSUM")
                nc.tensor.transpose(out=wt[:], in_=wx[:, j * 128:(j + 1) * 128],
                                    identity=ident_s[:])
                bmt = pool.tile([P, 128], f32r, tag="ga_bmt")
                nc.vector.scalar_tensor_tensor(
                    out=bmt[:], in0=wt[:], scalar=1.0, in1=at[:, j, :],
                    op0=Alu.bypass, op1=Alu.mult)
                nc.tensor.matmul(out=psn[:, 0:rhs_n], lhsT=bmt[:],
                                 rhs=X2ext_all[:, j, :], start=(j == 0),
                                 stop=(j == 7))
            rec = pool.tile([P, 1], f32, tag="ga_grec")
            nc.vector.reciprocal(out=rec[:], in_=psn[:, Fs:Fs + 1])
            grp = pool.tile([P, Fs], f32, tag="ga_grp")
            nc.vector.tensor_scalar(out=grp[:], in0=psn[:, 0:Fs],
                                    scalar1=rec[:], scalar2=0.5, op0=Alu.mult,
                                    op1=Alu.mult)
            return grp

        # ================= small-graph GAT layer 1 =================
        xg1_all = spool.tile([P, 8, 128], f32, tag="xg1")
        X2pT = cpool.tile([P, G], f32r, tag="X2pT")
        X2ext_all = spool.tile([P, 8, 256], f32r, tag="X2ext")
        nc.vector.memset(
            X2ext_all[:].rearrange("p a b -> p (a b)").bitcast(f32), 0.0)
        nc.vector.memset(X2ext_all[:, :, 128:129].bitcast(f32), 1.0)
        with tc.For_i(0, 8, 1) as t:
            ps = edge_gat_body(
                t, smtab1_d, smvtab1_d, idxsm_s, None,
                dstlsm_s, dglobsm_s,
                NBLK_SM, NBLK_SM, 128, 4, 256, True, "sg1")
            xo = xout_from_ps(ps, 128, 4, brep1b_s, "sm1")
            nc.vector.tensor_copy(out=xg1_all[:, t, :], in_=xo[:])
            pt = tpool.tile([P, 128], f32, tag="ptr", space="PSUM")
            nc.tensor.transpose(out=pt[:], in_=xo[:], identity=ident_s[:])
            nc.scalar.copy(out=X2pT[:, ts(t, 128)], in_=pt[:])
            nc.scalar.copy(out=X2ext_all[:, t, 0:128], in_=xo[:])

        # ================= big-graph layer 1 =================
        var49 = cpool.tile([P, NT], f32, tag="var49")
        s1_all = spool.tile([P, NT, 128], f32, tag="s1")
        with tc.For_i(0, NT, 1) as t:
            # build this tile's v-table rows (dsts of tile t are within tile t)
            lhsb = pool.tile([P, 128], bf16, tag="vt_lhsb")
            nc.sync.dma_start(out=lhsb[:], in_=x1st_d[:, ts(t, 128)])
            psv = npool.tile([P, 256], f32, tag="num", space="PSUM")
            nc.tensor.matmul(out=psv[:, 0:8], lhsT=lhsb[:],
                             rhs=wext1a_s[:, 128:136], start=True, stop=True)
            stv = pool.tile([P, 8], f32, tag="vt_st")
            nc.vector.scalar_tensor_tensor(
                out=stv[:], in0=psv[:, 0:8], scalar=1.0,
                in1=rc_s[:, 128:136], op0=Alu.bypass, op1=Alu.add)
            vst = pool.tile([P, 8], f32, tag="vt_vst")
            nc.scalar.activation(vst[:, 0:4], stv[:, 4:8], Act.Exp)
            nc.scalar.activation(vst[:, 4:8], stv[:, 4:8], Act.Exp, scale=0.2)
            nc.sync.dma_start(out=vtab1_d[ts(t, 128), 0:8], in_=vst[:])
            ps = edge_gat_body(
                t, tab1_d, vtab1_d, idx_lo_d, idx_hi_d,
                dstl_s, dglob_d,
                NBLK, NBLK_LO, 128, 4, 256, False, "bg1", tbase=tbase_s)
            xo = xout_from_ps(ps, 128, 4, brep1a_s, "b1")
            grp = group_attn(t, xo, X2pT[:], X2ext_all, 128, 256, "g1")
            s1 = pool.tile([P, 128], f32, tag="b1_s1")
            nc.vector.scalar_tensor_tensor(out=s1[:], in0=xo[:], scalar=0.5,
                                           in1=grp[:], op0=Alu.mult, op1=Alu.add)
            mu = pool.tile([P, 1], f32, tag="b1_mu")
            nc.vector.tensor_reduce(out=mu[:], in_=s1[:], axis=Ax.X, op=Alu.add)
            nc.vector.tensor_scalar(out=mu[:], in0=mu[:], scalar1=-1.0 / 128,
                                    scalar2=None, op0=Alu.mult)
            nc.vector.tensor_scalar(out=s1[:], in0=s1[:], scalar1=mu[:],
                                    scalar2=None, op0=Alu.add)
            nc.vector.tensor_copy(out=s1_all[:, t, :], in_=s1[:])
            sq = pool.tile([P, 128], f32, tag="b1_sq")
            nc.vector.tensor_tensor(out=sq[:], in0=s1[:], in1=s1[:], op=Alu.mult)
            nc.vector.tensor_reduce(out=var49[:, ds(t, 1)], in_=sq[:], axis=Ax.X,
                                    op=Alu.add)

        sd49 = cpool.tile([P, NT], f32, tag="sd49")
        nc.vector.tensor_scalar(out=sd49[:], in0=var49[:], scalar1=1.0 / 128,
                                scalar2=LN_EPS, op0=Alu.mult, op1=Alu.add)
        sq49 = cpool.tile([P, NT], f32, tag="sq49")
        nc.scalar.activation(sq49[:], sd49[:], Act.Sqrt)
        rstd49 = cpool.tile([P, NT], f32, tag="rstd49")
        nc.vector.reciprocal(out=rstd49[:], in_=sq49[:])

        with tc.For_i(0, NT, 1) as t:
            s1 = pool.tile([P, 128], f32, tag="l1_s1")
            nc.vector.tensor_copy(out=s1[:], in_=s1_all[:, t, :])
            rs = pool.tile([P, 1], f32, tag="l1_rs")
            nc.vector.tensor_copy(out=rs[:], in_=rstd49[:, ds(t, 1)])
            y = pool.tile([P, 128], f32, tag="b1_y")
            nc.vector.scalar_tensor_tensor(
                out=y[:], in0=s1[:], scalar=rs[:], in1=g1rep_s[:],
                op0=Alu.mult, op1=Alu.mult)
            nc.vector.tensor_tensor(out=y[:], in0=y[:], in1=b1rep_s[:],
                                    op=Alu.add)
            emin = pool.tile([P, 128], f32, tag="b1_emin")
            nc.vector.tensor_scalar(out=emin[:], in0=y[:], scalar1=0.0,
                                    scalar2=None, op0=Alu.min)
            nc.scalar.activation(emin[:], emin[:], Act.Exp)
            h1 = pool.tile([P, 128], f32, tag="b1_h1")
            nc.vector.tensor_scalar(out=h1[:], in0=y[:], scalar1=0.0,
                                    scalar2=-1.0, op0=Alu.max, op1=Alu.add)
            nc.vector.tensor_tensor(out=h1[:], in0=h1[:], in1=emin[:], op=Alu.add)
            pt = tpool.tile([P, 128], f32, tag="ptr", space="PSUM")
            nc.tensor.transpose(out=pt[:], in_=h1[:], identity=ident_s[:])
            h1T = pool.tile([P, 128], f32r, tag="b1_h1T")
            nc.scalar.copy(out=h1T[:], in_=pt[:])
            ps2 = npool.tile([P, 256], f32, tag="num", space="PSUM")
            nc.tensor.matmul(out=ps2[:, 0:64], lhsT=h1T[:], rhs=wext2a_s[:],
                             start=True, stop=True)
            st2 = pool.tile([P, 64], f32, tag="b1_st2")
            nc.vector.scalar_tensor_tensor(
                out=st2[:], in0=ps2[:, 0:64], scalar=1.0, in1=brep2a_s[:],
                op0=Alu.bypass, op1=Alu.add)
            vst = pool.tile([P, 2], f32, tag="b1_vst")
            nc.scalar.activation(vst[:, 0:1], st2[:, 33:34], Act.Exp)
            nc.scalar.activation(vst[:, 1:2], st2[:, 33:34], Act.Exp, scale=0.2)
            nc.scalar.activation(st2[:, 33:34], st2[:, 32:33], Act.Exp, scale=0.2)
            nc.scalar.activation(st2[:, 32:33], st2[:, 32:33], Act.Exp)
            nc.sync.dma_start(out=tab2own_d[ts(t, 128), :], in_=st2[:])
            nc.sync.dma_start(out=vtab2_d[ts(t, 128), 0:2], in_=vst[:, 0:2])

        nc.gpsimd.collective_compute(
            "AllGather", Alu.bypass, replica_groups=[list(range(NCORES))],
            ins=[tab2own_d[:]], outs=[tab2_d[:]])

        # ================= small-graph layer 2 =================
        with tc.For_i(0, 8, 1) as t:
            xg = pool.tile([P, 128], f32, tag="ts2_xg")
            nc.vector.tensor_copy(out=xg[:], in_=xg1_all[:, t, :])
            pt = tpool.tile([P, 128], f32, tag="ptr", space="PSUM")
            nc.tensor.transpose(out=pt[:], in_=xg[:], identity=ident_s[:])
            xT = pool.tile([P, 128], f32r, tag="ts2_xT")
            nc.scalar.copy(out=xT[:], in_=pt[:])
            ps2 = npool.tile([P, 256], f32, tag="num", space="PSUM")
            nc.tensor.matmul(out=ps2[:, 0:64], lhsT=xT[:], rhs=wext2b_s[:],
                             start=True, stop=True)
            st2 = pool.tile([P, 64], f32, tag="ts2_st")
            nc.vector.scalar_tensor_tensor(
                out=st2[:], in0=ps2[:, 0:64], scalar=1.0, in1=brep2b_s[:],
                op0=Alu.bypass, op1=Alu.add)
            vst = pool.tile([P, 2], f32, tag="ts2_vst")
            nc.scalar.activation(vst[:, 0:1], st2[:, 33:34], Act.Exp)
            nc.scalar.activation(vst[:, 1:2], st2[:, 33:34], Act.Exp, scale=0.2)
            nc.scalar.activation(st2[:, 33:34], st2[:, 32:33], Act.Exp, scale=0.2)
            nc.scalar.activation(st2[:, 32:33], st2[:, 32:33], Act.Exp)
            nc.sync.dma_start(out=smtab2_d[ts(t, 128), :], in_=st2[:])
            nc.sync.dma_start(out=smvtab2_d[ts(t, 128), 0:2], in_=vst[:, 0:2])

        xg2_all = spool.tile([P, 8, 32], f32, tag="xg2")
        X2p2T = cpool.tile([32, G], f32r, tag="X2p2T")
        X2ext2_all = spool.tile([P, 8, 40], f32r, tag="X2ext2")
        nc.vector.memset(
            X2ext2_all[:].rearrange("p a b -> p (a b)").bitcast(f32), 0.0)
        nc.vector.memset(X2ext2_all[:, :, 32:33].bitcast(f32), 1.0)
        with tc.For_i(0, 8, 1) as t:
            ps = edge_gat_body(
                t, smtab2_d, smvtab2_d, idxsm_s, None,
                dstlsm_s, dglobsm_s,
                NBLK_SM, NBLK_SM, 32, 1, 40, True, "sg2")
            xo = xout_from_ps(ps, 32, 1, brep2b_s, "sm2")
            nc.vector.tensor_copy(out=xg2_all[:, t, :], in_=xo[:])
            pt = tpool.tile([P, 128], f32, tag="ptr", space="PSUM")
            nc.tensor.transpose(out=pt[:32, :], in_=xo[:], identity=ident_s[:])
            nc.scalar.copy(out=X2p2T[:, ts(t, 128)], in_=pt[:32, :])
            nc.scalar.copy(out=X2ext2_all[:, t, 0:32], in_=xo[:])

        # ================= big-graph layer 2 =================
        var49b = cpool.tile([P, NT], f32, tag="var49b")
        o_all = spool.tile([P, NT, 32], f32, tag="o")
        with tc.For_i(0, NT, 1) as t:
            ps = edge_gat_body(
                t, tab2_d, vtab2_d, idx_lo_d, idx_hi_d,
                dstl_s, dglob_d,
                NBLK, NBLK_LO, 32, 1, 40, False, "bg2", tbase=tbase_s)
            xo = xout_from_ps(ps, 32, 1, brep2a_s, "b2")
            grp = group_attn(t, xo, X2p2T[:], X2ext2_all, 32, 40, "g2")
            o = pool.tile([P, 32], f32, tag="b2_o")
            nc.vector.scalar_tensor_tensor(out=o[:], in0=xo[:], scalar=0.5,
                                           in1=grp[:], op0=Alu.mult, op1=Alu.add)
            mu = pool.tile([P, 1], f32, tag="b2_mu")
            nc.vector.tensor_reduce(out=mu[:], in_=o[:], axis=Ax.X, op=Alu.add)
            nc.vector.tensor_scalar(out=mu[:], in0=mu[:], scalar1=-1.0 / 32,
                                    scalar2=None, op0=Alu.mult)
            nc.vector.tensor_scalar(out=o[:], in0=o[:], scalar1=mu[:],
                                    scalar2=None, op0=Alu.add)
            nc.vector.tensor_copy(out=o_all[:, t, :], in_=o[:])
            sq = pool.tile([P, 32], f32, tag="b2_sq")
            nc.vector.tensor_tensor(out=sq[:], in0=o[:], in1=o[:], op=Alu.mult)
            nc.vector.tensor_reduce(out=var49b[:, ds(t, 1)], in_=sq[:],
                                    axis=Ax.X, op=Alu.add)

        sd49b = cpool.tile([P, NT], f32, tag="sd49b")
        nc.vector.tensor_scalar(out=sd49b[:], in0=var49b[:], scalar1=1.0 / 32,
                                scalar2=LN_EPS, op0=Alu.mult, op1=Alu.add)
        sq49b = cpool.tile([P, NT], f32, tag="sq49b")
        nc.scalar.activation(sq49b[:], sd49b[:], Act.Sqrt)
        rstd49b = cpool.tile([P, NT], f32, tag="rstd49b")
        nc.vector.reciprocal(out=rstd49b[:], in_=sq49b[:])

        with tc.For_i(0, NT, 1) as t:
            o = pool.tile([P, 32], f32, tag="l2_o")
            nc.vector.tensor_copy(out=o[:], in_=o_all[:, t, :])
            rs = pool.tile([P, 1], f32, tag="l2_rs")
            nc.vector.tensor_copy(out=rs[:], in_=rstd49b[:, ds(t, 1)])
            y = pool.tile([P, 32], f32, tag="b2_y")
            nc.vector.scalar_tensor_tensor(
                out=y[:], in0=o[:], scalar=rs[:], in1=g2rep_s[:],
                op0=Alu.mult, op1=Alu.mult)
            nc.vector.tensor_tensor(out=y[:], in0=y[:], in1=b2rep_s[:],
                                    op=Alu.add)
            yb = pool.tile([P, 32], bf16, tag="b2_yb")
            nc.vector.tensor_copy(out=yb[:], in_=y[:])
            nc.sync.dma_start(out=out_d[ts(t, 128), :], in_=yb[:])

    nc.compile()
    return nc


# --------------------------------------------------------------------------
# entry point
# --------------------------------------------------------------------------

def kernel(**inputs):
    from concourse.bass_utils import run_bass_kernel_spmd

    shared, per_core, meta = host_prep(inputs)
    nc = build_nc(meta)
    in_maps = []
    for c in range(NCORES):
        m = dict(shared)
        m.update(per_core[c])
        in_maps.append(m)
    res = run_bass_kernel_spmd(nc, in_maps, list(range(NCORES)))
    out = np.concatenate([np.asarray(res.results[c]["out"])[:NPER]
                          for c in range(NCORES)])
    return out.astype(np.float32)
